# revision 37
# baseline (speedup 1.0000x reference)
"""Trainium2 Bass kernel for nn_AberrationCorrectionModule.

Reference pipeline:
  1. psf_predictor: 3x conv3x3 (128->256->128->900) on aberration_features,
     softmax over 225 taps per channel -> psf
  2. deconv: 15x15 spatially-varying weighted sum over reflect-padded raw
  3. freq corrector: rfft2 -> conv3x3 stack (8->64->64->8) -> irfft2, added
  4. per-channel refinement: 4 independent 1->16->16->1 conv stacks
  5. out = clip(raw + corrected, 0, 1)

Distribution: 8 NeuronCores, H-sharded (32 rows/core), SPMD dispatches with
host gather between (FFT stage needs full-image mixing).
"""
import json
import sys

sys.path.insert(0, "/opt/trn_rl_repo")

import ml_dtypes
import numpy as np

import bass_rust
import concourse.bass as bass
import concourse.tile as tile
from concourse import mybir
from concourse.bass_utils import run_bass_kernel_spmd

F32 = mybir.dt.float32
BF16 = mybir.dt.bfloat16
AF = mybir.ActivationFunctionType
ALU = mybir.AluOpType
AX = mybir.AxisListType

N_CORES = 8
C, H, W = 4, 256, 256
ROWS = H // N_CORES  # 32
KK = 15
PAD = KK // 2  # 7
WP = W + 2  # 258
TAPS = [(dy, dx) for dy in (-1, 0, 1) for dx in (-1, 0, 1)]


def _bf(x):
    return np.asarray(x, dtype=ml_dtypes.bfloat16)


def mkap(base_ap, offset, pairs):
    a = base_ap.copy()
    a.offset = offset
    a.ap = bass_rust.VecI64Pair([list(p) for p in pairs])
    return a


def _split_multiwaits(raw: bytes) -> bytes:
    """Workaround: this walrus build rejects >1 sync wait per instruction.
    Move extra waits onto NoOp carriers inserted just before the instruction."""
    m = json.loads(raw)
    ctr = 0
    for fn in m["functions"]:
        for bb in fn.get("blocks", []):
            insts = bb.get("instructions")
            if not insts:
                continue
            out = []
            for inst in insts:
                si = inst.get("sync_info")
                ow = (si or {}).get("on_wait") or []
                if len(ow) > 1:
                    for w in ow[:-1]:
                        out.append({
                            "debug": inst.get("debug", 0),
                            "engine": inst["engine"],
                            "ins": [], "outs": [],
                            "name": f"wsplit_{ctr}",
                            "opcode": "NoOp",
                            "sync_info": {"on_update": [], "on_wait": [w]},
                        })
                        ctr += 1
                    si["on_wait"] = [ow[-1]]
                out.append(inst)
            bb["instructions"] = out
    return json.dumps(m).encode()


def patch_nc(nc):
    orig = nc.to_json_bytes
    nc.to_json_bytes = lambda: _split_multiwaits(orig())
    return nc


def nchunks(total, step):
    out, o = [], 0
    while o < total:
        out.append((o, min(step, total - o)))
        o += step
    return out


# ================================================================ kernel A
# fp8 DoubleRow rewrite.
# conv1/conv2 feature-major on a 272-pitch grid (row pitch % 16 == 0 for
# DoubleRow lhsT k-tile strides). conv3 transposed: pixels on partitions,
# psf taps on the free axis (4ch x 228, 912 cols), softmax tail on
# vector/scalar engines. Patches pre-unfolded on host to [8192, 912].

RP = 272            # row pitch
EXTF = 38 * RP      # fb copy extent
EXTH1 = 36 * RP     # h1 half extent
EXTH2 = 34 * RP     # h2 copy extent
GD = 16             # leading guard cols
F8 = mybir.dt.float8e4
DR = mybir.MatmulPerfMode.DoubleRow
# conv tap pairs: 3 horizontal A/B-copy pairs, 1 vertical, 1 zero-padded
# (dy, dx) of kt0; kind 'AB' = kt1 from shifted copy (stride EXT),
# 'V' = kt1 one row down (stride RP), 'Z' = kt1 zero weights (stride RP)
PAIRS = [((-1, -1), 'AB'), ((0, -1), 'AB'), ((1, -1), 'AB'),
         ((-1, 1), 'V'), ((1, 1), 'Z')]


def pair_taps(p):
    """taps (as (dy,dx)) covered by pair p: (kt0, kt1 or None)."""
    (dy, dx), kind = PAIRS[p]
    if kind == 'AB':
        return (dy, dx), (dy, dx + 1)
    if kind == 'V':
        return (dy, dx), (dy + 1, dx)
    return (dy, dx), None


def build_A():
    nc = bass.Bass(trn_type="TRN2", name="kernA")
    fb = nc.dram_tensor("fb", (128, GD + 2 * EXTF), F8, kind="ExternalInput")
    w1 = nc.dram_tensor("w1", (128, 2 * 5 * 2 * 128), F8, kind="ExternalInput")
    b1 = nc.dram_tensor("b1", (128, 2), F32, kind="ExternalInput")
    w2 = nc.dram_tensor("w2", (128, 9 * 2 * 128), F8, kind="ExternalInput")
    b2 = nc.dram_tensor("b2", (128, 1), F32, kind="ExternalInput")
    w3 = nc.dram_tensor("w3", (128, 5 * 2 * 912), F8, kind="ExternalInput")
    xu = nc.dram_tensor("xu", (8192, 912), BF16, kind="ExternalInput")
    m36 = nc.dram_tensor("m36", (128, 36), F32, kind="ExternalInput")
    m34 = nc.dram_tensor("m34", (128, 34), F32, kind="ExternalInput")
    corr = nc.dram_tensor("corr", (64, 512), F32, kind="ExternalOutput")

    def win(tile_ap, off, stride, nl):
        return mkap_s(tile_ap, off, [[tile_ap.ap[0][0], 128], [stride, 2],
                                     [1, nl]])

    with tile.TileContext(nc) as tc:
        with tc.tile_pool(name="cst", bufs=1) as cst, \
             tc.tile_pool(name="hp", bufs=1) as hp, \
             tc.tile_pool(name="psum", bufs=2, space="PSUM") as psp:
            w3t = cst.tile([128, 5 * 2 * 912], F8)
            nc.sync.dma_start(w3t[:], w3[:])

            w2t = cst.tile([128, 9 * 2 * 128], F8)
            nc.sync.dma_start(w2t[:], w2[:])
            b2t = cst.tile([128, 1], F32)
            nc.sync.dma_start(b2t[:], b2[:])
            m34t = cst.tile([128, 34], F32)
            nc.sync.dma_start(m34t[:], m34[:])

            h2 = hp.tile([128, GD + 2 * EXTH2 + 144], F8)

            with tc.tile_pool(name="h1p", bufs=1) as h1p:
                h1 = h1p.tile([128, GD + 2 * EXTH1 + GD], F8)
                with tc.tile_pool(name="fp", bufs=1) as fp:
                    w1t = fp.tile([128, 2 * 5 * 2 * 128], F8)
                    nc.sync.dma_start(w1t[:], w1[:])
                    b1t = fp.tile([128, 2], F32)
                    nc.sync.dma_start(b1t[:], b1[:])
                    m36t = fp.tile([128, 36], F32)
                    nc.sync.dma_start(m36t[:], m36[:])
                    fbt = fp.tile([128, GD + 2 * EXTF], F8)
                    FB1 = GD + 13 * RP
                    FB2 = GD + 26 * RP
                    nc.sync.dma_start(fbt[:, :FB1], fb[:, :FB1])
                    nc.sync.dma_start(fbt[:, FB1:FB2], fb[:, FB1:FB2])
                    nc.sync.dma_start(fbt[:, FB2:], fb[:, FB2:])
                    w1v = w1t[:].rearrange("q (m p k c) -> q m p k c", m=2,
                                           p=5, k=2)

                    # conv1: 128 -> 256 (2 M halves), 5 DoubleRow passes
                    for m in range(2):
                        for n0, nl in nchunks(EXTH1, 512):
                            ps = psp.tile([128, 512], F32, tag="cv",
                                          name=f"c1_{m}_{n0}", bufs=2)
                            for p, ((dy, dx), kind) in enumerate(PAIRS):
                                off = GD + n0 + (1 + dy) * RP + dx
                                st = EXTF if kind == 'AB' else RP
                                nc.tensor.matmul(
                                    ps[:, :nl], lhsT=w1v[:, m, p, :, :],
                                    rhs=win(fbt[:], off, st, nl),
                                    start=(p == 0), stop=(p == 4),
                                    perf_mode=DR)
                            nc.scalar.activation(
                                h1[:, GD + m * EXTH1 + n0:
                                   GD + m * EXTH1 + n0 + nl],
                                ps[:, :nl], AF.Relu, bias=b1t[:, m:m + 1])
                    for m in range(2):
                        h3 = h1[:, GD + m * EXTH1:GD + (m + 1) * EXTH1] \
                            .rearrange("q (r c) -> q r c", r=36)
                        nc.vector.memset(h3[:, :, 0:1], 0.0)
                        nc.vector.memset(h3[:, :, 257:258], 0.0)
                        for r in (0, 1, 34, 35):
                            nc.vector.tensor_scalar_mul(
                                h3[:, r, :], h3[:, r, :], m36t[:, r:r + 1])

                # conv2: 256 -> 128, 9 DoubleRow passes over kc halves
                w2v = w2t[:].rearrange("q (t k c) -> q t k c", t=9, k=2)
                for n0, nl in nchunks(EXTH2, 512):
                    ps = psp.tile([128, 512], F32, tag="cv",
                                  name=f"c2_{n0}", bufs=2)
                    for t, (dy, dx) in enumerate(TAPS):
                        off = GD + n0 + (1 + dy) * RP + dx
                        nc.tensor.matmul(
                            ps[:, :nl], lhsT=w2v[:, t, :, :],
                            rhs=win(h1[:], off, EXTH1, nl),
                            start=(t == 0), stop=(t == 8), perf_mode=DR)
                    nc.scalar.activation(
                        h2[:, GD + n0:GD + n0 + nl], ps[:, :nl], AF.Relu,
                        bias=b2t[:])
                h23 = h2[:, GD:GD + EXTH2].rearrange("q (r c) -> q r c", r=34)
                nc.vector.memset(h23[:, :, 0:1], 0.0)
                nc.vector.memset(h23[:, :, 257:258], 0.0)
                for r in (0, 33):
                    nc.vector.tensor_scalar_mul(
                        h23[:, r, :], h23[:, r, :], m34t[:, r:r + 1])

            # shifted copy for conv3 lhsT k-tile pairing (copy1[x]=copy0[x+1])
            BND = 9 * RP
            for bb in range(4):
                a0 = bb * BND
                a1 = min(EXTH2 - 1, a0 + BND)
                nc.sync.dma_start(h2[:, GD + EXTH2 + a0:GD + EXTH2 + a1],
                                  h2[:, GD + 1 + a0:GD + 1 + a1])
            # ones region for the bias k-tile of conv3 pass 4
            OB = GD + 2 * EXTH2 + 2
            nc.vector.memset(h2[:, OB:OB + 128], 1.0)

            # conv3 transposed + softmax tail, per 128-pixel group.
            # bias lands in psum via a K=1 ones-matmul; exp(b3) is folded
            # into xu on host; D comes free from exp accum_out. Division
            # and output DMA are batched over 8 groups.
            w3v = w3t[:].rearrange("q (p k c) -> q p k c", p=5, k=2)
            GB = 8
            with tc.tile_pool(name="gp", bufs=4) as gp, \
                 tc.tile_pool(name="bp", bufs=2) as bp:
                for g in range(64):
                    r, cc = g // 2, g % 2
                    gi = g % GB
                    if gi == 0:
                        Ns = bp.tile([128, GB * 4], F32, tag="Ns",
                                     name=f"Ns{g}")
                        Ds = bp.tile([128, GB * 4], F32, tag="Ds",
                                     name=f"Ds{g}")
                    Xg = gp.tile([128, 912], BF16, tag="Xg", name=f"Xg{g}")
                    xq = nc.sync if g % 2 == 0 else nc.gpsimd
                    xq.dma_start(Xg[:], xu[g * 128:(g + 1) * 128, :])
                    pss = [psp.tile([128, 456], F32, tag=f"c3{j}",
                                    name=f"c3_{g}_{j}", bufs=3)
                           for j in range(2)]
                    for p, ((dy, dx), kind) in enumerate(PAIRS):
                        off = GD + (r + 1 + dy) * RP + cc * 128 + 1 + dx
                        if kind == 'AB':
                            st = EXTH2
                        elif kind == 'V':
                            st = RP
                        else:  # Z: kt1 = ones region (bias via w3 row 0)
                            st = OB - off
                        for j in range(2):
                            nc.tensor.matmul(
                                pss[j][:],
                                lhsT=win(h2[:], off, st, 128),
                                rhs=w3v[:, p, :, j * 456:(j + 1) * 456],
                                start=(p == 0), stop=(p == 4), perf_mode=DR)
                    E = gp.tile([128, 912], BF16, tag="E", name=f"E{g}")
                    for c in range(4):
                        nc.scalar.activation(
                            E[:, c * 228:(c + 1) * 228],
                            pss[c // 2][:, (c % 2) * 228:(c % 2) * 228 + 228],
                            AF.Exp, accum_out=Ds[:, gi * 4 + c:gi * 4 + c + 1])
                    Pt = gp.tile([128, 912], BF16, tag="Pt", name=f"Pt{g}")
                    nc.gpsimd.tensor_tensor(out=Pt[:], in0=E[:], in1=Xg[:],
                                            op=ALU.mult)
                    nc.vector.tensor_reduce(
                        Ns[:, gi * 4:gi * 4 + 4],
                        Pt[:].rearrange("q (a b) -> q a b", a=4),
                        AX.X, ALU.add)
                    if gi == GB - 1:
                        nc.vector.reciprocal(Ds[:], Ds[:])
                        nc.vector.tensor_tensor(out=Ns[:], in0=Ns[:],
                                                in1=Ds[:], op=ALU.mult)
                        nc.sync.dma_start(
                            mkap(corr[:], (g - GB + 1) * 512,
                                 [[1, 128], [512, GB], [128, 4]]), Ns[:])
    return nc


def mkap_s(base_ap, off, pairs):
    a = base_ap.copy()
    a.offset = base_ap.offset + off
    a.ap = bass_rust.VecI64Pair([list(p) for p in pairs])
    return a


def build_A_old():
    nc = bass.Bass(trn_type="TRN2", name="kernA")
    feat = nc.dram_tensor("feat", (128, 38 * 256), F32, kind="ExternalInput")
    raw46 = nc.dram_tensor("raw46", (C, 46, 270), BF16, kind="ExternalInput")
    w1 = nc.dram_tensor("w1", (128, 2 * 9 * 128), BF16, kind="ExternalInput")
    b1 = nc.dram_tensor("b1", (128, 2), F32, kind="ExternalInput")
    w2 = nc.dram_tensor("w2", (128, 2 * 9 * 128), BF16, kind="ExternalInput")
    b2 = nc.dram_tensor("b2", (128, 1), F32, kind="ExternalInput")
    w3 = nc.dram_tensor("w3", (128, 9 * 1024), BF16, kind="ExternalInput")
    b3 = nc.dram_tensor("b3", (128, 8), F32, kind="ExternalInput")
    m36 = nc.dram_tensor("m36", (128, 36), F32, kind="ExternalInput")
    m34 = nc.dram_tensor("m34", (128, 34), F32, kind="ExternalInput")
    corr = nc.dram_tensor("corr", (C, ROWS, W), F32, kind="ExternalOutput")

    NF36, NF34 = 36 * WP, 34 * WP

    with tile.TileContext(nc) as tc:
        with tc.tile_pool(name="cst", bufs=1) as cst, \
             tc.tile_pool(name="hp", bufs=1) as hp, \
             tc.tile_pool(name="psum", bufs=2, space="PSUM") as psp:
            w3t = cst.tile([128, 9 * 1024], BF16)
            nc.sync.dma_start(w3t[:], w3[:])
            b3t = cst.tile([128, 8], F32)
            nc.sync.dma_start(b3t[:], b3[:])
            b2t = cst.tile([128, 1], F32)
            nc.sync.dma_start(b2t[:], b2[:])
            m34t = cst.tile([128, 34], F32)
            nc.sync.dma_start(m34t[:], m34[:])
            ones = cst.tile([128, 1], BF16)
            nc.vector.memset(ones[:], 1.0)

            h2 = hp.tile([128, NF34], BF16)

            with tc.tile_pool(name="h1p", bufs=1) as h1p:
                h1 = [h1p.tile([128, NF36 + 8], BF16, name=f"h1_{m}", tag=f"h1_{m}") for m in range(2)]
                w2t = h1p.tile([128, 2 * 9 * 128], BF16)
                nc.sync.dma_start(w2t[:], w2[:])

                with tc.tile_pool(name="fp", bufs=1) as fp:
                    w1t = fp.tile([128, 2 * 9 * 128], BF16)
                    nc.sync.dma_start(w1t[:], w1[:])
                    b1t = fp.tile([128, 2], F32)
                    nc.sync.dma_start(b1t[:], b1[:])
                    m36t = fp.tile([128, 36], F32)
                    nc.sync.dma_start(m36t[:], m36[:])
                    ff = fp.tile([128, 38 * 256], F32)
                    nc.sync.dma_start(ff[:], feat[:])
                    fb = fp.tile([128, 38 * WP + 8], BF16)
                    nc.vector.memset(fb[:], 0.0)
                    nc.vector.tensor_copy(
                        fb[:, 1:1 + 38 * WP].rearrange(
                            "p (r c) -> p r c", r=38)[:, :, 1:257],
                        ff[:].rearrange("p (r c) -> p r c", r=38))

                    # conv1: 128 -> 256 (2 M chunks), taps-outer groups of 3
                    for m in range(2):
                        ch1 = nchunks(NF36, 512)
                        for g0 in range(0, len(ch1), 3):
                            grp = ch1[g0:g0 + 3]
                            pcs = [psp.tile([128, 512], F32, tag=f"pc{j}",
                                            name=f"c1_{m}_{g0}_{j}", bufs=1)
                                   for j in range(len(grp))]
                            for t, (dy, dx) in enumerate(TAPS):
                                base = (1 + dy) * WP + dx
                                for j, (n0, nl) in enumerate(grp):
                                    nc.tensor.matmul(
                                        pcs[j][:, :nl],
                                        lhsT=w1t[:, (m * 9 + t) * 128:(m * 9 + t + 1) * 128],
                                        rhs=fb[:, 1 + n0 + base:1 + n0 + base + nl],
                                        start=(t == 0), stop=(t == 8))
                            for j, (n0, nl) in enumerate(grp):
                                nc.scalar.activation(
                                    h1[m][:, 1 + n0:1 + n0 + nl], pcs[j][:, :nl],
                                    AF.Relu, bias=b1t[:, m:m + 1])
                        h3 = h1[m][:, 1:1 + NF36].rearrange("p (r c) -> p r c", r=36)
                        nc.vector.memset(h3[:, :, 0:1], 0.0)
                        nc.vector.memset(h3[:, :, 257:258], 0.0)
                        # zero out-of-image rows (only rows 0,1,34,35 can be OOI)
                        for r in (0, 1, 34, 35):
                            nc.vector.tensor_scalar_mul(
                                h3[:, r, :], h3[:, r, :], m36t[:, r:r + 1])

                # conv2: 256 -> 128 (2 K chunks), taps-outer groups of 3
                ch2 = nchunks(NF34, 512)
                for g0 in range(0, len(ch2), 3):
                    grp = ch2[g0:g0 + 3]
                    pcs = [psp.tile([128, 512], F32, tag=f"pc{j}",
                                    name=f"c2_{g0}_{j}", bufs=1)
                           for j in range(len(grp))]
                    ti = 0
                    for kc in range(2):
                        for t, (dy, dx) in enumerate(TAPS):
                            base = (1 + dy) * WP + dx
                            for j, (n0, nl) in enumerate(grp):
                                nc.tensor.matmul(
                                    pcs[j][:, :nl],
                                    lhsT=w2t[:, (kc * 9 + t) * 128:(kc * 9 + t + 1) * 128],
                                    rhs=h1[kc][:, 1 + n0 + base:1 + n0 + base + nl],
                                    start=(ti == 0), stop=(ti == 17))
                            ti += 1
                    for j, (n0, nl) in enumerate(grp):
                        nc.scalar.activation(
                            h2[:, n0:n0 + nl], pcs[j][:, :nl], AF.Relu, bias=b2t[:])
                h23 = h2[:].rearrange("p (r c) -> p r c", r=34)
                nc.vector.memset(h23[:, :, 0:1], 0.0)
                nc.vector.memset(h23[:, :, 257:258], 0.0)
                for r in (0, 33):
                    nc.vector.tensor_scalar_mul(
                        h23[:, r, :], h23[:, r, :], m34t[:, r:r + 1])

            # conv3 + softmax + deconv per (pixchunk, channel).
            # psf channels padded 900->1024: image channel c = M-chunks
            # {2c, 2c+1}; taps 0..224 real, 225..255 padded (bias -30).
            RPC = 8
            PCN = RPC * W  # 2048
            h2v = h2[:].rearrange("p (r q) -> p r q", r=34)
            with tc.tile_pool(name="ex", bufs=2) as exp_pool, \
                 tc.tile_pool(name="xp", bufs=2) as xpool, \
                 tc.tile_pool(name="scp", bufs=2) as scp, \
                 tc.tile_pool(name="dnp", bufs=2, space="DRAM") as dnp, \
                 tc.tile_pool(name="rbp", bufs=2) as rbp:
                for pc_i in range(ROWS // RPC):
                    r0 = pc_i * RPC
                    dnd = dnp.tile([C, 2 * PCN], F32, tag="dnd")
                    for c in range(C):
                        Ea = exp_pool.tile([128, PCN], BF16, tag="Ea")
                        Eb = exp_pool.tile([128, PCN], BF16, tag="Eb")
                        Pa = exp_pool.tile([128, PCN], BF16, tag="Pa")
                        Pb = exp_pool.tile([128, PCN], BF16, tag="Pb")
                        Xa = xpool.tile([128, PCN], BF16, tag="Xa")
                        Xb = xpool.tile([128, PCN], BF16, tag="Xb")
                        # patch strips: partition t = dy*15+dx, free = pixel
                        for dy in range(KK):
                            t0 = dy * KK
                            off = c * 46 * 270 + (r0 + dy) * 270
                            if t0 + KK <= 128:
                                nc.sync.dma_start(
                                    Xa[t0:t0 + KK, :],
                                    mkap(raw46[:], off, [[1, KK], [270, RPC], [1, W]]))
                            elif t0 >= 128:
                                nc.sync.dma_start(
                                    Xb[t0 - 128:t0 - 128 + KK, :],
                                    mkap(raw46[:], off, [[1, KK], [270, RPC], [1, W]]))
                            else:
                                n1 = 128 - t0
                                nc.sync.dma_start(
                                    Xa[t0:128, :],
                                    mkap(raw46[:], off, [[1, n1], [270, RPC], [1, W]]))
                                nc.sync.dma_start(
                                    Xb[0:KK - n1, :],
                                    mkap(raw46[:], off + n1,
                                         [[1, KK - n1], [270, RPC], [1, W]]))
                        # conv3 -> exp (bias fused into exp's activation)
                        for half, E in ((0, Ea), (1, Eb)):
                            mc = c * 2 + half
                            chunks = nchunks(PCN, 512)
                            pss = [psp.tile([128, 512], F32, tag=f"pc{j}",
                                            name=f"ps_{mc}_{j}", bufs=1)
                                   for j in range(len(chunks))]
                            for t, (dy, dx) in enumerate(TAPS):
                                for j, (s0, sl) in enumerate(chunks):
                                    rr = r0 + s0 // W + 1 + dy
                                    nc.tensor.matmul(
                                        pss[j][:, :sl],
                                        lhsT=w3t[:, t * 1024 + mc * 128:
                                                 t * 1024 + (mc + 1) * 128],
                                        rhs=h2v[:, rr:rr + 2, 1 + dx:257 + dx],
                                        start=(t == 0), stop=(t == 8))
                            for j, (s0, sl) in enumerate(chunks):
                                nc.scalar.activation(
                                    E[:, s0:s0 + sl], pss[j][:, :sl], AF.Exp,
                                    bias=b3t[:, mc:mc + 1])
                        # tap sums via ones-matmuls on PE (GPSIMD C-reduce
                        # is ~40us/op; PE does it in ~0.2us/chunk)
                        nc.vector.tensor_tensor(out=Pa[:, :], in0=Ea[:, :], in1=Xa[:, :], op=ALU.mult)
                        nc.vector.tensor_tensor(out=Pb[0:97, :], in0=Eb[0:97, :], in1=Xb[0:97, :], op=ALU.mult)
                        sc = scp.tile([1, 2 * PCN], F32, tag="sc")
                        da, na = sc[:, 0:PCN], sc[:, PCN:2 * PCN]
                        for s0, sl in nchunks(PCN, 512):
                            for dst, ta, tb in ((da, Ea, Eb), (na, Pa, Pb)):
                                pr = psp.tile([1, 512], F32, tag="pr", bufs=2)
                                nc.tensor.matmul(pr[:, :sl], lhsT=ones[:, :],
                                                 rhs=ta[:, s0:s0 + sl],
                                                 start=True, stop=False)
                                nc.tensor.matmul(pr[:, :sl], lhsT=ones[0:97, :],
                                                 rhs=tb[0:97, s0:s0 + sl],
                                                 start=False, stop=True)
                                nc.vector.tensor_copy(dst[:, s0:s0 + sl], pr[:, :sl])
                        nc.sync.dma_start(dnd[c, :], sc[:, :])
                    # reshape [1,2048]x2 per ch -> [128,64] so the divide
                    # runs on all 128 lanes instead of one
                    Dt = rbp.tile([128, 64], F32, tag="Dt")
                    Nt = rbp.tile([128, 64], F32, tag="Nt")
                    for c in range(C):
                        nc.sync.dma_start(
                            Dt[32 * c:32 * c + 32, :],
                            mkap(dnd[:], c * 2 * PCN, [[64, 32], [1, 64]]))
                        nc.sync.dma_start(
                            Nt[32 * c:32 * c + 32, :],
                            mkap(dnd[:], c * 2 * PCN + PCN, [[64, 32], [1, 64]]))
                    nc.vector.reciprocal(Dt[:], Dt[:])
                    nc.vector.tensor_tensor(out=Nt[:], in0=Nt[:], in1=Dt[:], op=ALU.mult)
                    nc.sync.dma_start(corr[:, r0:r0 + RPC, :], Nt[:])
    return nc




# ================================================================ kernel B1
# Forward rfft2 via DFT matmuls, replicated on every core; writes full fri.
# V[h,k] = sum_w x[h,w] Fw[w,k];  Y[k1,k] = sum_h Fh[k1,h] V[h,k]
# fri = [Yre(4ch), Yim(4ch)] as [8, 256, 129].

def build_B1():
    nc = bass.Bass(trn_type="TRN2", name="kernB1")
    corrT = nc.dram_tensor("corrT", (C, 256, 256), BF16, kind="ExternalInput")
    fwre = nc.dram_tensor("fwre", (256, 129), BF16, kind="ExternalInput")
    fwim = nc.dram_tensor("fwim", (256, 129), BF16, kind="ExternalInput")
    fhre = nc.dram_tensor("fhre", (256, 256), BF16, kind="ExternalInput")
    fhim = nc.dram_tensor("fhim", (256, 256), BF16, kind="ExternalInput")
    fhimn = nc.dram_tensor("fhimn", (256, 256), BF16, kind="ExternalInput")
    fri = nc.dram_tensor("fri", (8, 256, 129), F32, kind="ExternalOutput")

    with tile.TileContext(nc) as tc:
        with tc.tile_pool(name="cst", bufs=1) as cst, \
             tc.tile_pool(name="wk", bufs=2) as wk, \
             tc.tile_pool(name="ps", bufs=4, space="PSUM") as psp:
            fw = [cst.tile([128, 2 * 129], BF16, name=f"fw_{i}", tag=f"fw_{i}") for i in range(2)]
            for kc in range(2):
                nc.sync.dma_start(fw[kc][:, 0:129], fwre[kc * 128:(kc + 1) * 128, :])
                nc.sync.dma_start(fw[kc][:, 129:258], fwim[kc * 128:(kc + 1) * 128, :])
            fh = [cst.tile([128, 3 * 256], BF16, name=f"fh_{i}", tag=f"fh_{i}") for i in range(2)]
            for kc in range(2):
                nc.sync.dma_start(fh[kc][:, 0:256], fhre[kc * 128:(kc + 1) * 128, :])
                nc.sync.dma_start(fh[kc][:, 256:512], fhim[kc * 128:(kc + 1) * 128, :])
                nc.sync.dma_start(fh[kc][:, 512:768], fhimn[kc * 128:(kc + 1) * 128, :])
            for c in range(C):
                xT = [wk.tile([128, 256], BF16, name=f"xT{i}", tag=f"xT{i}") for i in range(2)]
                for kc in range(2):
                    nc.sync.dma_start(xT[kc][:], corrT[c, kc * 128:(kc + 1) * 128, :])
                V = [wk.tile([128, 2 * 129], BF16, name=f"V{i}", tag=f"V{i}") for i in range(2)]
                for mc in range(2):      # output h chunk
                    for ri in range(2):  # re / im
                        pv = psp.tile([128, 129], F32, tag="pv")
                        for kc in range(2):
                            nc.tensor.matmul(
                                pv[:, :],
                                lhsT=xT[kc][:, mc * 128:(mc + 1) * 128],
                                rhs=fw[kc][:, ri * 129:(ri + 1) * 129],
                                start=(kc == 0), stop=(kc == 1))
                        nc.vector.tensor_copy(V[mc][:, ri * 129:(ri + 1) * 129], pv[:, :])
                # Y: for re out: FhRe@Vre + FhImNeg@Vim ; im out: FhIm@Vre + FhRe@Vim
                for mc in range(2):      # k1 chunk
                    for ri in range(2):  # re / im output
                        py = psp.tile([128, 129], F32, tag="pv")
                        for kc in range(2):
                            if ri == 0:
                                t1, t2 = 0, 512   # re, imneg
                            else:
                                t1, t2 = 256, 0   # im, re
                            nc.tensor.matmul(
                                py[:, :],
                                lhsT=fh[kc][:, t1 + mc * 128:t1 + (mc + 1) * 128],
                                rhs=V[kc][:, 0:129],
                                start=(kc == 0), stop=False)
                            nc.tensor.matmul(
                                py[:, :],
                                lhsT=fh[kc][:, t2 + mc * 128:t2 + (mc + 1) * 128],
                                rhs=V[kc][:, 129:258],
                                start=False, stop=(kc == 1))
                        ys = wk.tile([128, 129], F32, tag="ys")
                        nc.scalar.activation(ys[:], py[:], AF.Copy)
                        nc.sync.dma_start(
                            fri[ri * 4 + c, mc * 128:(mc + 1) * 128, :], ys[:])
    return nc


# ================================================================ kernel B2
# freq conv stack on fri slab (38 rows, ch-major) + partial inverse fft.
WF = 131  # 129 + 2 pad cols

def build_B2():
    nc = bass.Bass(trn_type="TRN2", name="kernB2")
    fri = nc.dram_tensor("fri", (8, 38 * WF), BF16, kind="ExternalInput")
    gw1 = nc.dram_tensor("gw1", (8, 9 * 64), BF16, kind="ExternalInput")
    gb1 = nc.dram_tensor("gb1", (64, 1), F32, kind="ExternalInput")
    gw2 = nc.dram_tensor("gw2", (64, 9 * 64), BF16, kind="ExternalInput")
    gb2 = nc.dram_tensor("gb2", (64, 1), F32, kind="ExternalInput")
    gw3 = nc.dram_tensor("gw3", (64, 9 * 8), BF16, kind="ExternalInput")
    gb3 = nc.dram_tensor("gb3", (8, 1), F32, kind="ExternalInput")
    mf36 = nc.dram_tensor("mf36", (64, 36), F32, kind="ExternalInput")
    mf34 = nc.dram_tensor("mf34", (64, 34), F32, kind="ExternalInput")
    cfo = nc.dram_tensor("cfo", (8, 32 * 129), F32, kind="ExternalOutput")

    N36, N34, N32 = 36 * WF, 34 * WF, 32 * WF

    with tile.TileContext(nc) as tc:
        with tc.tile_pool(name="cst", bufs=1) as cst, \
             tc.tile_pool(name="gp", bufs=1) as gp, \
             tc.tile_pool(name="ps", bufs=4, space="PSUM") as psp:
            w1t = cst.tile([8, 9 * 64], BF16)
            nc.sync.dma_start(w1t[:], gw1[:])
            w2t = cst.tile([64, 9 * 64], BF16)
            nc.sync.dma_start(w2t[:], gw2[:])
            w3t = cst.tile([64, 9 * 8], BF16)
            nc.sync.dma_start(w3t[:], gw3[:])
            b1t = cst.tile([64, 1], F32)
            nc.sync.dma_start(b1t[:], gb1[:])
            b2t = cst.tile([64, 1], F32)
            nc.sync.dma_start(b2t[:], gb2[:])
            b3t = cst.tile([8, 1], F32)
            nc.sync.dma_start(b3t[:], gb3[:])
            m36t = cst.tile([64, 36], F32)
            nc.sync.dma_start(m36t[:], mf36[:])
            m34t = cst.tile([64, 34], F32)
            nc.sync.dma_start(m34t[:], mf34[:])

            ft = gp.tile([8, 1 + 38 * WF + 4], BF16)
            nc.sync.dma_start(ft[:, 1:1 + 38 * WF], fri[:, :])
            g1 = gp.tile([64, 1 + N36 + 4], BF16)
            g2 = gp.tile([64, 1 + N34 + 4], BF16)
            g3 = gp.tile([8, N32], F32)

            for n0, nl in nchunks(N36, 512):
                pc = psp.tile([64, 512], F32, tag="pg")
                for t, (dy, dx) in enumerate(TAPS):
                    base = (1 + dy) * WF + dx
                    nc.tensor.matmul(
                        pc[:, :nl],
                        lhsT=w1t[:, t * 64:(t + 1) * 64],
                        rhs=ft[:, 1 + n0 + base:1 + n0 + base + nl],
                        start=(t == 0), stop=(t == 8))
                nc.scalar.activation(g1[:, 1 + n0:1 + n0 + nl], pc[:, :nl],
                                     AF.Relu, bias=b1t[:])
            g1v = g1[:, 1:1 + N36].rearrange("p (r q) -> p r q", r=36)
            nc.vector.memset(g1v[:, :, 0:1], 0.0)
            nc.vector.memset(g1v[:, :, 130:131], 0.0)
            for r in (0, 1, 34, 35):
                nc.vector.tensor_scalar_mul(g1v[:, r, :], g1v[:, r, :],
                                            m36t[:, r:r + 1])
            for n0, nl in nchunks(N34, 512):
                pc = psp.tile([64, 512], F32, tag="pg")
                for t, (dy, dx) in enumerate(TAPS):
                    base = (1 + dy) * WF + dx
                    nc.tensor.matmul(
                        pc[:, :nl],
                        lhsT=w2t[:, t * 64:(t + 1) * 64],
                        rhs=g1[:, 1 + n0 + base:1 + n0 + base + nl],
                        start=(t == 0), stop=(t == 8))
                nc.scalar.activation(g2[:, 1 + n0:1 + n0 + nl], pc[:, :nl],
                                     AF.Relu, bias=b2t[:])
            g2v = g2[:, 1:1 + N34].rearrange("p (r q) -> p r q", r=34)
            nc.vector.memset(g2v[:, :, 0:1], 0.0)
            nc.vector.memset(g2v[:, :, 130:131], 0.0)
            for r in (0, 33):
                nc.vector.tensor_scalar_mul(g2v[:, r, :], g2v[:, r, :],
                                            m34t[:, r:r + 1])
            for n0, nl in nchunks(N32, 512):
                pc = psp.tile([8, 512], F32, tag="pg")
                for t, (dy, dx) in enumerate(TAPS):
                    base = (1 + dy) * WF + dx
                    nc.tensor.matmul(
                        pc[:, :nl],
                        lhsT=w3t[:, t * 8:(t + 1) * 8],
                        rhs=g2[:, 1 + n0 + base:1 + n0 + base + nl],
                        start=(t == 0), stop=(t == 8))
                nc.scalar.activation(g3[:, n0:n0 + nl], pc[:, :nl],
                                     AF.Copy, bias=0.0)
            # add bias gb3 separately (Copy cannot take AP bias)
            nc.vector.tensor_scalar(out=g3[:], in0=g3[:], scalar1=b3t[:],
                                    scalar2=None, op0=ALU.add)

            # write CF slab [8 (ri,c), 32 k1-rows, 129] (strip pad cols;
            # real bins live at cols 1..129 of the WF=131 grid)
            nc.sync.dma_start(
                cfo[:, :], mkap_s(g3[:], 1, [[N32, 8], [WF, 32], [1, 129]]))
    return nc


# ================================================================ kernel B
# merged forward DFT + freq convs, one dispatch. V (row FFT) needs all
# columns of the full image (replicated); Y (col FFT) computed only for
# this core's 38-row k1 slab; freq convs 2-half row-packed (bf16).
# partition layouts: ft/g3: p = (ri*4+c)*2 + h; g1/g2: p = u*2 + h.

def build_B():
    nc = bass.Bass(trn_type="TRN2", name="kernB")
    corrT = nc.dram_tensor("corrT", (C, 256, 256), BF16, kind="ExternalInput")
    fwre = nc.dram_tensor("fwre", (256, 129), BF16, kind="ExternalInput")
    fwim = nc.dram_tensor("fwim", (256, 129), BF16, kind="ExternalInput")
    fhs = nc.dram_tensor("fhs", (128, 2 * 3 * 38), BF16, kind="ExternalInput")
    gw1 = nc.dram_tensor("gw1", (16, 9 * 128), BF16, kind="ExternalInput")
    gb1 = nc.dram_tensor("gb1", (128, 1), F32, kind="ExternalInput")
    gw2 = nc.dram_tensor("gw2", (128, 9 * 128), BF16, kind="ExternalInput")
    gb2 = nc.dram_tensor("gb2", (128, 1), F32, kind="ExternalInput")
    gw3 = nc.dram_tensor("gw3", (128, 9 * 16), BF16, kind="ExternalInput")
    gb3 = nc.dram_tensor("gb3", (16, 1), F32, kind="ExternalInput")
    mf20 = nc.dram_tensor("mf20", (128, 20), F32, kind="ExternalInput")
    mf18 = nc.dram_tensor("mf18", (128, 18), F32, kind="ExternalInput")
    cfo = nc.dram_tensor("cfo", (8, 32 * 129), F32, kind="ExternalOutput")

    EXB = 22 * WF       # ft half extent (22 rows x 131)
    EXB1 = 20 * WF
    EXB2 = 18 * WF
    EXB3 = 16 * WF

    with tile.TileContext(nc) as tc:
        with tc.tile_pool(name="cst", bufs=1) as cst, \
             tc.tile_pool(name="gp", bufs=1) as gp, \
             tc.tile_pool(name="ps", bufs=2, space="PSUM") as psp:
            fw = cst.tile([128, 2 * 2 * 129], BF16)
            for kc in range(2):
                nc.sync.dma_start(fw[:, kc * 258:kc * 258 + 129],
                                  fwre[kc * 128:(kc + 1) * 128, :])
                nc.sync.dma_start(fw[:, kc * 258 + 129:kc * 258 + 258],
                                  fwim[kc * 128:(kc + 1) * 128, :])
            fhst = cst.tile([128, 2 * 3 * 38], BF16)
            nc.sync.dma_start(fhst[:], fhs[:])
            fhsv = fhst[:].rearrange("q (k m h) -> q k m h", k=2, m=3)
            w1t = cst.tile([16, 9 * 128], BF16)
            nc.sync.dma_start(w1t[:], gw1[:])
            w2t = cst.tile([128, 9 * 128], BF16)
            nc.sync.dma_start(w2t[:], gw2[:])
            w3t = cst.tile([128, 9 * 16], BF16)
            nc.sync.dma_start(w3t[:], gw3[:])
            b1t = cst.tile([128, 1], F32)
            nc.sync.dma_start(b1t[:], gb1[:])
            b2t = cst.tile([128, 1], F32)
            nc.sync.dma_start(b2t[:], gb2[:])
            b3t = cst.tile([16, 1], F32)
            nc.sync.dma_start(b3t[:], gb3[:])
            m20t = cst.tile([128, 20], F32)
            nc.sync.dma_start(m20t[:], mf20[:])
            m18t = cst.tile([128, 18], F32)
            nc.sync.dma_start(m18t[:], mf18[:])

            ft = gp.tile([16, GD + EXB + GD], BF16)
            nc.vector.memset(ft[:], 0.0)
            with tc.tile_pool(name="vp", bufs=3) as vp:
                for c in range(C):
                    xT = vp.tile([128, 2 * 256], BF16, tag="xT",
                                 name=f"xT{c}")
                    for kc in range(2):
                        nc.sync.dma_start(
                            xT[:, kc * 256:(kc + 1) * 256],
                            corrT[c, kc * 128:(kc + 1) * 128, :])
                    V = [vp.tile([128, 2 * 129], BF16, name=f"V{c}_{m}",
                                 tag=f"V{m}") for m in range(2)]
                    for mc in range(2):
                        pv = psp.tile([128, 258], F32, tag="pv",
                                      name=f"pv{c}_{mc}", bufs=2)
                        for kc in range(2):
                            nc.tensor.matmul(
                                pv[:],
                                lhsT=xT[:, kc * 256 + mc * 128:
                                        kc * 256 + (mc + 1) * 128],
                                rhs=fw[:, kc * 258:(kc + 1) * 258],
                                start=(kc == 0), stop=(kc == 1))
                        nc.scalar.activation(V[mc][:], pv[:], AF.Copy)
                    for ri in range(2):
                        py = psp.tile([38, 129], F32, tag="py",
                                      name=f"py{c}_{ri}", bufs=1)
                        t1, t2 = (0, 2) if ri == 0 else (1, 0)
                        ti = 0
                        for hc in range(2):
                            nc.tensor.matmul(
                                py[:], lhsT=fhsv[:, hc, t1, :],
                                rhs=V[hc][:, 0:129],
                                start=(ti == 0), stop=False)
                            ti += 1
                            nc.tensor.matmul(
                                py[:], lhsT=fhsv[:, hc, t2, :],
                                rhs=V[hc][:, 129:258],
                                start=False, stop=(ti == 3))
                            ti += 1
                        ys = vp.tile([38, 129], BF16, tag="ys",
                                     name=f"ys{c}_{ri}")
                        nc.scalar.activation(ys[:], py[:], AF.Copy)
                        for h in range(2):
                            p = (ri * 4 + c) * 2 + h
                            q = nc.sync if h == 0 else nc.gpsimd
                            q.dma_start(
                                mkap_s(ft[p:p + 1, :], GD + 1,
                                       [[GD + EXB + GD, 1], [WF, 22],
                                        [1, 129]]),
                                ys[h * 16:h * 16 + 22, :])

            g1 = gp.tile([128, GD + EXB1 + GD], BF16)
            g2 = gp.tile([128, GD + EXB2 + GD], BF16)
            w1v = w1t[:].rearrange("q (t c) -> q t c", t=9)
            for n0, nl in nchunks(EXB1, 512):
                ps = psp.tile([128, 512], F32, tag="cv", name=f"e1_{n0}",
                              bufs=3)
                for t, (dy, dx) in enumerate(TAPS):
                    off = GD + n0 + (1 + dy) * WF + dx
                    nc.tensor.matmul(ps[:, :nl], lhsT=w1v[:, t, :],
                                     rhs=ft[:, off:off + nl],
                                     start=(t == 0), stop=(t == 8))
                nc.scalar.activation(g1[:, GD + n0:GD + n0 + nl], ps[:, :nl],
                                     AF.Relu, bias=b1t[:])
            g1v = g1[:, GD:GD + EXB1].rearrange("q (r c) -> q r c", r=20)
            nc.vector.memset(g1v[:, :, 0:1], 0.0)
            nc.vector.memset(g1v[:, :, 130:131], 0.0)
            for r in (0, 1, 18, 19):
                nc.vector.tensor_scalar_mul(g1v[:, r, :], g1v[:, r, :],
                                            m20t[:, r:r + 1])
            w2v = w2t[:].rearrange("q (t c) -> q t c", t=9)
            for n0, nl in nchunks(EXB2, 512):
                ps = psp.tile([128, 512], F32, tag="cv", name=f"e2_{n0}",
                              bufs=3)
                for t, (dy, dx) in enumerate(TAPS):
                    off = GD + n0 + (1 + dy) * WF + dx
                    nc.tensor.matmul(ps[:, :nl], lhsT=w2v[:, t, :],
                                     rhs=g1[:, off:off + nl],
                                     start=(t == 0), stop=(t == 8))
                nc.scalar.activation(g2[:, GD + n0:GD + n0 + nl], ps[:, :nl],
                                     AF.Relu, bias=b2t[:])
            g2v = g2[:, GD:GD + EXB2].rearrange("q (r c) -> q r c", r=18)
            nc.vector.memset(g2v[:, :, 0:1], 0.0)
            nc.vector.memset(g2v[:, :, 130:131], 0.0)
            for r in (0, 17):
                nc.vector.tensor_scalar_mul(g2v[:, r, :], g2v[:, r, :],
                                            m18t[:, r:r + 1])
            g3 = gp.tile([16, EXB3], F32)
            w3v = w3t[:].rearrange("q (t c) -> q t c", t=9)
            for n0, nl in nchunks(EXB3, 512):
                ps = psp.tile([16, 512], F32, tag="cv3", name=f"e3_{n0}",
                              bufs=2)
                for t, (dy, dx) in enumerate(TAPS):
                    off = GD + n0 + (1 + dy) * WF + dx
                    nc.tensor.matmul(ps[:, :nl], lhsT=w3v[:, t, :],
                                     rhs=g2[:, off:off + nl],
                                     start=(t == 0), stop=(t == 8))
                nc.scalar.activation(g3[:, n0:n0 + nl], ps[:, :nl], AF.Copy)
            nc.vector.tensor_scalar(out=g3[:], in0=g3[:], scalar1=b3t[:],
                                    scalar2=None, op0=ALU.add)
            nc.sync.dma_start(
                mkap(cfo[:], 0, [[2064, 16], [129, 16], [1, 129]]),
                mkap_s(g3[:], 1, [[EXB3, 16], [WF, 16], [1, 129]]))
    return nc


def run_B(corr1, fw1, fb1, fw2, fb2, fw3, fb3, trace=False):
    if "B" not in _CACHE:
        _CACHE["B"] = patch_nc(build_B())
    fwre, fwim, _, _, _, _, _, _ = _dft_mats()
    corrTh = _bf(np.ascontiguousarray(corr1.transpose(0, 2, 1)))
    # block-diag weights: in p=(j)*2+h (j=ri*4+c), hid p=u*2+h, out p=j*2+h
    w1h = np.zeros((16, 9, 128), np.float32)
    w2h = np.zeros((128, 9, 128), np.float32)
    w3h = np.zeros((128, 9, 16), np.float32)
    b1h = np.zeros((128, 1), np.float32)
    b2h = np.zeros((128, 1), np.float32)
    b3h = np.zeros((16, 1), np.float32)
    for h in range(2):
        for u in range(64):
            b1h[u * 2 + h, 0] = fb1[u]
            b2h[u * 2 + h, 0] = fb2[u]
        for j in range(8):
            b3h[j * 2 + h, 0] = fb3[j]
    for t in range(9):
        dy, dx = t // 3, t % 3
        for h in range(2):
            for u in range(64):
                for j in range(8):
                    w1h[j * 2 + h, t, u * 2 + h] = fw1[u, j, dy, dx]
                    w3h[u * 2 + h, t, j * 2 + h] = fw3[j, u, dy, dx]
                for v in range(64):
                    w2h[v * 2 + h, t, u * 2 + h] = fw2[u, v, dy, dx]
    hhs = np.arange(256)
    ins = []
    for i in range(N_CORES):
        r0 = i * ROWS
        k1s = np.arange(r0 - 3, r0 + 35)
        ok = (k1s >= 0) & (k1s < 256)
        fhsh = np.zeros((128, 2, 3, 38), np.float32)
        for hc in range(2):
            h_ = np.arange(hc * 128, hc * 128 + 128)
            th = 2 * np.pi * np.outer(h_, k1s) / 256.0
            fhsh[:, hc, 0, :] = np.cos(th) / 16.0 * ok[None, :]
            fhsh[:, hc, 1, :] = -np.sin(th) / 16.0 * ok[None, :]
            fhsh[:, hc, 2, :] = np.sin(th) / 16.0 * ok[None, :]
        m20 = np.zeros((128, 20), np.float32)
        m18 = np.zeros((128, 18), np.float32)
        for p in range(128):
            h = p % 2
            base = r0 + h * 16
            for r in range(20):
                m20[p, r] = 1.0 if 0 <= base - 2 + r < 256 else 0.0
            for r in range(18):
                m18[p, r] = 1.0 if 0 <= base - 1 + r < 256 else 0.0
        ins.append({
            "corrT": corrTh, "fwre": fwre, "fwim": fwim,
            "fhs": _bf(fhsh.reshape(128, -1)),
            "gw1": _bf(w1h.reshape(16, -1)), "gb1": b1h,
            "gw2": _bf(w2h.reshape(128, -1)), "gb2": b2h,
            "gw3": _bf(w3h.reshape(128, -1)), "gb3": b3h,
            "mf20": m20, "mf18": m18,
        })
    res = run_bass_kernel_spmd(_CACHE["B"], ins, core_ids=list(range(N_CORES)),
                               trace=trace)
    cf = np.zeros((8, 256, 129), np.float32)
    for i in range(N_CORES):
        f = res.results[i]["cfo"].reshape(16, 16, 129)
        for j in range(8):
            for h in range(2):
                cf[j, i * ROWS + h * 16:i * ROWS + h * 16 + 16, :] = \
                    f[j * 2 + h]
    return None, cf, res


# ================================================================ kernel C
# inverse DFT from full CF (host-gathered) + per-channel refinement as
# 2-half row-packed block-diagonal convs (128 partitions, bf16).
# partition layouts: u/r3: p = c*2 + h; r1/r2: p = c*32 + u*2 + h.
WPC = 258
EXTC = 22 * WPC      # u half extent (22 rows)
EXTR1 = 20 * WPC
EXTR2 = 18 * WPC
EXTR3 = 16 * WPC


def build_C():
    nc = bass.Bass(trn_type="TRN2", name="kernC")
    u0 = nc.dram_tensor("u0", (8, EXTC), BF16, kind="ExternalInput")
    raws = nc.dram_tensor("raws", (8, EXTR3), F32, kind="ExternalInput")
    cfa = nc.dram_tensor("cfa", (128, 4 * 2 * 2 * 128), BF16,
                         kind="ExternalInput")
    cfb = nc.dram_tensor("cfb", (1, 4 * 2 * 2 * 128), BF16,
                         kind="ExternalInput")
    iwm = nc.dram_tensor("iwm", (128, 2 * 512), BF16, kind="ExternalInput")
    iwbm = nc.dram_tensor("iwbm", (1, 2 * 512), BF16, kind="ExternalInput")
    ihs = nc.dram_tensor("ihs", (128, 2 * 2 * 38), BF16,
                         kind="ExternalInput")
    cw1 = nc.dram_tensor("cw1", (8, 9 * 128), BF16, kind="ExternalInput")
    cb1 = nc.dram_tensor("cb1", (128, 1), F32, kind="ExternalInput")
    cw2 = nc.dram_tensor("cw2", (128, 9 * 128), BF16, kind="ExternalInput")
    cb2 = nc.dram_tensor("cb2", (128, 1), F32, kind="ExternalInput")
    cw3 = nc.dram_tensor("cw3", (128, 9 * 8), BF16, kind="ExternalInput")
    cb3 = nc.dram_tensor("cb3", (8, 1), F32, kind="ExternalInput")
    mr20 = nc.dram_tensor("mr20", (128, 20), F32, kind="ExternalInput")
    mr18 = nc.dram_tensor("mr18", (128, 18), F32, kind="ExternalInput")
    fin = nc.dram_tensor("fin", (8, 16 * 256), F32, kind="ExternalOutput")

    with tile.TileContext(nc) as tc:
        with tc.tile_pool(name="cst", bufs=1) as cst, \
             tc.tile_pool(name="gp", bufs=1) as gp, \
             tc.tile_pool(name="ps", bufs=1, space="PSUM") as psp:
            cfat = cst.tile([128, 4 * 2 * 2 * 128], BF16)
            nc.sync.dma_start(cfat[:], cfa[:])
            cfbt = cst.tile([1, 4 * 2 * 2 * 128], BF16)
            nc.sync.dma_start(cfbt[:], cfb[:])
            iwt = cst.tile([128, 2 * 512], BF16)
            nc.sync.dma_start(iwt[:], iwm[:])
            iwbt = cst.tile([1, 2 * 512], BF16)
            nc.sync.dma_start(iwbt[:], iwbm[:])
            ihst = cst.tile([128, 2 * 2 * 38], BF16)
            nc.sync.dma_start(ihst[:], ihs[:])
            w1t = cst.tile([8, 9 * 128], BF16)
            nc.sync.dma_start(w1t[:], cw1[:])
            w2t = cst.tile([128, 9 * 128], BF16)
            nc.sync.dma_start(w2t[:], cw2[:])
            w3t = cst.tile([128, 9 * 8], BF16)
            nc.sync.dma_start(w3t[:], cw3[:])
            b1t = cst.tile([128, 1], F32)
            nc.sync.dma_start(b1t[:], cb1[:])
            b2t = cst.tile([128, 1], F32)
            nc.sync.dma_start(b2t[:], cb2[:])
            b3t = cst.tile([8, 1], F32)
            nc.sync.dma_start(b3t[:], cb3[:])
            m20t = cst.tile([128, 20], F32)
            nc.sync.dma_start(m20t[:], mr20[:])
            m18t = cst.tile([128, 18], F32)
            nc.sync.dma_start(m18t[:], mr18[:])

            cfav = cfat[:].rearrange("q (c r k m) -> q c r k m", c=4, r=2, k=2)
            cfbv = cfbt[:].rearrange("q (c r k m) -> q c r k m", c=4, r=2, k=2)
            ihsv = ihst[:].rearrange("q (k t h) -> q k t h", k=2, t=2)

            # u = u0 + z (inverse DFT), packed [8, GD + EXTC + GD]
            u = gp.tile([8, GD + EXTC + GD], BF16)
            nc.vector.memset(u[:, 0:GD], 0.0)
            nc.vector.memset(u[:, GD + EXTC:], 0.0)
            nc.sync.dma_start(u[:, GD:GD + EXTC], u0[:])
            zu = gp.tile([8, EXTC], BF16)
            nc.vector.memset(zu[:], 0.0)
            with tc.tile_pool(name="ip", bufs=2) as ip:
                for c in range(C):
                    # B[kc][k1, ri, w] = sum_k cf[c,k1,k] iw[k,w] (complex)
                    Bt = [ip.tile([128, 2 * 256], BF16, tag=f"Bt{kc}",
                                  name=f"Bt_{c}_{kc}") for kc in range(2)]
                    for kc in range(2):
                        pb = psp.tile([128, 512], F32, tag="pb",
                                      name=f"pb_{c}_{kc}", bufs=2)
                        nc.tensor.matmul(
                            pb[:], lhsT=cfav[:, c, 0, kc, :],
                            rhs=iwt[:, 0:512], start=True, stop=False)
                        nc.tensor.matmul(
                            pb[:], lhsT=cfbv[:, c, 0, kc, :],
                            rhs=iwbt[:, 0:512], start=False, stop=False)
                        nc.tensor.matmul(
                            pb[:], lhsT=cfav[:, c, 1, kc, :],
                            rhs=iwt[:, 512:1024], start=False, stop=False)
                        nc.tensor.matmul(
                            pb[:], lhsT=cfbv[:, c, 1, kc, :],
                            rhs=iwbt[:, 512:1024], start=False, stop=True)
                        nc.scalar.activation(Bt[kc][:], pb[:], AF.Copy)
                    # z[hh, w] = sum_k1 ih[k1, hh] B[k1, w] (re part)
                    pz = psp.tile([38, 256], F32, tag="pz",
                                  name=f"pz_{c}", bufs=1)
                    ti = 0
                    for kc in range(2):
                        for term in range(2):
                            nc.tensor.matmul(
                                pz[:], lhsT=ihsv[:, kc, term, :],
                                rhs=Bt[kc][:, term * 256:(term + 1) * 256],
                                start=(ti == 0), stop=(ti == 3))
                            ti += 1
                    zs = ip.tile([38, 256], BF16, tag="zs", name=f"zs_{c}")
                    nc.scalar.activation(zs[:], pz[:], AF.Copy)
                    for h in range(2):
                        zq = nc.sync if h == 0 else nc.gpsimd
                        zq.dma_start(
                            mkap_s(zu[c * 2 + h:c * 2 + h + 1, :], 1,
                                   [[EXTC, 1], [WPC, 22], [1, 256]]),
                            zs[h * 16:h * 16 + 22, :])
            UH = 11 * WPC
            nc.vector.tensor_tensor(out=u[:, GD:GD + UH],
                                    in0=u[:, GD:GD + UH], in1=zu[:, :UH],
                                    op=ALU.add)
            nc.vector.tensor_tensor(out=u[:, GD + UH:GD + EXTC],
                                    in0=u[:, GD + UH:GD + EXTC],
                                    in1=zu[:, UH:], op=ALU.add)

            r1 = gp.tile([128, GD + EXTR1 + GD], BF16)
            r2 = gp.tile([128, GD + EXTR2 + GD], BF16)
            for n0, nl in nchunks(EXTR1, 512):
                ps = psp.tile([128, 512], F32, tag="cv", name=f"d1_{n0}",
                              bufs=3)
                for t, (dy, dx) in enumerate(TAPS):
                    off = GD + n0 + (1 + dy) * WPC + dx
                    nc.tensor.matmul(ps[:, :nl],
                                     lhsT=w1t[:].rearrange(
                                         "q (t c) -> q t c", t=9)[:, t, :],
                                     rhs=u[:, off:off + nl],
                                     start=(t == 0), stop=(t == 8))
                nc.scalar.activation(r1[:, GD + n0:GD + n0 + nl], ps[:, :nl],
                                     AF.Relu, bias=b1t[:])
            r1v = r1[:, GD:GD + EXTR1].rearrange("q (r c) -> q r c", r=20)
            nc.vector.memset(r1v[:, :, 0:1], 0.0)
            nc.vector.memset(r1v[:, :, 257:258], 0.0)
            for r in (0, 1, 18, 19):
                nc.vector.tensor_scalar_mul(r1v[:, r, :], r1v[:, r, :],
                                            m20t[:, r:r + 1])
            for n0, nl in nchunks(EXTR2, 512):
                ps = psp.tile([128, 512], F32, tag="cv", name=f"d2_{n0}",
                              bufs=3)
                for t, (dy, dx) in enumerate(TAPS):
                    off = GD + n0 + (1 + dy) * WPC + dx
                    nc.tensor.matmul(ps[:, :nl],
                                     lhsT=w2t[:].rearrange(
                                         "q (t c) -> q t c", t=9)[:, t, :],
                                     rhs=r1[:, off:off + nl],
                                     start=(t == 0), stop=(t == 8))
                nc.scalar.activation(r2[:, GD + n0:GD + n0 + nl], ps[:, :nl],
                                     AF.Relu, bias=b2t[:])
            r2v = r2[:, GD:GD + EXTR2].rearrange("q (r c) -> q r c", r=18)
            nc.vector.memset(r2v[:, :, 0:1], 0.0)
            nc.vector.memset(r2v[:, :, 257:258], 0.0)
            for r in (0, 17):
                nc.vector.tensor_scalar_mul(r2v[:, r, :], r2v[:, r, :],
                                            m18t[:, r:r + 1])
            r3 = gp.tile([8, EXTR3], F32)
            rawt = gp.tile([8, EXTR3], F32)
            nc.sync.dma_start(rawt[:], raws[:])
            for n0, nl in nchunks(EXTR3, 512):
                ps = psp.tile([8, 512], F32, tag="cv3", name=f"d3_{n0}",
                              bufs=2)
                for t, (dy, dx) in enumerate(TAPS):
                    off = GD + n0 + (1 + dy) * WPC + dx
                    nc.tensor.matmul(ps[:, :nl],
                                     lhsT=w3t[:].rearrange(
                                         "q (t c) -> q t c", t=9)[:, t, :],
                                     rhs=r2[:, off:off + nl],
                                     start=(t == 0), stop=(t == 8))
                nc.vector.tensor_tensor(out=r3[:, n0:n0 + nl],
                                        in0=ps[:, :nl],
                                        in1=rawt[:, n0:n0 + nl], op=ALU.add)
                nc.vector.tensor_scalar(out=r3[:, n0:n0 + nl],
                                        in0=r3[:, n0:n0 + nl], scalar1=0.0,
                                        scalar2=1.0, op0=ALU.max,
                                        op1=ALU.min)
            nc.sync.dma_start(
                fin[:, :], mkap_s(r3[:], 1, [[EXTR3, 8], [WPC, 16],
                                             [1, 256]]))
    return nc


def build_C_old():
    nc = bass.Bass(trn_type="TRN2", name="kernC")
    u = nc.dram_tensor("u", (C, 38 * WP), BF16, kind="ExternalInput")
    raw32 = nc.dram_tensor("raw32", (C, ROWS * W), F32, kind="ExternalInput")
    cw1 = nc.dram_tensor("cw1", (C, 9 * 64), BF16, kind="ExternalInput")
    cb1 = nc.dram_tensor("cb1", (64, 1), F32, kind="ExternalInput")
    cw2 = nc.dram_tensor("cw2", (64, 9 * 64), BF16, kind="ExternalInput")
    cb2 = nc.dram_tensor("cb2", (64, 1), F32, kind="ExternalInput")
    cw3 = nc.dram_tensor("cw3", (64, 9 * 4), BF16, kind="ExternalInput")
    cb3 = nc.dram_tensor("cb3", (4, 1), F32, kind="ExternalInput")
    mr36 = nc.dram_tensor("mr36", (64, 36), F32, kind="ExternalInput")
    mr34 = nc.dram_tensor("mr34", (64, 34), F32, kind="ExternalInput")
    fin = nc.dram_tensor("fin", (C, ROWS, W), F32, kind="ExternalOutput")

    N36, N34, N32 = 36 * WP, 34 * WP, 32 * WP

    def conv_taps_outer(pool_ps, lhsw, rhsrc, dstact, bias, Ntot, Kp, Mp, relu,
                        group=1):
        """taps-outer grouped conv: lhsw(t)->lhsT AP, rhsrc(t, n0, nl)->rhs AP,
        dstact(n0, nl, psum) consumes."""
        chunks = nchunks(Ntot, 512)
        for g0 in range(0, len(chunks), group):
            grp = chunks[g0:g0 + group]
            pss = [pool_ps.tile([Mp, 512], F32, tag=f"cg{j}", name=f"cg_{g0}_{j}",
                                bufs=6) for j in range(len(grp))]
            for t in range(9):
                for j, (n0, nl) in enumerate(grp):
                    nc.tensor.matmul(pss[j][:, :nl], lhsT=lhsw(t),
                                     rhs=rhsrc(t, n0, nl),
                                     start=(t == 0), stop=(t == 8))
            for j, (n0, nl) in enumerate(grp):
                dstact(n0, nl, pss[j])

    with tile.TileContext(nc) as tc:
        with tc.tile_pool(name="cst", bufs=1) as cst, \
             tc.tile_pool(name="gp", bufs=1) as gp, \
             tc.tile_pool(name="ps", bufs=1, space="PSUM") as psp:
            w1t = cst.tile([C, 9 * 64], BF16)
            nc.sync.dma_start(w1t[:], cw1[:])
            w2t = cst.tile([64, 9 * 64], BF16)
            nc.sync.dma_start(w2t[:], cw2[:])
            w3t = cst.tile([64, 9 * 4], BF16)
            nc.sync.dma_start(w3t[:], cw3[:])
            b1t = cst.tile([64, 1], F32)
            nc.sync.dma_start(b1t[:], cb1[:])
            b2t = cst.tile([64, 1], F32)
            nc.sync.dma_start(b2t[:], cb2[:])
            b3t = cst.tile([C, 1], F32)
            nc.sync.dma_start(b3t[:], cb3[:])
            m36t = cst.tile([64, 36], F32)
            nc.sync.dma_start(m36t[:], mr36[:])
            m34t = cst.tile([64, 34], F32)
            nc.sync.dma_start(m34t[:], mr34[:])

            ut = gp.tile([C, 1 + 38 * WP + 4], BF16)
            nc.sync.dma_start(ut[:, 1:1 + 38 * WP], u[:])
            r1 = gp.tile([64, 1 + N36 + 4], BF16)
            r2 = gp.tile([64, 1 + N34 + 4], BF16)

            conv_taps_outer(
                psp,
                lambda t: w1t[:, t * 64:(t + 1) * 64],
                lambda t, n0, nl: ut[:, 1 + n0 + (1 + TAPS[t][0]) * WP + TAPS[t][1]:
                                     1 + n0 + (1 + TAPS[t][0]) * WP + TAPS[t][1] + nl],
                lambda n0, nl, ps: nc.scalar.activation(
                    r1[:, 1 + n0:1 + n0 + nl], ps[:, :nl], AF.Relu, bias=b1t[:]),
                b1t, N36, 64, 64, True)
            r1v = r1[:, 1:1 + N36].rearrange("p (r q) -> p r q", r=36)
            nc.vector.memset(r1v[:, :, 0:1], 0.0)
            nc.vector.memset(r1v[:, :, 257:258], 0.0)
            for r in (0, 1, 34, 35):
                nc.vector.tensor_scalar_mul(r1v[:, r, :], r1v[:, r, :],
                                            m36t[:, r:r + 1])

            conv_taps_outer(
                psp,
                lambda t: w2t[:, t * 64:(t + 1) * 64],
                lambda t, n0, nl: r1[:, 1 + n0 + (1 + TAPS[t][0]) * WP + TAPS[t][1]:
                                     1 + n0 + (1 + TAPS[t][0]) * WP + TAPS[t][1] + nl],
                lambda n0, nl, ps: nc.scalar.activation(
                    r2[:, 1 + n0:1 + n0 + nl], ps[:, :nl], AF.Relu, bias=b2t[:]),
                b2t, N34, 64, 64, True)
            r2v = r2[:, 1:1 + N34].rearrange("p (r q) -> p r q", r=34)
            nc.vector.memset(r2v[:, :, 0:1], 0.0)
            nc.vector.memset(r2v[:, :, 257:258], 0.0)
            for r in (0, 33):
                nc.vector.tensor_scalar_mul(r2v[:, r, :], r2v[:, r, :],
                                            m34t[:, r:r + 1])

            with tc.tile_pool(name="fo", bufs=1) as fo:
                rawt = fo.tile([C, ROWS * W], F32)
                nc.sync.dma_start(rawt[:], raw32[:])
                r3 = fo.tile([C, N32], F32)
                conv_taps_outer(
                    psp,
                    lambda t: w3t[:, t * 4:(t + 1) * 4],
                    lambda t, n0, nl: r2[:, 1 + n0 + (1 + TAPS[t][0]) * WP + TAPS[t][1]:
                                         1 + n0 + (1 + TAPS[t][0]) * WP + TAPS[t][1] + nl],
                    lambda n0, nl, ps: nc.scalar.activation(
                        r3[:, n0:n0 + nl], ps[:, :nl], AF.Copy),
                    b3t, N32, 64, C, False)
                r3v = r3[:].rearrange("p (r q) -> p r q", r=32)[:, :, 1:257]
                rv = rawt[:].rearrange("p (r q) -> p r q", r=32)
                nc.vector.tensor_scalar(out=r3v, in0=r3v, scalar1=b3t[:],
                                        scalar2=None, op0=ALU.add)
                nc.vector.tensor_tensor(out=r3v, in0=r3v, in1=rv, op=ALU.add)
                nc.vector.tensor_scalar(out=r3v, in0=r3v, scalar1=0.0,
                                        scalar2=1.0, op0=ALU.max, op1=ALU.min)
                nc.sync.dma_start(fin[:, :, :], r3v)
    return nc


_CACHE = {}


def _f8(x):
    return np.asarray(x, dtype=np.float32).astype(ml_dtypes.float8_e4m3)


def _prep_A(raw, feat, pw1, pb1, pw2, pb2, pw3, pb3):
    # weights packed for DoubleRow passes (see PAIRS)
    def tap_w(pw, dydx):
        dy, dx = dydx
        return pw[:, :, dy + 1, dx + 1]  # [co, ci]

    # w1: [ci, m, p, kt, co128]
    w1h = np.zeros((128, 2, 5, 2, 128), np.float32)
    for m in range(2):
        for p in range(5):
            t0, t1 = pair_taps(p)
            w1h[:, m, p, 0, :] = tap_w(pw1, t0).T[:, m * 128:(m + 1) * 128]
            if t1 is not None:
                w1h[:, m, p, 1, :] = tap_w(pw1, t1).T[:, m * 128:(m + 1) * 128]
    # w2: [cip, t, kc, co]
    w2h = np.zeros((128, 9, 2, 128), np.float32)
    for t, (dy, dx) in enumerate(TAPS):
        wt = tap_w(pw2, (dy, dx))  # [128 co, 256 ci]
        for kc in range(2):
            w2h[:, t, kc, :] = wt[:, kc * 128:(kc + 1) * 128].T
    # w3: [ci, p, kt, 912] (col = c*228 + tpsf)
    w3h = np.zeros((128, 5, 2, 912), np.float32)
    for p in range(5):
        t0, t1 = pair_taps(p)
        for kt, tt in ((0, t0), (1, t1)):
            if tt is None:
                continue
            wt = tap_w(pw3, tt)  # [900, 128]
            for c in range(C):
                w3h[:, p, kt, c * 228:c * 228 + 225] = \
                    wt[c * 225:(c + 1) * 225].T
    b1h = np.ascontiguousarray(pb1.reshape(2, 128).T).astype(np.float32)
    b2h = pb2.reshape(128, 1).astype(np.float32)
    b3row = np.full((912,), -30.0, np.float32)
    for c in range(C):
        b3row[c * 228:c * 228 + 225] = pb3[c * 225:(c + 1) * 225]
    w3h[0, 4, 1, :] = b3row

    xpad = np.pad(raw, ((0, 0), (PAD, PAD), (PAD, PAD)), mode="reflect")
    # unfolded patches [4, 256, 256, 15, 15]
    sw = np.lib.stride_tricks.sliding_window_view(xpad, (15, 15),
                                                  axis=(1, 2))
    featp = np.pad(feat, ((0, 0), (3, 3), (0, 0)))

    ins = []
    for i in range(N_CORES):
        r0 = i * ROWS
        m36 = np.array([1.0 if 0 <= r0 - 2 + r < H else 0.0
                        for r in range(36)], np.float32)
        m34 = np.array([1.0 if 0 <= r0 - 1 + r < H else 0.0
                        for r in range(34)], np.float32)
        fbA = np.zeros((128, 38, RP), np.float32)
        fbA[:, :, 1:257] = featp[:, r0:r0 + 38, :]
        fbA = fbA.reshape(128, EXTF)
        fbh = np.zeros((128, GD + 2 * EXTF), np.float32)
        fbh[:, GD:GD + EXTF] = fbA
        fbh[:, GD + EXTF:GD + 2 * EXTF - 1] = fbA[:, 1:]
        # Xu: [8192 pix, 912] = (r, x) -> [c*228 + tpsf]; bias comes via
        # the psum ones-matmul, so patches stay unscaled
        slab = sw[:, r0:r0 + ROWS, :, :, :]  # [4, 32, 256, 15, 15]
        xuh = np.zeros((ROWS * W, 4, 228), np.float32)
        xuh[:, :, :225] = slab.reshape(4, ROWS * W, 225).transpose(1, 0, 2)
        xuh = xuh.reshape(ROWS * W, 912)
        ins.append({
            "fb": _f8(fbh),
            "w1": _f8(w1h.reshape(128, -1)), "b1": b1h,
            "w2": _f8(w2h.reshape(128, -1)), "b2": b2h,
            "w3": _f8(w3h.reshape(128, -1)),
            "xu": _bf(xuh),
            "m36": np.ascontiguousarray(np.broadcast_to(m36, (128, 36))),
            "m34": np.ascontiguousarray(np.broadcast_to(m34, (128, 34))),
        })
    return ins


def run_A(raw, feat, pw1, pb1, pw2, pb2, pw3, pb3, trace=False):
    if "A" not in _CACHE:
        _CACHE["A"] = patch_nc(build_A())
    ins = _prep_A(raw, feat, pw1, pb1, pw2, pb2, pw3, pb3)
    res = run_bass_kernel_spmd(_CACHE["A"], ins, core_ids=list(range(N_CORES)),
                               trace=trace)
    corr = np.concatenate(
        [res.results[i]["corr"].reshape(ROWS, 2, 4, 128)
         .transpose(2, 0, 1, 3).reshape(C, ROWS, W)
         for i in range(N_CORES)], axis=1)
    return corr, res


def _dft_mats():
    k = np.arange(129)
    w = np.arange(256)
    th = 2 * np.pi * np.outer(w, k) / 256.0          # [256, 129]
    fwre = _bf(np.cos(th) / 16.0)
    fwim = _bf(-np.sin(th) / 16.0)
    h = np.arange(256)
    k1 = np.arange(256)
    th2 = 2 * np.pi * np.outer(h, k1) / 256.0        # [256h, 256k1]
    fhre = _bf(np.cos(th2) / 16.0)
    fhim = _bf(-np.sin(th2) / 16.0)
    fhimn = _bf(np.sin(th2) / 16.0)
    ck = np.where((k == 0) | (k == 128), 1.0, 2.0)
    th3 = 2 * np.pi * np.outer(k, w) / 256.0         # [129k, 256w]
    iwre = _bf(ck[:, None] * np.cos(th3) / 16.0)
    iwim = _bf(ck[:, None] * np.sin(th3) / 16.0)
    iwimn = _bf(-ck[:, None] * np.sin(th3) / 16.0)
    return fwre, fwim, fhre, fhim, fhimn, iwre, iwim, iwimn


def run_B1(corr1, trace=False):
    if "B1" not in _CACHE:
        _CACHE["B1"] = patch_nc(build_B1())
    fwre, fwim, fhre, fhim, fhimn, _, _, _ = _dft_mats()
    corrT = _bf(np.ascontiguousarray(corr1.transpose(0, 2, 1)))
    inm = {"corrT": corrT, "fwre": fwre, "fwim": fwim,
           "fhre": fhre, "fhim": fhim, "fhimn": fhimn}
    res = run_bass_kernel_spmd(_CACHE["B1"], [inm] * N_CORES,
                               core_ids=list(range(N_CORES)), trace=trace)
    return res.results[0]["fri"], res


def run_B2(fri_full, fw1, fb1, fw2, fb2, fw3, fb3, trace=False):
    from einops import rearrange as rr
    if "B2" not in _CACHE:
        _CACHE["B2"] = patch_nc(build_B2())
    gw1 = _bf(rr(fw1, "co ci dy dx -> ci (dy dx co)"))
    gw2 = _bf(rr(fw2, "co ci dy dx -> ci (dy dx co)"))
    gw3 = _bf(rr(fw3, "co ci dy dx -> ci (dy dx co)"))
    gb1 = fb1.reshape(64, 1).astype(np.float32)
    gb2 = fb2.reshape(64, 1).astype(np.float32)
    gb3 = fb3.reshape(8, 1).astype(np.float32)
    ins = []
    for i in range(N_CORES):
        r0 = i * ROWS
        slab = np.zeros((8, 38, WF), np.float32)
        lo, hi = max(0, r0 - 3), min(256, r0 + 35)
        slab[:, lo - (r0 - 3):hi - (r0 - 3), 1:130] = fri_full[:, lo:hi, :]
        m36 = np.array([1.0 if 0 <= r0 - 2 + r < 256 else 0.0
                        for r in range(36)], np.float32)
        m34 = np.array([1.0 if 0 <= r0 - 1 + r < 256 else 0.0
                        for r in range(34)], np.float32)
        ins.append({
            "fri": _bf(slab.reshape(8, 38 * WF)),
            "gw1": gw1, "gb1": gb1, "gw2": gw2, "gb2": gb2,
            "gw3": gw3, "gb3": gb3,
            "mf36": np.ascontiguousarray(np.broadcast_to(m36, (64, 36))),
            "mf34": np.ascontiguousarray(np.broadcast_to(m34, (64, 34))),
        })
    res = run_bass_kernel_spmd(_CACHE["B2"], ins, core_ids=list(range(N_CORES)),
                               trace=trace)
    cf = np.concatenate([res.results[i]["cfo"].reshape(8, 32, 129)
                         for i in range(N_CORES)], axis=1)
    return cf, res


def run_C(corr1, cf, raw, cw1, cb1, cw2, cb2, cw3, cb3, trace=False):
    if "C" not in _CACHE:
        _CACHE["C"] = patch_nc(build_C())
    # block-diag weights, layouts: in p=c*2+h, hid p=c*32+u*2+h, out p=c*2+h
    w1h = np.zeros((8, 9, 128), np.float32)
    w2h = np.zeros((128, 9, 128), np.float32)
    w3h = np.zeros((128, 9, 8), np.float32)
    b1h = np.zeros((128, 1), np.float32)
    b2h = np.zeros((128, 1), np.float32)
    b3h = np.zeros((8, 1), np.float32)
    for c in range(C):
        for h in range(2):
            b3h[c * 2 + h, 0] = cb3[c, 0]
            for uu in range(16):
                b1h[c * 32 + uu * 2 + h, 0] = cb1[c, uu]
                b2h[c * 32 + uu * 2 + h, 0] = cb2[c, uu]
    for t, (dy, dx) in enumerate([(a, b) for a in range(3) for b in range(3)]):
        for c in range(C):
            for h in range(2):
                for uu in range(16):
                    w1h[c * 2 + h, t, c * 32 + uu * 2 + h] = \
                        cw1[c, uu, 0, dy, dx]
                    w3h[c * 32 + uu * 2 + h, t, c * 2 + h] = \
                        cw3[c, 0, uu, dy, dx]
                    for v in range(16):
                        w2h[c * 32 + v * 2 + h, t, c * 32 + uu * 2 + h] = \
                            cw2[c, uu, v, dy, dx]
    # inverse DFT constants (same for all cores except ihs)
    kk = np.arange(129)
    w_ = np.arange(256)
    ck = np.where((kk == 0) | (kk == 128), 1.0, 2.0)
    th3 = 2 * np.pi * np.outer(kk, w_) / 256.0
    iwre = ck[:, None] * np.cos(th3) / 16.0
    iwim = ck[:, None] * np.sin(th3) / 16.0
    iwh = np.zeros((128, 2 * 512), np.float32)
    iwbh = np.zeros((1, 2 * 512), np.float32)
    for j, m in enumerate((iwre, iwim, -iwim, iwre)):
        iwh[:, j * 256:(j + 1) * 256] = m[:128]
        iwbh[0, j * 256:(j + 1) * 256] = m[128]
    # cfa [128 k, (c, ri, kc, 128 k1)], cfb k=128 row
    cfah = np.zeros((128, 4, 2, 2, 128), np.float32)
    cfbh = np.zeros((1, 4, 2, 2, 128), np.float32)
    for c in range(C):
        for ri in range(2):
            m = cf[ri * 4 + c]  # [256 k1, 129 k]
            for kc in range(2):
                cfah[:, c, ri, kc, :] = m[kc * 128:(kc + 1) * 128, :128].T
                cfbh[0, c, ri, kc, :] = m[kc * 128:(kc + 1) * 128, 128]
    ins = []
    for i in range(N_CORES):
        r0 = i * ROWS
        u0h = np.zeros((8, 22, WPC), np.float32)
        rawh = np.zeros((8, 16, WPC), np.float32)
        ihsh = np.zeros((128, 2, 2, 38), np.float32)
        hh = np.arange(r0 - 3, r0 + 35)
        ok = (hh >= 0) & (hh < 256)
        for kc in range(2):
            k1 = np.arange(kc * 128, kc * 128 + 128)
            th = 2 * np.pi * np.outer(k1, hh) / 256.0
            ihsh[:, kc, 0, :] = np.cos(th) / 16.0 * ok[None, :]
            ihsh[:, kc, 1, :] = -np.sin(th) / 16.0 * ok[None, :]
        for c in range(C):
            for h in range(2):
                lo = r0 + h * 16 - 3
                a, b = max(0, lo), min(256, lo + 22)
                u0h[c * 2 + h, a - lo:b - lo, 1:257] = corr1[c, a:b, :]
                rawh[c * 2 + h, :, 1:257] = \
                    raw[c, r0 + h * 16:r0 + h * 16 + 16, :] + cb3[c, 0]
        m20 = np.zeros((128, 20), np.float32)
        m18 = np.zeros((128, 18), np.float32)
        for p in range(128):
            h = p % 2
            base = r0 + h * 16
            for r in range(20):
                m20[p, r] = 1.0 if 0 <= base - 2 + r < 256 else 0.0
            for r in range(18):
                m18[p, r] = 1.0 if 0 <= base - 1 + r < 256 else 0.0
        ins.append({
            "u0": _bf(u0h.reshape(8, EXTC)),
            "raws": rawh.reshape(8, EXTR3).astype(np.float32),
            "cfa": _bf(cfah.reshape(128, -1)),
            "cfb": _bf(cfbh.reshape(1, -1)),
            "iwm": _bf(iwh), "iwbm": _bf(iwbh),
            "ihs": _bf(ihsh.reshape(128, -1)),
            "cw1": _bf(w1h.reshape(8, -1)), "cb1": b1h,
            "cw2": _bf(w2h.reshape(128, -1)), "cb2": b2h,
            "cw3": _bf(w3h.reshape(128, -1)), "cb3": b3h,
            "mr20": m20, "mr18": m18,
        })
    res = run_bass_kernel_spmd(_CACHE["C"], ins, core_ids=list(range(N_CORES)),
                               trace=trace)
    fin = np.zeros((C, H, W), np.float32)
    for i in range(N_CORES):
        f = res.results[i]["fin"].reshape(8, 16, 256)
        for c in range(C):
            for h in range(2):
                fin[c, i * ROWS + h * 16:i * ROWS + h * 16 + 16, :] = \
                    f[c * 2 + h]
    return fin, res


def kernel(**inputs):
    inputs = {k: np.asarray(v, dtype=np.float32) for k, v in inputs.items()}
    raw = inputs["raw_image"][0]
    feat = inputs["aberration_features"][0]
    corr1, _ = run_A(raw, feat,
                     inputs["pw1"], inputs["pb1"], inputs["pw2"], inputs["pb2"],
                     inputs["pw3"], inputs["pb3"])
    _, cf, _ = run_B(corr1, inputs["fw1"], inputs["fb1"], inputs["fw2"],
                     inputs["fb2"], inputs["fw3"], inputs["fb3"])
    fin, _ = run_C(corr1, cf, raw, inputs["cw1"], inputs["cb1"],
                   inputs["cw2"], inputs["cb2"], inputs["cw3"],
                   inputs["cb3"])
    return fin[None].astype(np.float32)




# revision 38
# speedup vs baseline: 1.0979x; 1.0979x over previous
"""Trainium2 Bass kernel for nn_AberrationCorrectionModule.

Reference pipeline:
  1. psf_predictor: 3x conv3x3 (128->256->128->900) on aberration_features,
     softmax over 225 taps per channel -> psf
  2. deconv: 15x15 spatially-varying weighted sum over reflect-padded raw
  3. freq corrector: rfft2 -> conv3x3 stack (8->64->64->8) -> irfft2, added
  4. per-channel refinement: 4 independent 1->16->16->1 conv stacks
  5. out = clip(raw + corrected, 0, 1)

Distribution: 8 NeuronCores, H-sharded (32 rows/core), SPMD dispatches with
host gather between (FFT stage needs full-image mixing).
"""
import json
import sys

sys.path.insert(0, "/opt/trn_rl_repo")

import ml_dtypes
import numpy as np

import bass_rust
import concourse.bass as bass
import concourse.tile as tile
from concourse import mybir
from concourse.bass_utils import run_bass_kernel_spmd

F32 = mybir.dt.float32
BF16 = mybir.dt.bfloat16
AF = mybir.ActivationFunctionType
ALU = mybir.AluOpType
AX = mybir.AxisListType

N_CORES = 8
C, H, W = 4, 256, 256
ROWS = H // N_CORES  # 32
KK = 15
PAD = KK // 2  # 7
WP = W + 2  # 258
TAPS = [(dy, dx) for dy in (-1, 0, 1) for dx in (-1, 0, 1)]


def _bf(x):
    return np.asarray(x, dtype=ml_dtypes.bfloat16)


def mkap(base_ap, offset, pairs):
    a = base_ap.copy()
    a.offset = offset
    a.ap = bass_rust.VecI64Pair([list(p) for p in pairs])
    return a


def _split_multiwaits(raw: bytes) -> bytes:
    """Workaround: this walrus build rejects >1 sync wait per instruction.
    Move extra waits onto NoOp carriers inserted just before the instruction."""
    m = json.loads(raw)
    ctr = 0
    for fn in m["functions"]:
        for bb in fn.get("blocks", []):
            insts = bb.get("instructions")
            if not insts:
                continue
            out = []
            for inst in insts:
                si = inst.get("sync_info")
                ow = (si or {}).get("on_wait") or []
                if len(ow) > 1:
                    for w in ow[:-1]:
                        out.append({
                            "debug": inst.get("debug", 0),
                            "engine": inst["engine"],
                            "ins": [], "outs": [],
                            "name": f"wsplit_{ctr}",
                            "opcode": "NoOp",
                            "sync_info": {"on_update": [], "on_wait": [w]},
                        })
                        ctr += 1
                    si["on_wait"] = [ow[-1]]
                out.append(inst)
            bb["instructions"] = out
    return json.dumps(m).encode()


def patch_nc(nc):
    orig = nc.to_json_bytes
    nc.to_json_bytes = lambda: _split_multiwaits(orig())
    return nc


def nchunks(total, step):
    out, o = [], 0
    while o < total:
        out.append((o, min(step, total - o)))
        o += step
    return out


# ================================================================ kernel A
# fp8 DoubleRow rewrite.
# conv1/conv2 feature-major on a 272-pitch grid (row pitch % 16 == 0 for
# DoubleRow lhsT k-tile strides). conv3 transposed: pixels on partitions,
# psf taps on the free axis (4ch x 228, 912 cols), softmax tail on
# vector/scalar engines. Patches pre-unfolded on host to [8192, 912].

RP = 272            # row pitch
EXTF = 38 * RP      # fb copy extent
EXTH1 = 36 * RP     # h1 half extent
EXTH2 = 34 * RP     # h2 copy extent
GD = 16             # leading guard cols
F8 = mybir.dt.float8e4
DR = mybir.MatmulPerfMode.DoubleRow
# conv tap pairs: 3 horizontal A/B-copy pairs, 1 vertical, 1 zero-padded
# (dy, dx) of kt0; kind 'AB' = kt1 from shifted copy (stride EXT),
# 'V' = kt1 one row down (stride RP), 'Z' = kt1 zero weights (stride RP)
PAIRS = [((-1, -1), 'AB'), ((0, -1), 'AB'), ((1, -1), 'AB'),
         ((-1, 1), 'V'), ((1, 1), 'Z')]


def pair_taps(p):
    """taps (as (dy,dx)) covered by pair p: (kt0, kt1 or None)."""
    (dy, dx), kind = PAIRS[p]
    if kind == 'AB':
        return (dy, dx), (dy, dx + 1)
    if kind == 'V':
        return (dy, dx), (dy + 1, dx)
    return (dy, dx), None


def build_A():
    nc = bass.Bass(trn_type="TRN2", name="kernA")
    fb = nc.dram_tensor("fb", (128, GD + 2 * EXTF), F8, kind="ExternalInput")
    w1 = nc.dram_tensor("w1", (128, 2 * 5 * 2 * 128), F8, kind="ExternalInput")
    b1 = nc.dram_tensor("b1", (128, 2), F32, kind="ExternalInput")
    w2 = nc.dram_tensor("w2", (128, 9 * 2 * 128), F8, kind="ExternalInput")
    b2 = nc.dram_tensor("b2", (128, 1), F32, kind="ExternalInput")
    w3 = nc.dram_tensor("w3", (128, 5 * 2 * 912), F8, kind="ExternalInput")
    xu = nc.dram_tensor("xu", (8192, 912), BF16, kind="ExternalInput")
    m36 = nc.dram_tensor("m36", (128, 36), F32, kind="ExternalInput")
    m34 = nc.dram_tensor("m34", (128, 34), F32, kind="ExternalInput")
    corr = nc.dram_tensor("corr", (64, 512), F32, kind="ExternalOutput")

    def win(tile_ap, off, stride, nl):
        return mkap_s(tile_ap, off, [[tile_ap.ap[0][0], 128], [stride, 2],
                                     [1, nl]])

    with tile.TileContext(nc) as tc:
        with tc.tile_pool(name="cst", bufs=1) as cst, \
             tc.tile_pool(name="hp", bufs=1) as hp, \
             tc.tile_pool(name="psum", bufs=2, space="PSUM") as psp:
            w3t = cst.tile([128, 5 * 2 * 912], F8)
            nc.sync.dma_start(w3t[:], w3[:])

            w2t = cst.tile([128, 9 * 2 * 128], F8)
            nc.sync.dma_start(w2t[:], w2[:])
            b2t = cst.tile([128, 1], F32)
            nc.sync.dma_start(b2t[:], b2[:])
            m34t = cst.tile([128, 34], F32)
            nc.sync.dma_start(m34t[:], m34[:])

            h2 = hp.tile([128, GD + 2 * EXTH2 + 144], F8)

            with tc.tile_pool(name="h1p", bufs=1) as h1p:
                h1 = h1p.tile([128, GD + 2 * EXTH1 + GD], F8)
                with tc.tile_pool(name="fp", bufs=1) as fp:
                    w1t = fp.tile([128, 2 * 5 * 2 * 128], F8)
                    nc.sync.dma_start(w1t[:], w1[:])
                    b1t = fp.tile([128, 2], F32)
                    nc.sync.dma_start(b1t[:], b1[:])
                    m36t = fp.tile([128, 36], F32)
                    nc.sync.dma_start(m36t[:], m36[:])
                    fbt = fp.tile([128, GD + 2 * EXTF], F8)
                    FB1 = GD + 13 * RP
                    FB2 = GD + 26 * RP
                    nc.sync.dma_start(fbt[:, :FB1], fb[:, :FB1])
                    nc.sync.dma_start(fbt[:, FB1:FB2], fb[:, FB1:FB2])
                    nc.sync.dma_start(fbt[:, FB2:], fb[:, FB2:])
                    w1v = w1t[:].rearrange("q (m p k c) -> q m p k c", m=2,
                                           p=5, k=2)

                    # conv1: 128 -> 256 (2 M halves), 5 DoubleRow passes
                    for m in range(2):
                        for n0, nl in nchunks(EXTH1, 512):
                            ps = psp.tile([128, 512], F32, tag="cv",
                                          name=f"c1_{m}_{n0}", bufs=2)
                            for p, ((dy, dx), kind) in enumerate(PAIRS):
                                off = GD + n0 + (1 + dy) * RP + dx
                                st = EXTF if kind == 'AB' else RP
                                nc.tensor.matmul(
                                    ps[:, :nl], lhsT=w1v[:, m, p, :, :],
                                    rhs=win(fbt[:], off, st, nl),
                                    start=(p == 0), stop=(p == 4),
                                    perf_mode=DR)
                            nc.scalar.activation(
                                h1[:, GD + m * EXTH1 + n0:
                                   GD + m * EXTH1 + n0 + nl],
                                ps[:, :nl], AF.Relu, bias=b1t[:, m:m + 1])
                    for m in range(2):
                        h3 = h1[:, GD + m * EXTH1:GD + (m + 1) * EXTH1] \
                            .rearrange("q (r c) -> q r c", r=36)
                        nc.vector.memset(h3[:, :, 0:1], 0.0)
                        nc.vector.memset(h3[:, :, 257:258], 0.0)
                        for r in (0, 1, 34, 35):
                            nc.vector.tensor_scalar_mul(
                                h3[:, r, :], h3[:, r, :], m36t[:, r:r + 1])

                # conv2: 256 -> 128, 9 DoubleRow passes over kc halves
                w2v = w2t[:].rearrange("q (t k c) -> q t k c", t=9, k=2)
                for n0, nl in nchunks(EXTH2, 512):
                    ps = psp.tile([128, 512], F32, tag="cv",
                                  name=f"c2_{n0}", bufs=2)
                    for t, (dy, dx) in enumerate(TAPS):
                        off = GD + n0 + (1 + dy) * RP + dx
                        nc.tensor.matmul(
                            ps[:, :nl], lhsT=w2v[:, t, :, :],
                            rhs=win(h1[:], off, EXTH1, nl),
                            start=(t == 0), stop=(t == 8), perf_mode=DR)
                    nc.scalar.activation(
                        h2[:, GD + n0:GD + n0 + nl], ps[:, :nl], AF.Relu,
                        bias=b2t[:])
                h23 = h2[:, GD:GD + EXTH2].rearrange("q (r c) -> q r c", r=34)
                nc.vector.memset(h23[:, :, 0:1], 0.0)
                nc.vector.memset(h23[:, :, 257:258], 0.0)
                for r in (0, 33):
                    nc.vector.tensor_scalar_mul(
                        h23[:, r, :], h23[:, r, :], m34t[:, r:r + 1])

            # shifted copy for conv3 lhsT k-tile pairing (copy1[x]=copy0[x+1])
            BND = 9 * RP
            for bb in range(4):
                a0 = bb * BND
                a1 = min(EXTH2 - 1, a0 + BND)
                nc.sync.dma_start(h2[:, GD + EXTH2 + a0:GD + EXTH2 + a1],
                                  h2[:, GD + 1 + a0:GD + 1 + a1])
            # ones region for the bias k-tile of conv3 pass 4
            OB = GD + 2 * EXTH2 + 2
            nc.vector.memset(h2[:, OB:OB + 128], 1.0)

            # conv3 transposed + softmax tail, per 128-pixel group.
            # bias lands in psum via a K=1 ones-matmul; exp(b3) is folded
            # into xu on host; D comes free from exp accum_out. Division
            # and output DMA are batched over 8 groups.
            w3v = w3t[:].rearrange("q (p k c) -> q p k c", p=5, k=2)
            GB = 8
            with tc.tile_pool(name="gp", bufs=4) as gp, \
                 tc.tile_pool(name="bp", bufs=2) as bp:
                for g in range(64):
                    r, cc = g // 2, g % 2
                    gi = g % GB
                    if gi == 0:
                        Ns = bp.tile([128, GB * 4], F32, tag="Ns",
                                     name=f"Ns{g}")
                        Ds = bp.tile([128, GB * 4], F32, tag="Ds",
                                     name=f"Ds{g}")
                    Xg = gp.tile([128, 912], BF16, tag="Xg", name=f"Xg{g}")
                    xq = nc.sync if g % 2 == 0 else nc.gpsimd
                    xq.dma_start(Xg[:], xu[g * 128:(g + 1) * 128, :])
                    pss = [psp.tile([128, 456], F32, tag=f"c3{j}",
                                    name=f"c3_{g}_{j}", bufs=3)
                           for j in range(2)]
                    for p, ((dy, dx), kind) in enumerate(PAIRS):
                        off = GD + (r + 1 + dy) * RP + cc * 128 + 1 + dx
                        if kind == 'AB':
                            st = EXTH2
                        elif kind == 'V':
                            st = RP
                        else:  # Z: kt1 = ones region (bias via w3 row 0)
                            st = OB - off
                        for j in range(2):
                            nc.tensor.matmul(
                                pss[j][:],
                                lhsT=win(h2[:], off, st, 128),
                                rhs=w3v[:, p, :, j * 456:(j + 1) * 456],
                                start=(p == 0), stop=(p == 4), perf_mode=DR)
                    E = gp.tile([128, 912], BF16, tag="E", name=f"E{g}")
                    for c in range(4):
                        nc.scalar.activation(
                            E[:, c * 228:(c + 1) * 228],
                            pss[c // 2][:, (c % 2) * 228:(c % 2) * 228 + 228],
                            AF.Exp, accum_out=Ds[:, gi * 4 + c:gi * 4 + c + 1])
                    Pt = gp.tile([128, 912], BF16, tag="Pt", name=f"Pt{g}")
                    nc.vector.tensor_tensor(out=Pt[:], in0=E[:], in1=Xg[:],
                                            op=ALU.mult)
                    nc.vector.tensor_reduce(
                        Ns[:, gi * 4:gi * 4 + 4],
                        Pt[:].rearrange("q (a b) -> q a b", a=4),
                        AX.X, ALU.add)
                    if gi == GB - 1:
                        nc.vector.reciprocal(Ds[:], Ds[:])
                        nc.vector.tensor_tensor(out=Ns[:], in0=Ns[:],
                                                in1=Ds[:], op=ALU.mult)
                        nc.sync.dma_start(
                            mkap(corr[:], (g - GB + 1) * 512,
                                 [[1, 128], [512, GB], [128, 4]]), Ns[:])
    return nc


def mkap_s(base_ap, off, pairs):
    a = base_ap.copy()
    a.offset = base_ap.offset + off
    a.ap = bass_rust.VecI64Pair([list(p) for p in pairs])
    return a


def build_A_old():
    nc = bass.Bass(trn_type="TRN2", name="kernA")
    feat = nc.dram_tensor("feat", (128, 38 * 256), F32, kind="ExternalInput")
    raw46 = nc.dram_tensor("raw46", (C, 46, 270), BF16, kind="ExternalInput")
    w1 = nc.dram_tensor("w1", (128, 2 * 9 * 128), BF16, kind="ExternalInput")
    b1 = nc.dram_tensor("b1", (128, 2), F32, kind="ExternalInput")
    w2 = nc.dram_tensor("w2", (128, 2 * 9 * 128), BF16, kind="ExternalInput")
    b2 = nc.dram_tensor("b2", (128, 1), F32, kind="ExternalInput")
    w3 = nc.dram_tensor("w3", (128, 9 * 1024), BF16, kind="ExternalInput")
    b3 = nc.dram_tensor("b3", (128, 8), F32, kind="ExternalInput")
    m36 = nc.dram_tensor("m36", (128, 36), F32, kind="ExternalInput")
    m34 = nc.dram_tensor("m34", (128, 34), F32, kind="ExternalInput")
    corr = nc.dram_tensor("corr", (C, ROWS, W), F32, kind="ExternalOutput")

    NF36, NF34 = 36 * WP, 34 * WP

    with tile.TileContext(nc) as tc:
        with tc.tile_pool(name="cst", bufs=1) as cst, \
             tc.tile_pool(name="hp", bufs=1) as hp, \
             tc.tile_pool(name="psum", bufs=2, space="PSUM") as psp:
            w3t = cst.tile([128, 9 * 1024], BF16)
            nc.sync.dma_start(w3t[:], w3[:])
            b3t = cst.tile([128, 8], F32)
            nc.sync.dma_start(b3t[:], b3[:])
            b2t = cst.tile([128, 1], F32)
            nc.sync.dma_start(b2t[:], b2[:])
            m34t = cst.tile([128, 34], F32)
            nc.sync.dma_start(m34t[:], m34[:])
            ones = cst.tile([128, 1], BF16)
            nc.vector.memset(ones[:], 1.0)

            h2 = hp.tile([128, NF34], BF16)

            with tc.tile_pool(name="h1p", bufs=1) as h1p:
                h1 = [h1p.tile([128, NF36 + 8], BF16, name=f"h1_{m}", tag=f"h1_{m}") for m in range(2)]
                w2t = h1p.tile([128, 2 * 9 * 128], BF16)
                nc.sync.dma_start(w2t[:], w2[:])

                with tc.tile_pool(name="fp", bufs=1) as fp:
                    w1t = fp.tile([128, 2 * 9 * 128], BF16)
                    nc.sync.dma_start(w1t[:], w1[:])
                    b1t = fp.tile([128, 2], F32)
                    nc.sync.dma_start(b1t[:], b1[:])
                    m36t = fp.tile([128, 36], F32)
                    nc.sync.dma_start(m36t[:], m36[:])
                    ff = fp.tile([128, 38 * 256], F32)
                    nc.sync.dma_start(ff[:], feat[:])
                    fb = fp.tile([128, 38 * WP + 8], BF16)
                    nc.vector.memset(fb[:], 0.0)
                    nc.vector.tensor_copy(
                        fb[:, 1:1 + 38 * WP].rearrange(
                            "p (r c) -> p r c", r=38)[:, :, 1:257],
                        ff[:].rearrange("p (r c) -> p r c", r=38))

                    # conv1: 128 -> 256 (2 M chunks), taps-outer groups of 3
                    for m in range(2):
                        ch1 = nchunks(NF36, 512)
                        for g0 in range(0, len(ch1), 3):
                            grp = ch1[g0:g0 + 3]
                            pcs = [psp.tile([128, 512], F32, tag=f"pc{j}",
                                            name=f"c1_{m}_{g0}_{j}", bufs=1)
                                   for j in range(len(grp))]
                            for t, (dy, dx) in enumerate(TAPS):
                                base = (1 + dy) * WP + dx
                                for j, (n0, nl) in enumerate(grp):
                                    nc.tensor.matmul(
                                        pcs[j][:, :nl],
                                        lhsT=w1t[:, (m * 9 + t) * 128:(m * 9 + t + 1) * 128],
                                        rhs=fb[:, 1 + n0 + base:1 + n0 + base + nl],
                                        start=(t == 0), stop=(t == 8))
                            for j, (n0, nl) in enumerate(grp):
                                nc.scalar.activation(
                                    h1[m][:, 1 + n0:1 + n0 + nl], pcs[j][:, :nl],
                                    AF.Relu, bias=b1t[:, m:m + 1])
                        h3 = h1[m][:, 1:1 + NF36].rearrange("p (r c) -> p r c", r=36)
                        nc.vector.memset(h3[:, :, 0:1], 0.0)
                        nc.vector.memset(h3[:, :, 257:258], 0.0)
                        # zero out-of-image rows (only rows 0,1,34,35 can be OOI)
                        for r in (0, 1, 34, 35):
                            nc.vector.tensor_scalar_mul(
                                h3[:, r, :], h3[:, r, :], m36t[:, r:r + 1])

                # conv2: 256 -> 128 (2 K chunks), taps-outer groups of 3
                ch2 = nchunks(NF34, 512)
                for g0 in range(0, len(ch2), 3):
                    grp = ch2[g0:g0 + 3]
                    pcs = [psp.tile([128, 512], F32, tag=f"pc{j}",
                                    name=f"c2_{g0}_{j}", bufs=1)
                           for j in range(len(grp))]
                    ti = 0
                    for kc in range(2):
                        for t, (dy, dx) in enumerate(TAPS):
                            base = (1 + dy) * WP + dx
                            for j, (n0, nl) in enumerate(grp):
                                nc.tensor.matmul(
                                    pcs[j][:, :nl],
                                    lhsT=w2t[:, (kc * 9 + t) * 128:(kc * 9 + t + 1) * 128],
                                    rhs=h1[kc][:, 1 + n0 + base:1 + n0 + base + nl],
                                    start=(ti == 0), stop=(ti == 17))
                            ti += 1
                    for j, (n0, nl) in enumerate(grp):
                        nc.scalar.activation(
                            h2[:, n0:n0 + nl], pcs[j][:, :nl], AF.Relu, bias=b2t[:])
                h23 = h2[:].rearrange("p (r c) -> p r c", r=34)
                nc.vector.memset(h23[:, :, 0:1], 0.0)
                nc.vector.memset(h23[:, :, 257:258], 0.0)
                for r in (0, 33):
                    nc.vector.tensor_scalar_mul(
                        h23[:, r, :], h23[:, r, :], m34t[:, r:r + 1])

            # conv3 + softmax + deconv per (pixchunk, channel).
            # psf channels padded 900->1024: image channel c = M-chunks
            # {2c, 2c+1}; taps 0..224 real, 225..255 padded (bias -30).
            RPC = 8
            PCN = RPC * W  # 2048
            h2v = h2[:].rearrange("p (r q) -> p r q", r=34)
            with tc.tile_pool(name="ex", bufs=2) as exp_pool, \
                 tc.tile_pool(name="xp", bufs=2) as xpool, \
                 tc.tile_pool(name="scp", bufs=2) as scp, \
                 tc.tile_pool(name="dnp", bufs=2, space="DRAM") as dnp, \
                 tc.tile_pool(name="rbp", bufs=2) as rbp:
                for pc_i in range(ROWS // RPC):
                    r0 = pc_i * RPC
                    dnd = dnp.tile([C, 2 * PCN], F32, tag="dnd")
                    for c in range(C):
                        Ea = exp_pool.tile([128, PCN], BF16, tag="Ea")
                        Eb = exp_pool.tile([128, PCN], BF16, tag="Eb")
                        Pa = exp_pool.tile([128, PCN], BF16, tag="Pa")
                        Pb = exp_pool.tile([128, PCN], BF16, tag="Pb")
                        Xa = xpool.tile([128, PCN], BF16, tag="Xa")
                        Xb = xpool.tile([128, PCN], BF16, tag="Xb")
                        # patch strips: partition t = dy*15+dx, free = pixel
                        for dy in range(KK):
                            t0 = dy * KK
                            off = c * 46 * 270 + (r0 + dy) * 270
                            if t0 + KK <= 128:
                                nc.sync.dma_start(
                                    Xa[t0:t0 + KK, :],
                                    mkap(raw46[:], off, [[1, KK], [270, RPC], [1, W]]))
                            elif t0 >= 128:
                                nc.sync.dma_start(
                                    Xb[t0 - 128:t0 - 128 + KK, :],
                                    mkap(raw46[:], off, [[1, KK], [270, RPC], [1, W]]))
                            else:
                                n1 = 128 - t0
                                nc.sync.dma_start(
                                    Xa[t0:128, :],
                                    mkap(raw46[:], off, [[1, n1], [270, RPC], [1, W]]))
                                nc.sync.dma_start(
                                    Xb[0:KK - n1, :],
                                    mkap(raw46[:], off + n1,
                                         [[1, KK - n1], [270, RPC], [1, W]]))
                        # conv3 -> exp (bias fused into exp's activation)
                        for half, E in ((0, Ea), (1, Eb)):
                            mc = c * 2 + half
                            chunks = nchunks(PCN, 512)
                            pss = [psp.tile([128, 512], F32, tag=f"pc{j}",
                                            name=f"ps_{mc}_{j}", bufs=1)
                                   for j in range(len(chunks))]
                            for t, (dy, dx) in enumerate(TAPS):
                                for j, (s0, sl) in enumerate(chunks):
                                    rr = r0 + s0 // W + 1 + dy
                                    nc.tensor.matmul(
                                        pss[j][:, :sl],
                                        lhsT=w3t[:, t * 1024 + mc * 128:
                                                 t * 1024 + (mc + 1) * 128],
                                        rhs=h2v[:, rr:rr + 2, 1 + dx:257 + dx],
                                        start=(t == 0), stop=(t == 8))
                            for j, (s0, sl) in enumerate(chunks):
                                nc.scalar.activation(
                                    E[:, s0:s0 + sl], pss[j][:, :sl], AF.Exp,
                                    bias=b3t[:, mc:mc + 1])
                        # tap sums via ones-matmuls on PE (GPSIMD C-reduce
                        # is ~40us/op; PE does it in ~0.2us/chunk)
                        nc.vector.tensor_tensor(out=Pa[:, :], in0=Ea[:, :], in1=Xa[:, :], op=ALU.mult)
                        nc.vector.tensor_tensor(out=Pb[0:97, :], in0=Eb[0:97, :], in1=Xb[0:97, :], op=ALU.mult)
                        sc = scp.tile([1, 2 * PCN], F32, tag="sc")
                        da, na = sc[:, 0:PCN], sc[:, PCN:2 * PCN]
                        for s0, sl in nchunks(PCN, 512):
                            for dst, ta, tb in ((da, Ea, Eb), (na, Pa, Pb)):
                                pr = psp.tile([1, 512], F32, tag="pr", bufs=2)
                                nc.tensor.matmul(pr[:, :sl], lhsT=ones[:, :],
                                                 rhs=ta[:, s0:s0 + sl],
                                                 start=True, stop=False)
                                nc.tensor.matmul(pr[:, :sl], lhsT=ones[0:97, :],
                                                 rhs=tb[0:97, s0:s0 + sl],
                                                 start=False, stop=True)
                                nc.vector.tensor_copy(dst[:, s0:s0 + sl], pr[:, :sl])
                        nc.sync.dma_start(dnd[c, :], sc[:, :])
                    # reshape [1,2048]x2 per ch -> [128,64] so the divide
                    # runs on all 128 lanes instead of one
                    Dt = rbp.tile([128, 64], F32, tag="Dt")
                    Nt = rbp.tile([128, 64], F32, tag="Nt")
                    for c in range(C):
                        nc.sync.dma_start(
                            Dt[32 * c:32 * c + 32, :],
                            mkap(dnd[:], c * 2 * PCN, [[64, 32], [1, 64]]))
                        nc.sync.dma_start(
                            Nt[32 * c:32 * c + 32, :],
                            mkap(dnd[:], c * 2 * PCN + PCN, [[64, 32], [1, 64]]))
                    nc.vector.reciprocal(Dt[:], Dt[:])
                    nc.vector.tensor_tensor(out=Nt[:], in0=Nt[:], in1=Dt[:], op=ALU.mult)
                    nc.sync.dma_start(corr[:, r0:r0 + RPC, :], Nt[:])
    return nc




# ================================================================ kernel B1
# Forward rfft2 via DFT matmuls, replicated on every core; writes full fri.
# V[h,k] = sum_w x[h,w] Fw[w,k];  Y[k1,k] = sum_h Fh[k1,h] V[h,k]
# fri = [Yre(4ch), Yim(4ch)] as [8, 256, 129].

def build_B1():
    nc = bass.Bass(trn_type="TRN2", name="kernB1")
    corrT = nc.dram_tensor("corrT", (C, 256, 256), BF16, kind="ExternalInput")
    fwre = nc.dram_tensor("fwre", (256, 129), BF16, kind="ExternalInput")
    fwim = nc.dram_tensor("fwim", (256, 129), BF16, kind="ExternalInput")
    fhre = nc.dram_tensor("fhre", (256, 256), BF16, kind="ExternalInput")
    fhim = nc.dram_tensor("fhim", (256, 256), BF16, kind="ExternalInput")
    fhimn = nc.dram_tensor("fhimn", (256, 256), BF16, kind="ExternalInput")
    fri = nc.dram_tensor("fri", (8, 256, 129), F32, kind="ExternalOutput")

    with tile.TileContext(nc) as tc:
        with tc.tile_pool(name="cst", bufs=1) as cst, \
             tc.tile_pool(name="wk", bufs=2) as wk, \
             tc.tile_pool(name="ps", bufs=4, space="PSUM") as psp:
            fw = [cst.tile([128, 2 * 129], BF16, name=f"fw_{i}", tag=f"fw_{i}") for i in range(2)]
            for kc in range(2):
                nc.sync.dma_start(fw[kc][:, 0:129], fwre[kc * 128:(kc + 1) * 128, :])
                nc.sync.dma_start(fw[kc][:, 129:258], fwim[kc * 128:(kc + 1) * 128, :])
            fh = [cst.tile([128, 3 * 256], BF16, name=f"fh_{i}", tag=f"fh_{i}") for i in range(2)]
            for kc in range(2):
                nc.sync.dma_start(fh[kc][:, 0:256], fhre[kc * 128:(kc + 1) * 128, :])
                nc.sync.dma_start(fh[kc][:, 256:512], fhim[kc * 128:(kc + 1) * 128, :])
                nc.sync.dma_start(fh[kc][:, 512:768], fhimn[kc * 128:(kc + 1) * 128, :])
            for c in range(C):
                xT = [wk.tile([128, 256], BF16, name=f"xT{i}", tag=f"xT{i}") for i in range(2)]
                for kc in range(2):
                    nc.sync.dma_start(xT[kc][:], corrT[c, kc * 128:(kc + 1) * 128, :])
                V = [wk.tile([128, 2 * 129], BF16, name=f"V{i}", tag=f"V{i}") for i in range(2)]
                for mc in range(2):      # output h chunk
                    for ri in range(2):  # re / im
                        pv = psp.tile([128, 129], F32, tag="pv")
                        for kc in range(2):
                            nc.tensor.matmul(
                                pv[:, :],
                                lhsT=xT[kc][:, mc * 128:(mc + 1) * 128],
                                rhs=fw[kc][:, ri * 129:(ri + 1) * 129],
                                start=(kc == 0), stop=(kc == 1))
                        nc.vector.tensor_copy(V[mc][:, ri * 129:(ri + 1) * 129], pv[:, :])
                # Y: for re out: FhRe@Vre + FhImNeg@Vim ; im out: FhIm@Vre + FhRe@Vim
                for mc in range(2):      # k1 chunk
                    for ri in range(2):  # re / im output
                        py = psp.tile([128, 129], F32, tag="pv")
                        for kc in range(2):
                            if ri == 0:
                                t1, t2 = 0, 512   # re, imneg
                            else:
                                t1, t2 = 256, 0   # im, re
                            nc.tensor.matmul(
                                py[:, :],
                                lhsT=fh[kc][:, t1 + mc * 128:t1 + (mc + 1) * 128],
                                rhs=V[kc][:, 0:129],
                                start=(kc == 0), stop=False)
                            nc.tensor.matmul(
                                py[:, :],
                                lhsT=fh[kc][:, t2 + mc * 128:t2 + (mc + 1) * 128],
                                rhs=V[kc][:, 129:258],
                                start=False, stop=(kc == 1))
                        ys = wk.tile([128, 129], F32, tag="ys")
                        nc.scalar.activation(ys[:], py[:], AF.Copy)
                        nc.sync.dma_start(
                            fri[ri * 4 + c, mc * 128:(mc + 1) * 128, :], ys[:])
    return nc


# ================================================================ kernel B2
# freq conv stack on fri slab (38 rows, ch-major) + partial inverse fft.
WF = 131  # 129 + 2 pad cols

def build_B2():
    nc = bass.Bass(trn_type="TRN2", name="kernB2")
    fri = nc.dram_tensor("fri", (8, 38 * WF), BF16, kind="ExternalInput")
    gw1 = nc.dram_tensor("gw1", (8, 9 * 64), BF16, kind="ExternalInput")
    gb1 = nc.dram_tensor("gb1", (64, 1), F32, kind="ExternalInput")
    gw2 = nc.dram_tensor("gw2", (64, 9 * 64), BF16, kind="ExternalInput")
    gb2 = nc.dram_tensor("gb2", (64, 1), F32, kind="ExternalInput")
    gw3 = nc.dram_tensor("gw3", (64, 9 * 8), BF16, kind="ExternalInput")
    gb3 = nc.dram_tensor("gb3", (8, 1), F32, kind="ExternalInput")
    mf36 = nc.dram_tensor("mf36", (64, 36), F32, kind="ExternalInput")
    mf34 = nc.dram_tensor("mf34", (64, 34), F32, kind="ExternalInput")
    cfo = nc.dram_tensor("cfo", (8, 32 * 129), F32, kind="ExternalOutput")

    N36, N34, N32 = 36 * WF, 34 * WF, 32 * WF

    with tile.TileContext(nc) as tc:
        with tc.tile_pool(name="cst", bufs=1) as cst, \
             tc.tile_pool(name="gp", bufs=1) as gp, \
             tc.tile_pool(name="ps", bufs=4, space="PSUM") as psp:
            w1t = cst.tile([8, 9 * 64], BF16)
            nc.sync.dma_start(w1t[:], gw1[:])
            w2t = cst.tile([64, 9 * 64], BF16)
            nc.sync.dma_start(w2t[:], gw2[:])
            w3t = cst.tile([64, 9 * 8], BF16)
            nc.sync.dma_start(w3t[:], gw3[:])
            b1t = cst.tile([64, 1], F32)
            nc.sync.dma_start(b1t[:], gb1[:])
            b2t = cst.tile([64, 1], F32)
            nc.sync.dma_start(b2t[:], gb2[:])
            b3t = cst.tile([8, 1], F32)
            nc.sync.dma_start(b3t[:], gb3[:])
            m36t = cst.tile([64, 36], F32)
            nc.sync.dma_start(m36t[:], mf36[:])
            m34t = cst.tile([64, 34], F32)
            nc.sync.dma_start(m34t[:], mf34[:])

            ft = gp.tile([8, 1 + 38 * WF + 4], BF16)
            nc.sync.dma_start(ft[:, 1:1 + 38 * WF], fri[:, :])
            g1 = gp.tile([64, 1 + N36 + 4], BF16)
            g2 = gp.tile([64, 1 + N34 + 4], BF16)
            g3 = gp.tile([8, N32], F32)

            for n0, nl in nchunks(N36, 512):
                pc = psp.tile([64, 512], F32, tag="pg")
                for t, (dy, dx) in enumerate(TAPS):
                    base = (1 + dy) * WF + dx
                    nc.tensor.matmul(
                        pc[:, :nl],
                        lhsT=w1t[:, t * 64:(t + 1) * 64],
                        rhs=ft[:, 1 + n0 + base:1 + n0 + base + nl],
                        start=(t == 0), stop=(t == 8))
                nc.scalar.activation(g1[:, 1 + n0:1 + n0 + nl], pc[:, :nl],
                                     AF.Relu, bias=b1t[:])
            g1v = g1[:, 1:1 + N36].rearrange("p (r q) -> p r q", r=36)
            nc.vector.memset(g1v[:, :, 0:1], 0.0)
            nc.vector.memset(g1v[:, :, 130:131], 0.0)
            for r in (0, 1, 34, 35):
                nc.vector.tensor_scalar_mul(g1v[:, r, :], g1v[:, r, :],
                                            m36t[:, r:r + 1])
            for n0, nl in nchunks(N34, 512):
                pc = psp.tile([64, 512], F32, tag="pg")
                for t, (dy, dx) in enumerate(TAPS):
                    base = (1 + dy) * WF + dx
                    nc.tensor.matmul(
                        pc[:, :nl],
                        lhsT=w2t[:, t * 64:(t + 1) * 64],
                        rhs=g1[:, 1 + n0 + base:1 + n0 + base + nl],
                        start=(t == 0), stop=(t == 8))
                nc.scalar.activation(g2[:, 1 + n0:1 + n0 + nl], pc[:, :nl],
                                     AF.Relu, bias=b2t[:])
            g2v = g2[:, 1:1 + N34].rearrange("p (r q) -> p r q", r=34)
            nc.vector.memset(g2v[:, :, 0:1], 0.0)
            nc.vector.memset(g2v[:, :, 130:131], 0.0)
            for r in (0, 33):
                nc.vector.tensor_scalar_mul(g2v[:, r, :], g2v[:, r, :],
                                            m34t[:, r:r + 1])
            for n0, nl in nchunks(N32, 512):
                pc = psp.tile([8, 512], F32, tag="pg")
                for t, (dy, dx) in enumerate(TAPS):
                    base = (1 + dy) * WF + dx
                    nc.tensor.matmul(
                        pc[:, :nl],
                        lhsT=w3t[:, t * 8:(t + 1) * 8],
                        rhs=g2[:, 1 + n0 + base:1 + n0 + base + nl],
                        start=(t == 0), stop=(t == 8))
                nc.scalar.activation(g3[:, n0:n0 + nl], pc[:, :nl],
                                     AF.Copy, bias=0.0)
            # add bias gb3 separately (Copy cannot take AP bias)
            nc.vector.tensor_scalar(out=g3[:], in0=g3[:], scalar1=b3t[:],
                                    scalar2=None, op0=ALU.add)

            # write CF slab [8 (ri,c), 32 k1-rows, 129] (strip pad cols;
            # real bins live at cols 1..129 of the WF=131 grid)
            nc.sync.dma_start(
                cfo[:, :], mkap_s(g3[:], 1, [[N32, 8], [WF, 32], [1, 129]]))
    return nc


# ================================================================ kernel B
# merged forward DFT + freq convs, one dispatch. V (row FFT) needs all
# columns of the full image (replicated); Y (col FFT) computed only for
# this core's 38-row k1 slab; freq convs 2-half row-packed (bf16).
# partition layouts: ft/g3: p = (ri*4+c)*2 + h; g1/g2: p = u*2 + h.

def build_B():
    nc = bass.Bass(trn_type="TRN2", name="kernB")
    corrT = nc.dram_tensor("corrT", (C, 256, 256), BF16, kind="ExternalInput")
    fwre = nc.dram_tensor("fwre", (256, 129), BF16, kind="ExternalInput")
    fwim = nc.dram_tensor("fwim", (256, 129), BF16, kind="ExternalInput")
    fhs = nc.dram_tensor("fhs", (128, 2 * 3 * 38), BF16, kind="ExternalInput")
    gw1 = nc.dram_tensor("gw1", (16, 9 * 128), BF16, kind="ExternalInput")
    gb1 = nc.dram_tensor("gb1", (128, 1), F32, kind="ExternalInput")
    gw2 = nc.dram_tensor("gw2", (128, 9 * 128), BF16, kind="ExternalInput")
    gb2 = nc.dram_tensor("gb2", (128, 1), F32, kind="ExternalInput")
    gw3 = nc.dram_tensor("gw3", (128, 9 * 16), BF16, kind="ExternalInput")
    gb3 = nc.dram_tensor("gb3", (16, 1), F32, kind="ExternalInput")
    mf20 = nc.dram_tensor("mf20", (128, 20), F32, kind="ExternalInput")
    mf18 = nc.dram_tensor("mf18", (128, 18), F32, kind="ExternalInput")
    cfo = nc.dram_tensor("cfo", (8, 32 * 129), F32, kind="ExternalOutput")

    EXB = 22 * WF       # ft half extent (22 rows x 131)
    EXB1 = 20 * WF
    EXB2 = 18 * WF
    EXB3 = 16 * WF

    with tile.TileContext(nc) as tc:
        with tc.tile_pool(name="cst", bufs=1) as cst, \
             tc.tile_pool(name="gp", bufs=1) as gp, \
             tc.tile_pool(name="ps", bufs=2, space="PSUM") as psp:
            fw = cst.tile([128, 2 * 2 * 129], BF16)
            for kc in range(2):
                nc.sync.dma_start(fw[:, kc * 258:kc * 258 + 129],
                                  fwre[kc * 128:(kc + 1) * 128, :])
                nc.sync.dma_start(fw[:, kc * 258 + 129:kc * 258 + 258],
                                  fwim[kc * 128:(kc + 1) * 128, :])
            fhst = cst.tile([128, 2 * 3 * 38], BF16)
            nc.sync.dma_start(fhst[:], fhs[:])
            fhsv = fhst[:].rearrange("q (k m h) -> q k m h", k=2, m=3)
            w1t = cst.tile([16, 9 * 128], BF16)
            nc.sync.dma_start(w1t[:], gw1[:])
            w2t = cst.tile([128, 9 * 128], BF16)
            nc.sync.dma_start(w2t[:], gw2[:])
            w3t = cst.tile([128, 9 * 16], BF16)
            nc.sync.dma_start(w3t[:], gw3[:])
            b1t = cst.tile([128, 1], F32)
            nc.sync.dma_start(b1t[:], gb1[:])
            b2t = cst.tile([128, 1], F32)
            nc.sync.dma_start(b2t[:], gb2[:])
            b3t = cst.tile([16, 1], F32)
            nc.sync.dma_start(b3t[:], gb3[:])
            m20t = cst.tile([128, 20], F32)
            nc.sync.dma_start(m20t[:], mf20[:])
            m18t = cst.tile([128, 18], F32)
            nc.sync.dma_start(m18t[:], mf18[:])

            ft = gp.tile([16, GD + EXB + GD], BF16)
            nc.vector.memset(ft[:], 0.0)
            with tc.tile_pool(name="vp", bufs=3) as vp:
                for c in range(C):
                    xT = vp.tile([128, 2 * 256], BF16, tag="xT",
                                 name=f"xT{c}")
                    for kc in range(2):
                        nc.sync.dma_start(
                            xT[:, kc * 256:(kc + 1) * 256],
                            corrT[c, kc * 128:(kc + 1) * 128, :])
                    V = [vp.tile([128, 2 * 129], BF16, name=f"V{c}_{m}",
                                 tag=f"V{m}") for m in range(2)]
                    for mc in range(2):
                        pv = psp.tile([128, 258], F32, tag="pv",
                                      name=f"pv{c}_{mc}", bufs=2)
                        for kc in range(2):
                            nc.tensor.matmul(
                                pv[:],
                                lhsT=xT[:, kc * 256 + mc * 128:
                                        kc * 256 + (mc + 1) * 128],
                                rhs=fw[:, kc * 258:(kc + 1) * 258],
                                start=(kc == 0), stop=(kc == 1))
                        nc.scalar.activation(V[mc][:], pv[:], AF.Copy)
                    for ri in range(2):
                        py = psp.tile([38, 129], F32, tag="py",
                                      name=f"py{c}_{ri}", bufs=1)
                        t1, t2 = (0, 2) if ri == 0 else (1, 0)
                        ti = 0
                        for hc in range(2):
                            nc.tensor.matmul(
                                py[:], lhsT=fhsv[:, hc, t1, :],
                                rhs=V[hc][:, 0:129],
                                start=(ti == 0), stop=False)
                            ti += 1
                            nc.tensor.matmul(
                                py[:], lhsT=fhsv[:, hc, t2, :],
                                rhs=V[hc][:, 129:258],
                                start=False, stop=(ti == 3))
                            ti += 1
                        ys = vp.tile([38, 129], BF16, tag="ys",
                                     name=f"ys{c}_{ri}")
                        nc.scalar.activation(ys[:], py[:], AF.Copy)
                        for h in range(2):
                            p = (ri * 4 + c) * 2 + h
                            q = nc.sync if h == 0 else nc.gpsimd
                            q.dma_start(
                                mkap_s(ft[p:p + 1, :], GD + 1,
                                       [[GD + EXB + GD, 1], [WF, 22],
                                        [1, 129]]),
                                ys[h * 16:h * 16 + 22, :])

            g1 = gp.tile([128, GD + EXB1 + GD], BF16)
            g2 = gp.tile([128, GD + EXB2 + GD], BF16)
            w1v = w1t[:].rearrange("q (t c) -> q t c", t=9)
            for n0, nl in nchunks(EXB1, 512):
                ps = psp.tile([128, 512], F32, tag="cv", name=f"e1_{n0}",
                              bufs=3)
                for t, (dy, dx) in enumerate(TAPS):
                    off = GD + n0 + (1 + dy) * WF + dx
                    nc.tensor.matmul(ps[:, :nl], lhsT=w1v[:, t, :],
                                     rhs=ft[:, off:off + nl],
                                     start=(t == 0), stop=(t == 8))
                nc.scalar.activation(g1[:, GD + n0:GD + n0 + nl], ps[:, :nl],
                                     AF.Relu, bias=b1t[:])
            g1v = g1[:, GD:GD + EXB1].rearrange("q (r c) -> q r c", r=20)
            nc.vector.memset(g1v[:, :, 0:1], 0.0)
            nc.vector.memset(g1v[:, :, 130:131], 0.0)
            for r in (0, 1, 18, 19):
                nc.vector.tensor_scalar_mul(g1v[:, r, :], g1v[:, r, :],
                                            m20t[:, r:r + 1])
            w2v = w2t[:].rearrange("q (t c) -> q t c", t=9)
            for n0, nl in nchunks(EXB2, 512):
                ps = psp.tile([128, 512], F32, tag="cv", name=f"e2_{n0}",
                              bufs=3)
                for t, (dy, dx) in enumerate(TAPS):
                    off = GD + n0 + (1 + dy) * WF + dx
                    nc.tensor.matmul(ps[:, :nl], lhsT=w2v[:, t, :],
                                     rhs=g1[:, off:off + nl],
                                     start=(t == 0), stop=(t == 8))
                nc.scalar.activation(g2[:, GD + n0:GD + n0 + nl], ps[:, :nl],
                                     AF.Relu, bias=b2t[:])
            g2v = g2[:, GD:GD + EXB2].rearrange("q (r c) -> q r c", r=18)
            nc.vector.memset(g2v[:, :, 0:1], 0.0)
            nc.vector.memset(g2v[:, :, 130:131], 0.0)
            for r in (0, 17):
                nc.vector.tensor_scalar_mul(g2v[:, r, :], g2v[:, r, :],
                                            m18t[:, r:r + 1])
            g3 = gp.tile([16, EXB3], F32)
            w3v = w3t[:].rearrange("q (t c) -> q t c", t=9)
            for n0, nl in nchunks(EXB3, 512):
                ps = psp.tile([16, 512], F32, tag="cv3", name=f"e3_{n0}",
                              bufs=2)
                for t, (dy, dx) in enumerate(TAPS):
                    off = GD + n0 + (1 + dy) * WF + dx
                    nc.tensor.matmul(ps[:, :nl], lhsT=w3v[:, t, :],
                                     rhs=g2[:, off:off + nl],
                                     start=(t == 0), stop=(t == 8))
                nc.scalar.activation(g3[:, n0:n0 + nl], ps[:, :nl], AF.Copy)
            nc.vector.tensor_scalar(out=g3[:], in0=g3[:], scalar1=b3t[:],
                                    scalar2=None, op0=ALU.add)
            nc.sync.dma_start(
                mkap(cfo[:], 0, [[2064, 16], [129, 16], [1, 129]]),
                mkap_s(g3[:], 1, [[EXB3, 16], [WF, 16], [1, 129]]))
    return nc


def run_B(corr1, fw1, fb1, fw2, fb2, fw3, fb3, trace=False):
    if "B" not in _CACHE:
        _CACHE["B"] = patch_nc(build_B())
    fwre, fwim, _, _, _, _, _, _ = _dft_mats()
    corrTh = _bf(np.ascontiguousarray(corr1.transpose(0, 2, 1)))
    # block-diag weights: in p=(j)*2+h (j=ri*4+c), hid p=u*2+h, out p=j*2+h
    w1h = np.zeros((16, 9, 128), np.float32)
    w2h = np.zeros((128, 9, 128), np.float32)
    w3h = np.zeros((128, 9, 16), np.float32)
    b1h = np.zeros((128, 1), np.float32)
    b2h = np.zeros((128, 1), np.float32)
    b3h = np.zeros((16, 1), np.float32)
    for h in range(2):
        for u in range(64):
            b1h[u * 2 + h, 0] = fb1[u]
            b2h[u * 2 + h, 0] = fb2[u]
        for j in range(8):
            b3h[j * 2 + h, 0] = fb3[j]
    for t in range(9):
        dy, dx = t // 3, t % 3
        for h in range(2):
            for u in range(64):
                for j in range(8):
                    w1h[j * 2 + h, t, u * 2 + h] = fw1[u, j, dy, dx]
                    w3h[u * 2 + h, t, j * 2 + h] = fw3[j, u, dy, dx]
                for v in range(64):
                    w2h[v * 2 + h, t, u * 2 + h] = fw2[u, v, dy, dx]
    hhs = np.arange(256)
    ins = []
    for i in range(N_CORES):
        r0 = i * ROWS
        k1s = np.arange(r0 - 3, r0 + 35)
        ok = (k1s >= 0) & (k1s < 256)
        fhsh = np.zeros((128, 2, 3, 38), np.float32)
        for hc in range(2):
            h_ = np.arange(hc * 128, hc * 128 + 128)
            th = 2 * np.pi * np.outer(h_, k1s) / 256.0
            fhsh[:, hc, 0, :] = np.cos(th) / 16.0 * ok[None, :]
            fhsh[:, hc, 1, :] = -np.sin(th) / 16.0 * ok[None, :]
            fhsh[:, hc, 2, :] = np.sin(th) / 16.0 * ok[None, :]
        m20 = np.zeros((128, 20), np.float32)
        m18 = np.zeros((128, 18), np.float32)
        for p in range(128):
            h = p % 2
            base = r0 + h * 16
            for r in range(20):
                m20[p, r] = 1.0 if 0 <= base - 2 + r < 256 else 0.0
            for r in range(18):
                m18[p, r] = 1.0 if 0 <= base - 1 + r < 256 else 0.0
        ins.append({
            "corrT": corrTh, "fwre": fwre, "fwim": fwim,
            "fhs": _bf(fhsh.reshape(128, -1)),
            "gw1": _bf(w1h.reshape(16, -1)), "gb1": b1h,
            "gw2": _bf(w2h.reshape(128, -1)), "gb2": b2h,
            "gw3": _bf(w3h.reshape(128, -1)), "gb3": b3h,
            "mf20": m20, "mf18": m18,
        })
    res = run_bass_kernel_spmd(_CACHE["B"], ins, core_ids=list(range(N_CORES)),
                               trace=trace)
    cf = np.zeros((8, 256, 129), np.float32)
    for i in range(N_CORES):
        f = res.results[i]["cfo"].reshape(16, 16, 129)
        for j in range(8):
            for h in range(2):
                cf[j, i * ROWS + h * 16:i * ROWS + h * 16 + 16, :] = \
                    f[j * 2 + h]
    return None, cf, res


# ================================================================ kernel C
# inverse DFT from full CF (host-gathered) + per-channel refinement as
# 2-half row-packed block-diagonal convs (128 partitions, bf16).
# partition layouts: u/r3: p = c*2 + h; r1/r2: p = c*32 + u*2 + h.
WPC = 258
EXTC = 22 * WPC      # u half extent (22 rows)
EXTR1 = 20 * WPC
EXTR2 = 18 * WPC
EXTR3 = 16 * WPC


def build_C():
    nc = bass.Bass(trn_type="TRN2", name="kernC")
    u0 = nc.dram_tensor("u0", (8, EXTC), BF16, kind="ExternalInput")
    raws = nc.dram_tensor("raws", (8, EXTR3), F32, kind="ExternalInput")
    cfa = nc.dram_tensor("cfa", (128, 4 * 2 * 2 * 128), BF16,
                         kind="ExternalInput")
    cfb = nc.dram_tensor("cfb", (1, 4 * 2 * 2 * 128), BF16,
                         kind="ExternalInput")
    iwm = nc.dram_tensor("iwm", (128, 2 * 512), BF16, kind="ExternalInput")
    iwbm = nc.dram_tensor("iwbm", (1, 2 * 512), BF16, kind="ExternalInput")
    ihs = nc.dram_tensor("ihs", (128, 2 * 2 * 38), BF16,
                         kind="ExternalInput")
    cw1 = nc.dram_tensor("cw1", (8, 9 * 128), BF16, kind="ExternalInput")
    cb1 = nc.dram_tensor("cb1", (128, 1), F32, kind="ExternalInput")
    cw2 = nc.dram_tensor("cw2", (128, 9 * 128), BF16, kind="ExternalInput")
    cb2 = nc.dram_tensor("cb2", (128, 1), F32, kind="ExternalInput")
    cw3 = nc.dram_tensor("cw3", (128, 9 * 8), BF16, kind="ExternalInput")
    cb3 = nc.dram_tensor("cb3", (8, 1), F32, kind="ExternalInput")
    mr20 = nc.dram_tensor("mr20", (128, 20), F32, kind="ExternalInput")
    mr18 = nc.dram_tensor("mr18", (128, 18), F32, kind="ExternalInput")
    fin = nc.dram_tensor("fin", (8, 16 * 256), F32, kind="ExternalOutput")

    with tile.TileContext(nc) as tc:
        with tc.tile_pool(name="cst", bufs=1) as cst, \
             tc.tile_pool(name="gp", bufs=1) as gp, \
             tc.tile_pool(name="ps", bufs=1, space="PSUM") as psp:
            cfat = cst.tile([128, 4 * 2 * 2 * 128], BF16)
            nc.sync.dma_start(cfat[:], cfa[:])
            cfbt = cst.tile([1, 4 * 2 * 2 * 128], BF16)
            nc.sync.dma_start(cfbt[:], cfb[:])
            iwt = cst.tile([128, 2 * 512], BF16)
            nc.sync.dma_start(iwt[:], iwm[:])
            iwbt = cst.tile([1, 2 * 512], BF16)
            nc.sync.dma_start(iwbt[:], iwbm[:])
            ihst = cst.tile([128, 2 * 2 * 38], BF16)
            nc.sync.dma_start(ihst[:], ihs[:])
            w1t = cst.tile([8, 9 * 128], BF16)
            nc.sync.dma_start(w1t[:], cw1[:])
            w2t = cst.tile([128, 9 * 128], BF16)
            nc.sync.dma_start(w2t[:], cw2[:])
            w3t = cst.tile([128, 9 * 8], BF16)
            nc.sync.dma_start(w3t[:], cw3[:])
            b1t = cst.tile([128, 1], F32)
            nc.sync.dma_start(b1t[:], cb1[:])
            b2t = cst.tile([128, 1], F32)
            nc.sync.dma_start(b2t[:], cb2[:])
            b3t = cst.tile([8, 1], F32)
            nc.sync.dma_start(b3t[:], cb3[:])
            m20t = cst.tile([128, 20], F32)
            nc.sync.dma_start(m20t[:], mr20[:])
            m18t = cst.tile([128, 18], F32)
            nc.sync.dma_start(m18t[:], mr18[:])

            cfav = cfat[:].rearrange("q (c r k m) -> q c r k m", c=4, r=2, k=2)
            cfbv = cfbt[:].rearrange("q (c r k m) -> q c r k m", c=4, r=2, k=2)
            ihsv = ihst[:].rearrange("q (k t h) -> q k t h", k=2, t=2)

            # u = u0 + z (inverse DFT), packed [8, GD + EXTC + GD]
            u = gp.tile([8, GD + EXTC + GD], BF16)
            nc.vector.memset(u[:, 0:GD], 0.0)
            nc.vector.memset(u[:, GD + EXTC:], 0.0)
            nc.sync.dma_start(u[:, GD:GD + EXTC], u0[:])
            zu = gp.tile([8, EXTC], BF16)
            nc.vector.memset(zu[:], 0.0)
            with tc.tile_pool(name="ip", bufs=2) as ip:
                for c in range(C):
                    # B[kc][k1, ri, w] = sum_k cf[c,k1,k] iw[k,w] (complex)
                    Bt = [ip.tile([128, 2 * 256], BF16, tag=f"Bt{kc}",
                                  name=f"Bt_{c}_{kc}") for kc in range(2)]
                    for kc in range(2):
                        pb = psp.tile([128, 512], F32, tag="pb",
                                      name=f"pb_{c}_{kc}", bufs=2)
                        nc.tensor.matmul(
                            pb[:], lhsT=cfav[:, c, 0, kc, :],
                            rhs=iwt[:, 0:512], start=True, stop=False)
                        nc.tensor.matmul(
                            pb[:], lhsT=cfbv[:, c, 0, kc, :],
                            rhs=iwbt[:, 0:512], start=False, stop=False)
                        nc.tensor.matmul(
                            pb[:], lhsT=cfav[:, c, 1, kc, :],
                            rhs=iwt[:, 512:1024], start=False, stop=False)
                        nc.tensor.matmul(
                            pb[:], lhsT=cfbv[:, c, 1, kc, :],
                            rhs=iwbt[:, 512:1024], start=False, stop=True)
                        nc.scalar.activation(Bt[kc][:], pb[:], AF.Copy)
                    # z[hh, w] = sum_k1 ih[k1, hh] B[k1, w] (re part)
                    pz = psp.tile([38, 256], F32, tag="pz",
                                  name=f"pz_{c}", bufs=1)
                    ti = 0
                    for kc in range(2):
                        for term in range(2):
                            nc.tensor.matmul(
                                pz[:], lhsT=ihsv[:, kc, term, :],
                                rhs=Bt[kc][:, term * 256:(term + 1) * 256],
                                start=(ti == 0), stop=(ti == 3))
                            ti += 1
                    zs = ip.tile([38, 256], BF16, tag="zs", name=f"zs_{c}")
                    nc.scalar.activation(zs[:], pz[:], AF.Copy)
                    for h in range(2):
                        zq = nc.sync if h == 0 else nc.gpsimd
                        zq.dma_start(
                            mkap_s(zu[c * 2 + h:c * 2 + h + 1, :], 1,
                                   [[EXTC, 1], [WPC, 22], [1, 256]]),
                            zs[h * 16:h * 16 + 22, :])
            UH = 11 * WPC
            nc.vector.tensor_tensor(out=u[:, GD:GD + UH],
                                    in0=u[:, GD:GD + UH], in1=zu[:, :UH],
                                    op=ALU.add)
            nc.vector.tensor_tensor(out=u[:, GD + UH:GD + EXTC],
                                    in0=u[:, GD + UH:GD + EXTC],
                                    in1=zu[:, UH:], op=ALU.add)

            r1 = gp.tile([128, GD + EXTR1 + GD], BF16)
            r2 = gp.tile([128, GD + EXTR2 + GD], BF16)
            for n0, nl in nchunks(EXTR1, 512):
                ps = psp.tile([128, 512], F32, tag="cv", name=f"d1_{n0}",
                              bufs=3)
                for t, (dy, dx) in enumerate(TAPS):
                    off = GD + n0 + (1 + dy) * WPC + dx
                    nc.tensor.matmul(ps[:, :nl],
                                     lhsT=w1t[:].rearrange(
                                         "q (t c) -> q t c", t=9)[:, t, :],
                                     rhs=u[:, off:off + nl],
                                     start=(t == 0), stop=(t == 8))
                nc.scalar.activation(r1[:, GD + n0:GD + n0 + nl], ps[:, :nl],
                                     AF.Relu, bias=b1t[:])
            r1v = r1[:, GD:GD + EXTR1].rearrange("q (r c) -> q r c", r=20)
            nc.vector.memset(r1v[:, :, 0:1], 0.0)
            nc.vector.memset(r1v[:, :, 257:258], 0.0)
            for r in (0, 1, 18, 19):
                nc.vector.tensor_scalar_mul(r1v[:, r, :], r1v[:, r, :],
                                            m20t[:, r:r + 1])
            for n0, nl in nchunks(EXTR2, 512):
                ps = psp.tile([128, 512], F32, tag="cv", name=f"d2_{n0}",
                              bufs=3)
                for t, (dy, dx) in enumerate(TAPS):
                    off = GD + n0 + (1 + dy) * WPC + dx
                    nc.tensor.matmul(ps[:, :nl],
                                     lhsT=w2t[:].rearrange(
                                         "q (t c) -> q t c", t=9)[:, t, :],
                                     rhs=r1[:, off:off + nl],
                                     start=(t == 0), stop=(t == 8))
                nc.scalar.activation(r2[:, GD + n0:GD + n0 + nl], ps[:, :nl],
                                     AF.Relu, bias=b2t[:])
            r2v = r2[:, GD:GD + EXTR2].rearrange("q (r c) -> q r c", r=18)
            nc.vector.memset(r2v[:, :, 0:1], 0.0)
            nc.vector.memset(r2v[:, :, 257:258], 0.0)
            for r in (0, 17):
                nc.vector.tensor_scalar_mul(r2v[:, r, :], r2v[:, r, :],
                                            m18t[:, r:r + 1])
            r3 = gp.tile([8, EXTR3], F32)
            rawt = gp.tile([8, EXTR3], F32)
            nc.sync.dma_start(rawt[:], raws[:])
            for n0, nl in nchunks(EXTR3, 512):
                ps = psp.tile([8, 512], F32, tag="cv3", name=f"d3_{n0}",
                              bufs=2)
                for t, (dy, dx) in enumerate(TAPS):
                    off = GD + n0 + (1 + dy) * WPC + dx
                    nc.tensor.matmul(ps[:, :nl],
                                     lhsT=w3t[:].rearrange(
                                         "q (t c) -> q t c", t=9)[:, t, :],
                                     rhs=r2[:, off:off + nl],
                                     start=(t == 0), stop=(t == 8))
                nc.vector.tensor_tensor(out=r3[:, n0:n0 + nl],
                                        in0=ps[:, :nl],
                                        in1=rawt[:, n0:n0 + nl], op=ALU.add)
                nc.vector.tensor_scalar(out=r3[:, n0:n0 + nl],
                                        in0=r3[:, n0:n0 + nl], scalar1=0.0,
                                        scalar2=1.0, op0=ALU.max,
                                        op1=ALU.min)
            nc.sync.dma_start(
                fin[:, :], mkap_s(r3[:], 1, [[EXTR3, 8], [WPC, 16],
                                             [1, 256]]))
    return nc


def build_C_old():
    nc = bass.Bass(trn_type="TRN2", name="kernC")
    u = nc.dram_tensor("u", (C, 38 * WP), BF16, kind="ExternalInput")
    raw32 = nc.dram_tensor("raw32", (C, ROWS * W), F32, kind="ExternalInput")
    cw1 = nc.dram_tensor("cw1", (C, 9 * 64), BF16, kind="ExternalInput")
    cb1 = nc.dram_tensor("cb1", (64, 1), F32, kind="ExternalInput")
    cw2 = nc.dram_tensor("cw2", (64, 9 * 64), BF16, kind="ExternalInput")
    cb2 = nc.dram_tensor("cb2", (64, 1), F32, kind="ExternalInput")
    cw3 = nc.dram_tensor("cw3", (64, 9 * 4), BF16, kind="ExternalInput")
    cb3 = nc.dram_tensor("cb3", (4, 1), F32, kind="ExternalInput")
    mr36 = nc.dram_tensor("mr36", (64, 36), F32, kind="ExternalInput")
    mr34 = nc.dram_tensor("mr34", (64, 34), F32, kind="ExternalInput")
    fin = nc.dram_tensor("fin", (C, ROWS, W), F32, kind="ExternalOutput")

    N36, N34, N32 = 36 * WP, 34 * WP, 32 * WP

    def conv_taps_outer(pool_ps, lhsw, rhsrc, dstact, bias, Ntot, Kp, Mp, relu,
                        group=1):
        """taps-outer grouped conv: lhsw(t)->lhsT AP, rhsrc(t, n0, nl)->rhs AP,
        dstact(n0, nl, psum) consumes."""
        chunks = nchunks(Ntot, 512)
        for g0 in range(0, len(chunks), group):
            grp = chunks[g0:g0 + group]
            pss = [pool_ps.tile([Mp, 512], F32, tag=f"cg{j}", name=f"cg_{g0}_{j}",
                                bufs=6) for j in range(len(grp))]
            for t in range(9):
                for j, (n0, nl) in enumerate(grp):
                    nc.tensor.matmul(pss[j][:, :nl], lhsT=lhsw(t),
                                     rhs=rhsrc(t, n0, nl),
                                     start=(t == 0), stop=(t == 8))
            for j, (n0, nl) in enumerate(grp):
                dstact(n0, nl, pss[j])

    with tile.TileContext(nc) as tc:
        with tc.tile_pool(name="cst", bufs=1) as cst, \
             tc.tile_pool(name="gp", bufs=1) as gp, \
             tc.tile_pool(name="ps", bufs=1, space="PSUM") as psp:
            w1t = cst.tile([C, 9 * 64], BF16)
            nc.sync.dma_start(w1t[:], cw1[:])
            w2t = cst.tile([64, 9 * 64], BF16)
            nc.sync.dma_start(w2t[:], cw2[:])
            w3t = cst.tile([64, 9 * 4], BF16)
            nc.sync.dma_start(w3t[:], cw3[:])
            b1t = cst.tile([64, 1], F32)
            nc.sync.dma_start(b1t[:], cb1[:])
            b2t = cst.tile([64, 1], F32)
            nc.sync.dma_start(b2t[:], cb2[:])
            b3t = cst.tile([C, 1], F32)
            nc.sync.dma_start(b3t[:], cb3[:])
            m36t = cst.tile([64, 36], F32)
            nc.sync.dma_start(m36t[:], mr36[:])
            m34t = cst.tile([64, 34], F32)
            nc.sync.dma_start(m34t[:], mr34[:])

            ut = gp.tile([C, 1 + 38 * WP + 4], BF16)
            nc.sync.dma_start(ut[:, 1:1 + 38 * WP], u[:])
            r1 = gp.tile([64, 1 + N36 + 4], BF16)
            r2 = gp.tile([64, 1 + N34 + 4], BF16)

            conv_taps_outer(
                psp,
                lambda t: w1t[:, t * 64:(t + 1) * 64],
                lambda t, n0, nl: ut[:, 1 + n0 + (1 + TAPS[t][0]) * WP + TAPS[t][1]:
                                     1 + n0 + (1 + TAPS[t][0]) * WP + TAPS[t][1] + nl],
                lambda n0, nl, ps: nc.scalar.activation(
                    r1[:, 1 + n0:1 + n0 + nl], ps[:, :nl], AF.Relu, bias=b1t[:]),
                b1t, N36, 64, 64, True)
            r1v = r1[:, 1:1 + N36].rearrange("p (r q) -> p r q", r=36)
            nc.vector.memset(r1v[:, :, 0:1], 0.0)
            nc.vector.memset(r1v[:, :, 257:258], 0.0)
            for r in (0, 1, 34, 35):
                nc.vector.tensor_scalar_mul(r1v[:, r, :], r1v[:, r, :],
                                            m36t[:, r:r + 1])

            conv_taps_outer(
                psp,
                lambda t: w2t[:, t * 64:(t + 1) * 64],
                lambda t, n0, nl: r1[:, 1 + n0 + (1 + TAPS[t][0]) * WP + TAPS[t][1]:
                                     1 + n0 + (1 + TAPS[t][0]) * WP + TAPS[t][1] + nl],
                lambda n0, nl, ps: nc.scalar.activation(
                    r2[:, 1 + n0:1 + n0 + nl], ps[:, :nl], AF.Relu, bias=b2t[:]),
                b2t, N34, 64, 64, True)
            r2v = r2[:, 1:1 + N34].rearrange("p (r q) -> p r q", r=34)
            nc.vector.memset(r2v[:, :, 0:1], 0.0)
            nc.vector.memset(r2v[:, :, 257:258], 0.0)
            for r in (0, 33):
                nc.vector.tensor_scalar_mul(r2v[:, r, :], r2v[:, r, :],
                                            m34t[:, r:r + 1])

            with tc.tile_pool(name="fo", bufs=1) as fo:
                rawt = fo.tile([C, ROWS * W], F32)
                nc.sync.dma_start(rawt[:], raw32[:])
                r3 = fo.tile([C, N32], F32)
                conv_taps_outer(
                    psp,
                    lambda t: w3t[:, t * 4:(t + 1) * 4],
                    lambda t, n0, nl: r2[:, 1 + n0 + (1 + TAPS[t][0]) * WP + TAPS[t][1]:
                                         1 + n0 + (1 + TAPS[t][0]) * WP + TAPS[t][1] + nl],
                    lambda n0, nl, ps: nc.scalar.activation(
                        r3[:, n0:n0 + nl], ps[:, :nl], AF.Copy),
                    b3t, N32, 64, C, False)
                r3v = r3[:].rearrange("p (r q) -> p r q", r=32)[:, :, 1:257]
                rv = rawt[:].rearrange("p (r q) -> p r q", r=32)
                nc.vector.tensor_scalar(out=r3v, in0=r3v, scalar1=b3t[:],
                                        scalar2=None, op0=ALU.add)
                nc.vector.tensor_tensor(out=r3v, in0=r3v, in1=rv, op=ALU.add)
                nc.vector.tensor_scalar(out=r3v, in0=r3v, scalar1=0.0,
                                        scalar2=1.0, op0=ALU.max, op1=ALU.min)
                nc.sync.dma_start(fin[:, :, :], r3v)
    return nc


_CACHE = {}


def _f8(x):
    return np.asarray(x, dtype=np.float32).astype(ml_dtypes.float8_e4m3)


def _prep_A(raw, feat, pw1, pb1, pw2, pb2, pw3, pb3):
    # weights packed for DoubleRow passes (see PAIRS)
    def tap_w(pw, dydx):
        dy, dx = dydx
        return pw[:, :, dy + 1, dx + 1]  # [co, ci]

    # w1: [ci, m, p, kt, co128]
    w1h = np.zeros((128, 2, 5, 2, 128), np.float32)
    for m in range(2):
        for p in range(5):
            t0, t1 = pair_taps(p)
            w1h[:, m, p, 0, :] = tap_w(pw1, t0).T[:, m * 128:(m + 1) * 128]
            if t1 is not None:
                w1h[:, m, p, 1, :] = tap_w(pw1, t1).T[:, m * 128:(m + 1) * 128]
    # w2: [cip, t, kc, co]
    w2h = np.zeros((128, 9, 2, 128), np.float32)
    for t, (dy, dx) in enumerate(TAPS):
        wt = tap_w(pw2, (dy, dx))  # [128 co, 256 ci]
        for kc in range(2):
            w2h[:, t, kc, :] = wt[:, kc * 128:(kc + 1) * 128].T
    # w3: [ci, p, kt, 912] (col = c*228 + tpsf)
    w3h = np.zeros((128, 5, 2, 912), np.float32)
    for p in range(5):
        t0, t1 = pair_taps(p)
        for kt, tt in ((0, t0), (1, t1)):
            if tt is None:
                continue
            wt = tap_w(pw3, tt)  # [900, 128]
            for c in range(C):
                w3h[:, p, kt, c * 228:c * 228 + 225] = \
                    wt[c * 225:(c + 1) * 225].T
    b1h = np.ascontiguousarray(pb1.reshape(2, 128).T).astype(np.float32)
    b2h = pb2.reshape(128, 1).astype(np.float32)
    b3row = np.full((912,), -30.0, np.float32)
    for c in range(C):
        b3row[c * 228:c * 228 + 225] = pb3[c * 225:(c + 1) * 225]
    w3h[0, 4, 1, :] = b3row

    xpad = np.pad(raw, ((0, 0), (PAD, PAD), (PAD, PAD)), mode="reflect")
    # unfolded patches [4, 256, 256, 15, 15]
    sw = np.lib.stride_tricks.sliding_window_view(xpad, (15, 15),
                                                  axis=(1, 2))
    featp = np.pad(feat, ((0, 0), (3, 3), (0, 0)))

    ins = []
    for i in range(N_CORES):
        r0 = i * ROWS
        m36 = np.array([1.0 if 0 <= r0 - 2 + r < H else 0.0
                        for r in range(36)], np.float32)
        m34 = np.array([1.0 if 0 <= r0 - 1 + r < H else 0.0
                        for r in range(34)], np.float32)
        fbA = np.zeros((128, 38, RP), np.float32)
        fbA[:, :, 1:257] = featp[:, r0:r0 + 38, :]
        fbA = fbA.reshape(128, EXTF)
        fbh = np.zeros((128, GD + 2 * EXTF), np.float32)
        fbh[:, GD:GD + EXTF] = fbA
        fbh[:, GD + EXTF:GD + 2 * EXTF - 1] = fbA[:, 1:]
        # Xu: [8192 pix, 912] = (r, x) -> [c*228 + tpsf]; bias comes via
        # the psum ones-matmul, so patches stay unscaled
        slab = sw[:, r0:r0 + ROWS, :, :, :]  # [4, 32, 256, 15, 15]
        xuh = np.zeros((ROWS * W, 4, 228), np.float32)
        xuh[:, :, :225] = slab.reshape(4, ROWS * W, 225).transpose(1, 0, 2)
        xuh = xuh.reshape(ROWS * W, 912)
        ins.append({
            "fb": _f8(fbh),
            "w1": _f8(w1h.reshape(128, -1)), "b1": b1h,
            "w2": _f8(w2h.reshape(128, -1)), "b2": b2h,
            "w3": _f8(w3h.reshape(128, -1)),
            "xu": _bf(xuh),
            "m36": np.ascontiguousarray(np.broadcast_to(m36, (128, 36))),
            "m34": np.ascontiguousarray(np.broadcast_to(m34, (128, 34))),
        })
    return ins


def run_A(raw, feat, pw1, pb1, pw2, pb2, pw3, pb3, trace=False):
    if "A" not in _CACHE:
        _CACHE["A"] = patch_nc(build_A())
    ins = _prep_A(raw, feat, pw1, pb1, pw2, pb2, pw3, pb3)
    res = run_bass_kernel_spmd(_CACHE["A"], ins, core_ids=list(range(N_CORES)),
                               trace=trace)
    corr = np.concatenate(
        [res.results[i]["corr"].reshape(ROWS, 2, 4, 128)
         .transpose(2, 0, 1, 3).reshape(C, ROWS, W)
         for i in range(N_CORES)], axis=1)
    return corr, res


def _dft_mats():
    k = np.arange(129)
    w = np.arange(256)
    th = 2 * np.pi * np.outer(w, k) / 256.0          # [256, 129]
    fwre = _bf(np.cos(th) / 16.0)
    fwim = _bf(-np.sin(th) / 16.0)
    h = np.arange(256)
    k1 = np.arange(256)
    th2 = 2 * np.pi * np.outer(h, k1) / 256.0        # [256h, 256k1]
    fhre = _bf(np.cos(th2) / 16.0)
    fhim = _bf(-np.sin(th2) / 16.0)
    fhimn = _bf(np.sin(th2) / 16.0)
    ck = np.where((k == 0) | (k == 128), 1.0, 2.0)
    th3 = 2 * np.pi * np.outer(k, w) / 256.0         # [129k, 256w]
    iwre = _bf(ck[:, None] * np.cos(th3) / 16.0)
    iwim = _bf(ck[:, None] * np.sin(th3) / 16.0)
    iwimn = _bf(-ck[:, None] * np.sin(th3) / 16.0)
    return fwre, fwim, fhre, fhim, fhimn, iwre, iwim, iwimn


def run_B1(corr1, trace=False):
    if "B1" not in _CACHE:
        _CACHE["B1"] = patch_nc(build_B1())
    fwre, fwim, fhre, fhim, fhimn, _, _, _ = _dft_mats()
    corrT = _bf(np.ascontiguousarray(corr1.transpose(0, 2, 1)))
    inm = {"corrT": corrT, "fwre": fwre, "fwim": fwim,
           "fhre": fhre, "fhim": fhim, "fhimn": fhimn}
    res = run_bass_kernel_spmd(_CACHE["B1"], [inm] * N_CORES,
                               core_ids=list(range(N_CORES)), trace=trace)
    return res.results[0]["fri"], res


def run_B2(fri_full, fw1, fb1, fw2, fb2, fw3, fb3, trace=False):
    from einops import rearrange as rr
    if "B2" not in _CACHE:
        _CACHE["B2"] = patch_nc(build_B2())
    gw1 = _bf(rr(fw1, "co ci dy dx -> ci (dy dx co)"))
    gw2 = _bf(rr(fw2, "co ci dy dx -> ci (dy dx co)"))
    gw3 = _bf(rr(fw3, "co ci dy dx -> ci (dy dx co)"))
    gb1 = fb1.reshape(64, 1).astype(np.float32)
    gb2 = fb2.reshape(64, 1).astype(np.float32)
    gb3 = fb3.reshape(8, 1).astype(np.float32)
    ins = []
    for i in range(N_CORES):
        r0 = i * ROWS
        slab = np.zeros((8, 38, WF), np.float32)
        lo, hi = max(0, r0 - 3), min(256, r0 + 35)
        slab[:, lo - (r0 - 3):hi - (r0 - 3), 1:130] = fri_full[:, lo:hi, :]
        m36 = np.array([1.0 if 0 <= r0 - 2 + r < 256 else 0.0
                        for r in range(36)], np.float32)
        m34 = np.array([1.0 if 0 <= r0 - 1 + r < 256 else 0.0
                        for r in range(34)], np.float32)
        ins.append({
            "fri": _bf(slab.reshape(8, 38 * WF)),
            "gw1": gw1, "gb1": gb1, "gw2": gw2, "gb2": gb2,
            "gw3": gw3, "gb3": gb3,
            "mf36": np.ascontiguousarray(np.broadcast_to(m36, (64, 36))),
            "mf34": np.ascontiguousarray(np.broadcast_to(m34, (64, 34))),
        })
    res = run_bass_kernel_spmd(_CACHE["B2"], ins, core_ids=list(range(N_CORES)),
                               trace=trace)
    cf = np.concatenate([res.results[i]["cfo"].reshape(8, 32, 129)
                         for i in range(N_CORES)], axis=1)
    return cf, res


def run_C(corr1, cf, raw, cw1, cb1, cw2, cb2, cw3, cb3, trace=False):
    if "C" not in _CACHE:
        _CACHE["C"] = patch_nc(build_C())
    # block-diag weights, layouts: in p=c*2+h, hid p=c*32+u*2+h, out p=c*2+h
    w1h = np.zeros((8, 9, 128), np.float32)
    w2h = np.zeros((128, 9, 128), np.float32)
    w3h = np.zeros((128, 9, 8), np.float32)
    b1h = np.zeros((128, 1), np.float32)
    b2h = np.zeros((128, 1), np.float32)
    b3h = np.zeros((8, 1), np.float32)
    for c in range(C):
        for h in range(2):
            b3h[c * 2 + h, 0] = cb3[c, 0]
            for uu in range(16):
                b1h[c * 32 + uu * 2 + h, 0] = cb1[c, uu]
                b2h[c * 32 + uu * 2 + h, 0] = cb2[c, uu]
    for t, (dy, dx) in enumerate([(a, b) for a in range(3) for b in range(3)]):
        for c in range(C):
            for h in range(2):
                for uu in range(16):
                    w1h[c * 2 + h, t, c * 32 + uu * 2 + h] = \
                        cw1[c, uu, 0, dy, dx]
                    w3h[c * 32 + uu * 2 + h, t, c * 2 + h] = \
                        cw3[c, 0, uu, dy, dx]
                    for v in range(16):
                        w2h[c * 32 + v * 2 + h, t, c * 32 + uu * 2 + h] = \
                            cw2[c, uu, v, dy, dx]
    # inverse DFT constants (same for all cores except ihs)
    kk = np.arange(129)
    w_ = np.arange(256)
    ck = np.where((kk == 0) | (kk == 128), 1.0, 2.0)
    th3 = 2 * np.pi * np.outer(kk, w_) / 256.0
    iwre = ck[:, None] * np.cos(th3) / 16.0
    iwim = ck[:, None] * np.sin(th3) / 16.0
    iwh = np.zeros((128, 2 * 512), np.float32)
    iwbh = np.zeros((1, 2 * 512), np.float32)
    for j, m in enumerate((iwre, iwim, -iwim, iwre)):
        iwh[:, j * 256:(j + 1) * 256] = m[:128]
        iwbh[0, j * 256:(j + 1) * 256] = m[128]
    # cfa [128 k, (c, ri, kc, 128 k1)], cfb k=128 row
    cfah = np.zeros((128, 4, 2, 2, 128), np.float32)
    cfbh = np.zeros((1, 4, 2, 2, 128), np.float32)
    for c in range(C):
        for ri in range(2):
            m = cf[ri * 4 + c]  # [256 k1, 129 k]
            for kc in range(2):
                cfah[:, c, ri, kc, :] = m[kc * 128:(kc + 1) * 128, :128].T
                cfbh[0, c, ri, kc, :] = m[kc * 128:(kc + 1) * 128, 128]
    ins = []
    for i in range(N_CORES):
        r0 = i * ROWS
        u0h = np.zeros((8, 22, WPC), np.float32)
        rawh = np.zeros((8, 16, WPC), np.float32)
        ihsh = np.zeros((128, 2, 2, 38), np.float32)
        hh = np.arange(r0 - 3, r0 + 35)
        ok = (hh >= 0) & (hh < 256)
        for kc in range(2):
            k1 = np.arange(kc * 128, kc * 128 + 128)
            th = 2 * np.pi * np.outer(k1, hh) / 256.0
            ihsh[:, kc, 0, :] = np.cos(th) / 16.0 * ok[None, :]
            ihsh[:, kc, 1, :] = -np.sin(th) / 16.0 * ok[None, :]
        for c in range(C):
            for h in range(2):
                lo = r0 + h * 16 - 3
                a, b = max(0, lo), min(256, lo + 22)
                u0h[c * 2 + h, a - lo:b - lo, 1:257] = corr1[c, a:b, :]
                rawh[c * 2 + h, :, 1:257] = \
                    raw[c, r0 + h * 16:r0 + h * 16 + 16, :] + cb3[c, 0]
        m20 = np.zeros((128, 20), np.float32)
        m18 = np.zeros((128, 18), np.float32)
        for p in range(128):
            h = p % 2
            base = r0 + h * 16
            for r in range(20):
                m20[p, r] = 1.0 if 0 <= base - 2 + r < 256 else 0.0
            for r in range(18):
                m18[p, r] = 1.0 if 0 <= base - 1 + r < 256 else 0.0
        ins.append({
            "u0": _bf(u0h.reshape(8, EXTC)),
            "raws": rawh.reshape(8, EXTR3).astype(np.float32),
            "cfa": _bf(cfah.reshape(128, -1)),
            "cfb": _bf(cfbh.reshape(1, -1)),
            "iwm": _bf(iwh), "iwbm": _bf(iwbh),
            "ihs": _bf(ihsh.reshape(128, -1)),
            "cw1": _bf(w1h.reshape(8, -1)), "cb1": b1h,
            "cw2": _bf(w2h.reshape(128, -1)), "cb2": b2h,
            "cw3": _bf(w3h.reshape(128, -1)), "cb3": b3h,
            "mr20": m20, "mr18": m18,
        })
    res = run_bass_kernel_spmd(_CACHE["C"], ins, core_ids=list(range(N_CORES)),
                               trace=trace)
    fin = np.zeros((C, H, W), np.float32)
    for i in range(N_CORES):
        f = res.results[i]["fin"].reshape(8, 16, 256)
        for c in range(C):
            for h in range(2):
                fin[c, i * ROWS + h * 16:i * ROWS + h * 16 + 16, :] = \
                    f[c * 2 + h]
    return fin, res


def kernel(**inputs):
    inputs = {k: np.asarray(v, dtype=np.float32) for k, v in inputs.items()}
    raw = inputs["raw_image"][0]
    feat = inputs["aberration_features"][0]
    corr1, _ = run_A(raw, feat,
                     inputs["pw1"], inputs["pb1"], inputs["pw2"], inputs["pb2"],
                     inputs["pw3"], inputs["pb3"])
    _, cf, _ = run_B(corr1, inputs["fw1"], inputs["fb1"], inputs["fw2"],
                     inputs["fb2"], inputs["fw3"], inputs["fb3"])
    fin, _ = run_C(corr1, cf, raw, inputs["cw1"], inputs["cb1"],
                   inputs["cw2"], inputs["cb2"], inputs["cw3"],
                   inputs["cb3"])
    return fin[None].astype(np.float32)




# revision 40
# speedup vs baseline: 1.1027x; 1.0044x over previous
"""Trainium2 Bass kernel for nn_AberrationCorrectionModule.

Reference pipeline:
  1. psf_predictor: 3x conv3x3 (128->256->128->900) on aberration_features,
     softmax over 225 taps per channel -> psf
  2. deconv: 15x15 spatially-varying weighted sum over reflect-padded raw
  3. freq corrector: rfft2 -> conv3x3 stack (8->64->64->8) -> irfft2, added
  4. per-channel refinement: 4 independent 1->16->16->1 conv stacks
  5. out = clip(raw + corrected, 0, 1)

Distribution: 8 NeuronCores, H-sharded (32 rows/core), SPMD dispatches with
host gather between (FFT stage needs full-image mixing).
"""
import json
import sys

sys.path.insert(0, "/opt/trn_rl_repo")

import ml_dtypes
import numpy as np

import bass_rust
import concourse.bass as bass
import concourse.tile as tile
from concourse import mybir
from concourse.bass_utils import run_bass_kernel_spmd

F32 = mybir.dt.float32
BF16 = mybir.dt.bfloat16
AF = mybir.ActivationFunctionType
ALU = mybir.AluOpType
AX = mybir.AxisListType

N_CORES = 8
C, H, W = 4, 256, 256
ROWS = H // N_CORES  # 32
KK = 15
PAD = KK // 2  # 7
WP = W + 2  # 258
TAPS = [(dy, dx) for dy in (-1, 0, 1) for dx in (-1, 0, 1)]


def _bf(x):
    return np.asarray(x, dtype=ml_dtypes.bfloat16)


def mkap(base_ap, offset, pairs):
    a = base_ap.copy()
    a.offset = offset
    a.ap = bass_rust.VecI64Pair([list(p) for p in pairs])
    return a


def _split_multiwaits(raw: bytes) -> bytes:
    """Workaround: this walrus build rejects >1 sync wait per instruction.
    Move extra waits onto NoOp carriers inserted just before the instruction."""
    m = json.loads(raw)
    ctr = 0
    for fn in m["functions"]:
        for bb in fn.get("blocks", []):
            insts = bb.get("instructions")
            if not insts:
                continue
            out = []
            for inst in insts:
                si = inst.get("sync_info")
                ow = (si or {}).get("on_wait") or []
                if len(ow) > 1:
                    for w in ow[:-1]:
                        out.append({
                            "debug": inst.get("debug", 0),
                            "engine": inst["engine"],
                            "ins": [], "outs": [],
                            "name": f"wsplit_{ctr}",
                            "opcode": "NoOp",
                            "sync_info": {"on_update": [], "on_wait": [w]},
                        })
                        ctr += 1
                    si["on_wait"] = [ow[-1]]
                out.append(inst)
            bb["instructions"] = out
    return json.dumps(m).encode()


def patch_nc(nc):
    orig = nc.to_json_bytes
    nc.to_json_bytes = lambda: _split_multiwaits(orig())
    return nc


def nchunks(total, step):
    out, o = [], 0
    while o < total:
        out.append((o, min(step, total - o)))
        o += step
    return out


# ================================================================ kernel A
# fp8 DoubleRow rewrite.
# conv1/conv2 feature-major on a 272-pitch grid (row pitch % 16 == 0 for
# DoubleRow lhsT k-tile strides). conv3 transposed: pixels on partitions,
# psf taps on the free axis (4ch x 228, 912 cols), softmax tail on
# vector/scalar engines. Patches pre-unfolded on host to [8192, 912].

RP = 272            # row pitch
EXTF = 38 * RP      # fb copy extent
EXTH1 = 36 * RP     # h1 half extent
EXTH2 = 34 * RP     # h2 copy extent
GD = 16             # leading guard cols
F8 = mybir.dt.float8e4
DR = mybir.MatmulPerfMode.DoubleRow
# conv tap pairs: 3 horizontal A/B-copy pairs, 1 vertical, 1 zero-padded
# (dy, dx) of kt0; kind 'AB' = kt1 from shifted copy (stride EXT),
# 'V' = kt1 one row down (stride RP), 'Z' = kt1 zero weights (stride RP)
PAIRS = [((-1, -1), 'AB'), ((0, -1), 'AB'), ((1, -1), 'AB'),
         ((-1, 1), 'V'), ((1, 1), 'Z')]


def pair_taps(p):
    """taps (as (dy,dx)) covered by pair p: (kt0, kt1 or None)."""
    (dy, dx), kind = PAIRS[p]
    if kind == 'AB':
        return (dy, dx), (dy, dx + 1)
    if kind == 'V':
        return (dy, dx), (dy + 1, dx)
    return (dy, dx), None


def build_A():
    nc = bass.Bass(trn_type="TRN2", name="kernA")
    fb = nc.dram_tensor("fb", (128, GD + 2 * EXTF), F8, kind="ExternalInput")
    w1 = nc.dram_tensor("w1", (128, 2 * 5 * 2 * 128), F8, kind="ExternalInput")
    b1 = nc.dram_tensor("b1", (128, 2), F32, kind="ExternalInput")
    w2 = nc.dram_tensor("w2", (128, 9 * 2 * 128), F8, kind="ExternalInput")
    b2 = nc.dram_tensor("b2", (128, 1), F32, kind="ExternalInput")
    w3 = nc.dram_tensor("w3", (128, 5 * 2 * 912), F8, kind="ExternalInput")
    xu = nc.dram_tensor("xu", (8192, 912), BF16, kind="ExternalInput")
    m36 = nc.dram_tensor("m36", (128, 36), F32, kind="ExternalInput")
    m34 = nc.dram_tensor("m34", (128, 34), F32, kind="ExternalInput")
    corr = nc.dram_tensor("corr", (64, 512), F32, kind="ExternalOutput")

    def win(tile_ap, off, stride, nl):
        return mkap_s(tile_ap, off, [[tile_ap.ap[0][0], 128], [stride, 2],
                                     [1, nl]])

    with tile.TileContext(nc) as tc:
        with tc.tile_pool(name="cst", bufs=1) as cst, \
             tc.tile_pool(name="hp", bufs=1) as hp, \
             tc.tile_pool(name="psum", bufs=2, space="PSUM") as psp:
            w3t = cst.tile([128, 5 * 2 * 912], F8)
            nc.sync.dma_start(w3t[:], w3[:])

            w2t = cst.tile([128, 9 * 2 * 128], F8)
            nc.sync.dma_start(w2t[:], w2[:])
            b2t = cst.tile([128, 1], F32)
            nc.sync.dma_start(b2t[:], b2[:])
            m34t = cst.tile([128, 34], F32)
            nc.sync.dma_start(m34t[:], m34[:])

            h2 = hp.tile([128, GD + 2 * EXTH2 + 144], F8)
            dum = cst.tile([128, 512], BF16)
            nc.vector.memset(dum[:], 0.0)
            for wi in range(8):
                pw = psp.tile([128, 512], F32, tag="cv", name=f"warm{wi}",
                              bufs=2)
                nc.tensor.matmul(pw[:], lhsT=dum[:, 0:128], rhs=dum[:],
                                 start=True, stop=True)

            with tc.tile_pool(name="h1p", bufs=1) as h1p:
                h1 = h1p.tile([128, GD + 2 * EXTH1 + GD], F8)
                with tc.tile_pool(name="fp", bufs=1) as fp:
                    w1t = fp.tile([128, 2 * 5 * 2 * 128], F8)
                    nc.sync.dma_start(w1t[:], w1[:])
                    b1t = fp.tile([128, 2], F32)
                    nc.sync.dma_start(b1t[:], b1[:])
                    m36t = fp.tile([128, 36], F32)
                    nc.sync.dma_start(m36t[:], m36[:])
                    fbt = fp.tile([128, GD + 2 * EXTF], F8)
                    FB1 = GD + 13 * RP
                    FB2 = GD + 26 * RP
                    nc.sync.dma_start(fbt[:, :FB1], fb[:, :FB1])
                    nc.sync.dma_start(fbt[:, FB1:FB2], fb[:, FB1:FB2])
                    nc.sync.dma_start(fbt[:, FB2:], fb[:, FB2:])
                    w1v = w1t[:].rearrange("q (m p k c) -> q m p k c", m=2,
                                           p=5, k=2)

                    # conv1: 128 -> 256 (2 M halves), 5 DoubleRow passes
                    for m in range(2):
                        for n0, nl in nchunks(EXTH1, 512):
                            ps = psp.tile([128, 512], F32, tag="cv",
                                          name=f"c1_{m}_{n0}", bufs=2)
                            for p, ((dy, dx), kind) in enumerate(PAIRS):
                                off = GD + n0 + (1 + dy) * RP + dx
                                st = EXTF if kind == 'AB' else RP
                                nc.tensor.matmul(
                                    ps[:, :nl], lhsT=w1v[:, m, p, :, :],
                                    rhs=win(fbt[:], off, st, nl),
                                    start=(p == 0), stop=(p == 4),
                                    perf_mode=DR)
                            nc.scalar.activation(
                                h1[:, GD + m * EXTH1 + n0:
                                   GD + m * EXTH1 + n0 + nl],
                                ps[:, :nl], AF.Relu, bias=b1t[:, m:m + 1])
                    for m in range(2):
                        h3 = h1[:, GD + m * EXTH1:GD + (m + 1) * EXTH1] \
                            .rearrange("q (r c) -> q r c", r=36)
                        nc.vector.memset(h3[:, :, 0:1], 0.0)
                        nc.vector.memset(h3[:, :, 257:258], 0.0)
                        for r in (0, 1, 34, 35):
                            nc.vector.tensor_scalar_mul(
                                h3[:, r, :], h3[:, r, :], m36t[:, r:r + 1])

                # conv2: 256 -> 128, 9 DoubleRow passes over kc halves
                w2v = w2t[:].rearrange("q (t k c) -> q t k c", t=9, k=2)
                for n0, nl in nchunks(EXTH2, 512):
                    ps = psp.tile([128, 512], F32, tag="cv",
                                  name=f"c2_{n0}", bufs=2)
                    for t, (dy, dx) in enumerate(TAPS):
                        off = GD + n0 + (1 + dy) * RP + dx
                        nc.tensor.matmul(
                            ps[:, :nl], lhsT=w2v[:, t, :, :],
                            rhs=win(h1[:], off, EXTH1, nl),
                            start=(t == 0), stop=(t == 8), perf_mode=DR)
                    nc.scalar.activation(
                        h2[:, GD + n0:GD + n0 + nl], ps[:, :nl], AF.Relu,
                        bias=b2t[:])
                h23 = h2[:, GD:GD + EXTH2].rearrange("q (r c) -> q r c", r=34)
                nc.vector.memset(h23[:, :, 0:1], 0.0)
                nc.vector.memset(h23[:, :, 257:258], 0.0)
                for r in (0, 33):
                    nc.vector.tensor_scalar_mul(
                        h23[:, r, :], h23[:, r, :], m34t[:, r:r + 1])

            # shifted copy for conv3 lhsT k-tile pairing (copy1[x]=copy0[x+1])
            BND = 9 * RP
            for bb in range(4):
                a0 = bb * BND
                a1 = min(EXTH2 - 1, a0 + BND)
                nc.sync.dma_start(h2[:, GD + EXTH2 + a0:GD + EXTH2 + a1],
                                  h2[:, GD + 1 + a0:GD + 1 + a1])
            # ones region for the bias k-tile of conv3 pass 4
            OB = GD + 2 * EXTH2 + 2
            nc.vector.memset(h2[:, OB:OB + 128], 1.0)

            # conv3 transposed + softmax tail, per 128-pixel group.
            # bias lands in psum via a K=1 ones-matmul; exp(b3) is folded
            # into xu on host; D comes free from exp accum_out. Division
            # and output DMA are batched over 8 groups.
            w3v = w3t[:].rearrange("q (p k c) -> q p k c", p=5, k=2)
            GB = 8
            with tc.tile_pool(name="gp", bufs=4) as gp, \
                 tc.tile_pool(name="bp", bufs=2) as bp:
                for g in range(64):
                    r, cc = g // 2, g % 2
                    gi = g % GB
                    if gi == 0:
                        Ns = bp.tile([128, GB * 4], F32, tag="Ns",
                                     name=f"Ns{g}")
                        Ds = bp.tile([128, GB * 4], F32, tag="Ds",
                                     name=f"Ds{g}")
                    Xg = gp.tile([128, 912], BF16, tag="Xg", name=f"Xg{g}")
                    xq = nc.sync if g % 2 == 0 else nc.gpsimd
                    xq.dma_start(Xg[:], xu[g * 128:(g + 1) * 128, :])
                    pss = [psp.tile([128, 456], F32, tag=f"c3{j}",
                                    name=f"c3_{g}_{j}", bufs=3)
                           for j in range(2)]
                    for p, ((dy, dx), kind) in enumerate(PAIRS):
                        off = GD + (r + 1 + dy) * RP + cc * 128 + 1 + dx
                        if kind == 'AB':
                            st = EXTH2
                        elif kind == 'V':
                            st = RP
                        else:  # Z: kt1 = ones region (bias via w3 row 0)
                            st = OB - off
                        for j in range(2):
                            nc.tensor.matmul(
                                pss[j][:],
                                lhsT=win(h2[:], off, st, 128),
                                rhs=w3v[:, p, :, j * 456:(j + 1) * 456],
                                start=(p == 0), stop=(p == 4), perf_mode=DR)
                    E = gp.tile([128, 912], BF16, tag="E", name=f"E{g}")
                    for c in range(4):
                        nc.scalar.activation(
                            E[:, c * 228:(c + 1) * 228],
                            pss[c // 2][:, (c % 2) * 228:(c % 2) * 228 + 228],
                            AF.Exp, accum_out=Ds[:, gi * 4 + c:gi * 4 + c + 1])
                    Pt = gp.tile([128, 912], BF16, tag="Pt", name=f"Pt{g}")
                    nc.vector.tensor_tensor(out=Pt[:], in0=E[:], in1=Xg[:],
                                            op=ALU.mult)
                    nc.vector.tensor_reduce(
                        Ns[:, gi * 4:gi * 4 + 4],
                        Pt[:].rearrange("q (a b) -> q a b", a=4),
                        AX.X, ALU.add)
                    if gi == GB - 1:
                        nc.vector.reciprocal(Ds[:], Ds[:])
                        nc.vector.tensor_tensor(out=Ns[:], in0=Ns[:],
                                                in1=Ds[:], op=ALU.mult)
                        nc.sync.dma_start(
                            mkap(corr[:], (g - GB + 1) * 512,
                                 [[1, 128], [512, GB], [128, 4]]), Ns[:])
    return nc


def mkap_s(base_ap, off, pairs):
    a = base_ap.copy()
    a.offset = base_ap.offset + off
    a.ap = bass_rust.VecI64Pair([list(p) for p in pairs])
    return a


def build_A_old():
    nc = bass.Bass(trn_type="TRN2", name="kernA")
    feat = nc.dram_tensor("feat", (128, 38 * 256), F32, kind="ExternalInput")
    raw46 = nc.dram_tensor("raw46", (C, 46, 270), BF16, kind="ExternalInput")
    w1 = nc.dram_tensor("w1", (128, 2 * 9 * 128), BF16, kind="ExternalInput")
    b1 = nc.dram_tensor("b1", (128, 2), F32, kind="ExternalInput")
    w2 = nc.dram_tensor("w2", (128, 2 * 9 * 128), BF16, kind="ExternalInput")
    b2 = nc.dram_tensor("b2", (128, 1), F32, kind="ExternalInput")
    w3 = nc.dram_tensor("w3", (128, 9 * 1024), BF16, kind="ExternalInput")
    b3 = nc.dram_tensor("b3", (128, 8), F32, kind="ExternalInput")
    m36 = nc.dram_tensor("m36", (128, 36), F32, kind="ExternalInput")
    m34 = nc.dram_tensor("m34", (128, 34), F32, kind="ExternalInput")
    corr = nc.dram_tensor("corr", (C, ROWS, W), F32, kind="ExternalOutput")

    NF36, NF34 = 36 * WP, 34 * WP

    with tile.TileContext(nc) as tc:
        with tc.tile_pool(name="cst", bufs=1) as cst, \
             tc.tile_pool(name="hp", bufs=1) as hp, \
             tc.tile_pool(name="psum", bufs=2, space="PSUM") as psp:
            w3t = cst.tile([128, 9 * 1024], BF16)
            nc.sync.dma_start(w3t[:], w3[:])
            b3t = cst.tile([128, 8], F32)
            nc.sync.dma_start(b3t[:], b3[:])
            b2t = cst.tile([128, 1], F32)
            nc.sync.dma_start(b2t[:], b2[:])
            m34t = cst.tile([128, 34], F32)
            nc.sync.dma_start(m34t[:], m34[:])
            ones = cst.tile([128, 1], BF16)
            nc.vector.memset(ones[:], 1.0)

            h2 = hp.tile([128, NF34], BF16)

            with tc.tile_pool(name="h1p", bufs=1) as h1p:
                h1 = [h1p.tile([128, NF36 + 8], BF16, name=f"h1_{m}", tag=f"h1_{m}") for m in range(2)]
                w2t = h1p.tile([128, 2 * 9 * 128], BF16)
                nc.sync.dma_start(w2t[:], w2[:])

                with tc.tile_pool(name="fp", bufs=1) as fp:
                    w1t = fp.tile([128, 2 * 9 * 128], BF16)
                    nc.sync.dma_start(w1t[:], w1[:])
                    b1t = fp.tile([128, 2], F32)
                    nc.sync.dma_start(b1t[:], b1[:])
                    m36t = fp.tile([128, 36], F32)
                    nc.sync.dma_start(m36t[:], m36[:])
                    ff = fp.tile([128, 38 * 256], F32)
                    nc.sync.dma_start(ff[:], feat[:])
                    fb = fp.tile([128, 38 * WP + 8], BF16)
                    nc.vector.memset(fb[:], 0.0)
                    nc.vector.tensor_copy(
                        fb[:, 1:1 + 38 * WP].rearrange(
                            "p (r c) -> p r c", r=38)[:, :, 1:257],
                        ff[:].rearrange("p (r c) -> p r c", r=38))

                    # conv1: 128 -> 256 (2 M chunks), taps-outer groups of 3
                    for m in range(2):
                        ch1 = nchunks(NF36, 512)
                        for g0 in range(0, len(ch1), 3):
                            grp = ch1[g0:g0 + 3]
                            pcs = [psp.tile([128, 512], F32, tag=f"pc{j}",
                                            name=f"c1_{m}_{g0}_{j}", bufs=1)
                                   for j in range(len(grp))]
                            for t, (dy, dx) in enumerate(TAPS):
                                base = (1 + dy) * WP + dx
                                for j, (n0, nl) in enumerate(grp):
                                    nc.tensor.matmul(
                                        pcs[j][:, :nl],
                                        lhsT=w1t[:, (m * 9 + t) * 128:(m * 9 + t + 1) * 128],
                                        rhs=fb[:, 1 + n0 + base:1 + n0 + base + nl],
                                        start=(t == 0), stop=(t == 8))
                            for j, (n0, nl) in enumerate(grp):
                                nc.scalar.activation(
                                    h1[m][:, 1 + n0:1 + n0 + nl], pcs[j][:, :nl],
                                    AF.Relu, bias=b1t[:, m:m + 1])
                        h3 = h1[m][:, 1:1 + NF36].rearrange("p (r c) -> p r c", r=36)
                        nc.vector.memset(h3[:, :, 0:1], 0.0)
                        nc.vector.memset(h3[:, :, 257:258], 0.0)
                        # zero out-of-image rows (only rows 0,1,34,35 can be OOI)
                        for r in (0, 1, 34, 35):
                            nc.vector.tensor_scalar_mul(
                                h3[:, r, :], h3[:, r, :], m36t[:, r:r + 1])

                # conv2: 256 -> 128 (2 K chunks), taps-outer groups of 3
                ch2 = nchunks(NF34, 512)
                for g0 in range(0, len(ch2), 3):
                    grp = ch2[g0:g0 + 3]
                    pcs = [psp.tile([128, 512], F32, tag=f"pc{j}",
                                    name=f"c2_{g0}_{j}", bufs=1)
                           for j in range(len(grp))]
                    ti = 0
                    for kc in range(2):
                        for t, (dy, dx) in enumerate(TAPS):
                            base = (1 + dy) * WP + dx
                            for j, (n0, nl) in enumerate(grp):
                                nc.tensor.matmul(
                                    pcs[j][:, :nl],
                                    lhsT=w2t[:, (kc * 9 + t) * 128:(kc * 9 + t + 1) * 128],
                                    rhs=h1[kc][:, 1 + n0 + base:1 + n0 + base + nl],
                                    start=(ti == 0), stop=(ti == 17))
                            ti += 1
                    for j, (n0, nl) in enumerate(grp):
                        nc.scalar.activation(
                            h2[:, n0:n0 + nl], pcs[j][:, :nl], AF.Relu, bias=b2t[:])
                h23 = h2[:].rearrange("p (r c) -> p r c", r=34)
                nc.vector.memset(h23[:, :, 0:1], 0.0)
                nc.vector.memset(h23[:, :, 257:258], 0.0)
                for r in (0, 33):
                    nc.vector.tensor_scalar_mul(
                        h23[:, r, :], h23[:, r, :], m34t[:, r:r + 1])

            # conv3 + softmax + deconv per (pixchunk, channel).
            # psf channels padded 900->1024: image channel c = M-chunks
            # {2c, 2c+1}; taps 0..224 real, 225..255 padded (bias -30).
            RPC = 8
            PCN = RPC * W  # 2048
            h2v = h2[:].rearrange("p (r q) -> p r q", r=34)
            with tc.tile_pool(name="ex", bufs=2) as exp_pool, \
                 tc.tile_pool(name="xp", bufs=2) as xpool, \
                 tc.tile_pool(name="scp", bufs=2) as scp, \
                 tc.tile_pool(name="dnp", bufs=2, space="DRAM") as dnp, \
                 tc.tile_pool(name="rbp", bufs=2) as rbp:
                for pc_i in range(ROWS // RPC):
                    r0 = pc_i * RPC
                    dnd = dnp.tile([C, 2 * PCN], F32, tag="dnd")
                    for c in range(C):
                        Ea = exp_pool.tile([128, PCN], BF16, tag="Ea")
                        Eb = exp_pool.tile([128, PCN], BF16, tag="Eb")
                        Pa = exp_pool.tile([128, PCN], BF16, tag="Pa")
                        Pb = exp_pool.tile([128, PCN], BF16, tag="Pb")
                        Xa = xpool.tile([128, PCN], BF16, tag="Xa")
                        Xb = xpool.tile([128, PCN], BF16, tag="Xb")
                        # patch strips: partition t = dy*15+dx, free = pixel
                        for dy in range(KK):
                            t0 = dy * KK
                            off = c * 46 * 270 + (r0 + dy) * 270
                            if t0 + KK <= 128:
                                nc.sync.dma_start(
                                    Xa[t0:t0 + KK, :],
                                    mkap(raw46[:], off, [[1, KK], [270, RPC], [1, W]]))
                            elif t0 >= 128:
                                nc.sync.dma_start(
                                    Xb[t0 - 128:t0 - 128 + KK, :],
                                    mkap(raw46[:], off, [[1, KK], [270, RPC], [1, W]]))
                            else:
                                n1 = 128 - t0
                                nc.sync.dma_start(
                                    Xa[t0:128, :],
                                    mkap(raw46[:], off, [[1, n1], [270, RPC], [1, W]]))
                                nc.sync.dma_start(
                                    Xb[0:KK - n1, :],
                                    mkap(raw46[:], off + n1,
                                         [[1, KK - n1], [270, RPC], [1, W]]))
                        # conv3 -> exp (bias fused into exp's activation)
                        for half, E in ((0, Ea), (1, Eb)):
                            mc = c * 2 + half
                            chunks = nchunks(PCN, 512)
                            pss = [psp.tile([128, 512], F32, tag=f"pc{j}",
                                            name=f"ps_{mc}_{j}", bufs=1)
                                   for j in range(len(chunks))]
                            for t, (dy, dx) in enumerate(TAPS):
                                for j, (s0, sl) in enumerate(chunks):
                                    rr = r0 + s0 // W + 1 + dy
                                    nc.tensor.matmul(
                                        pss[j][:, :sl],
                                        lhsT=w3t[:, t * 1024 + mc * 128:
                                                 t * 1024 + (mc + 1) * 128],
                                        rhs=h2v[:, rr:rr + 2, 1 + dx:257 + dx],
                                        start=(t == 0), stop=(t == 8))
                            for j, (s0, sl) in enumerate(chunks):
                                nc.scalar.activation(
                                    E[:, s0:s0 + sl], pss[j][:, :sl], AF.Exp,
                                    bias=b3t[:, mc:mc + 1])
                        # tap sums via ones-matmuls on PE (GPSIMD C-reduce
                        # is ~40us/op; PE does it in ~0.2us/chunk)
                        nc.vector.tensor_tensor(out=Pa[:, :], in0=Ea[:, :], in1=Xa[:, :], op=ALU.mult)
                        nc.vector.tensor_tensor(out=Pb[0:97, :], in0=Eb[0:97, :], in1=Xb[0:97, :], op=ALU.mult)
                        sc = scp.tile([1, 2 * PCN], F32, tag="sc")
                        da, na = sc[:, 0:PCN], sc[:, PCN:2 * PCN]
                        for s0, sl in nchunks(PCN, 512):
                            for dst, ta, tb in ((da, Ea, Eb), (na, Pa, Pb)):
                                pr = psp.tile([1, 512], F32, tag="pr", bufs=2)
                                nc.tensor.matmul(pr[:, :sl], lhsT=ones[:, :],
                                                 rhs=ta[:, s0:s0 + sl],
                                                 start=True, stop=False)
                                nc.tensor.matmul(pr[:, :sl], lhsT=ones[0:97, :],
                                                 rhs=tb[0:97, s0:s0 + sl],
                                                 start=False, stop=True)
                                nc.vector.tensor_copy(dst[:, s0:s0 + sl], pr[:, :sl])
                        nc.sync.dma_start(dnd[c, :], sc[:, :])
                    # reshape [1,2048]x2 per ch -> [128,64] so the divide
                    # runs on all 128 lanes instead of one
                    Dt = rbp.tile([128, 64], F32, tag="Dt")
                    Nt = rbp.tile([128, 64], F32, tag="Nt")
                    for c in range(C):
                        nc.sync.dma_start(
                            Dt[32 * c:32 * c + 32, :],
                            mkap(dnd[:], c * 2 * PCN, [[64, 32], [1, 64]]))
                        nc.sync.dma_start(
                            Nt[32 * c:32 * c + 32, :],
                            mkap(dnd[:], c * 2 * PCN + PCN, [[64, 32], [1, 64]]))
                    nc.vector.reciprocal(Dt[:], Dt[:])
                    nc.vector.tensor_tensor(out=Nt[:], in0=Nt[:], in1=Dt[:], op=ALU.mult)
                    nc.sync.dma_start(corr[:, r0:r0 + RPC, :], Nt[:])
    return nc




# ================================================================ kernel B1
# Forward rfft2 via DFT matmuls, replicated on every core; writes full fri.
# V[h,k] = sum_w x[h,w] Fw[w,k];  Y[k1,k] = sum_h Fh[k1,h] V[h,k]
# fri = [Yre(4ch), Yim(4ch)] as [8, 256, 129].

def build_B1():
    nc = bass.Bass(trn_type="TRN2", name="kernB1")
    corrT = nc.dram_tensor("corrT", (C, 256, 256), BF16, kind="ExternalInput")
    fwre = nc.dram_tensor("fwre", (256, 129), BF16, kind="ExternalInput")
    fwim = nc.dram_tensor("fwim", (256, 129), BF16, kind="ExternalInput")
    fhre = nc.dram_tensor("fhre", (256, 256), BF16, kind="ExternalInput")
    fhim = nc.dram_tensor("fhim", (256, 256), BF16, kind="ExternalInput")
    fhimn = nc.dram_tensor("fhimn", (256, 256), BF16, kind="ExternalInput")
    fri = nc.dram_tensor("fri", (8, 256, 129), F32, kind="ExternalOutput")

    with tile.TileContext(nc) as tc:
        with tc.tile_pool(name="cst", bufs=1) as cst, \
             tc.tile_pool(name="wk", bufs=2) as wk, \
             tc.tile_pool(name="ps", bufs=4, space="PSUM") as psp:
            fw = [cst.tile([128, 2 * 129], BF16, name=f"fw_{i}", tag=f"fw_{i}") for i in range(2)]
            for kc in range(2):
                nc.sync.dma_start(fw[kc][:, 0:129], fwre[kc * 128:(kc + 1) * 128, :])
                nc.sync.dma_start(fw[kc][:, 129:258], fwim[kc * 128:(kc + 1) * 128, :])
            fh = [cst.tile([128, 3 * 256], BF16, name=f"fh_{i}", tag=f"fh_{i}") for i in range(2)]
            for kc in range(2):
                nc.sync.dma_start(fh[kc][:, 0:256], fhre[kc * 128:(kc + 1) * 128, :])
                nc.sync.dma_start(fh[kc][:, 256:512], fhim[kc * 128:(kc + 1) * 128, :])
                nc.sync.dma_start(fh[kc][:, 512:768], fhimn[kc * 128:(kc + 1) * 128, :])
            for c in range(C):
                xT = [wk.tile([128, 256], BF16, name=f"xT{i}", tag=f"xT{i}") for i in range(2)]
                for kc in range(2):
                    nc.sync.dma_start(xT[kc][:], corrT[c, kc * 128:(kc + 1) * 128, :])
                V = [wk.tile([128, 2 * 129], BF16, name=f"V{i}", tag=f"V{i}") for i in range(2)]
                for mc in range(2):      # output h chunk
                    for ri in range(2):  # re / im
                        pv = psp.tile([128, 129], F32, tag="pv")
                        for kc in range(2):
                            nc.tensor.matmul(
                                pv[:, :],
                                lhsT=xT[kc][:, mc * 128:(mc + 1) * 128],
                                rhs=fw[kc][:, ri * 129:(ri + 1) * 129],
                                start=(kc == 0), stop=(kc == 1))
                        nc.vector.tensor_copy(V[mc][:, ri * 129:(ri + 1) * 129], pv[:, :])
                # Y: for re out: FhRe@Vre + FhImNeg@Vim ; im out: FhIm@Vre + FhRe@Vim
                for mc in range(2):      # k1 chunk
                    for ri in range(2):  # re / im output
                        py = psp.tile([128, 129], F32, tag="pv")
                        for kc in range(2):
                            if ri == 0:
                                t1, t2 = 0, 512   # re, imneg
                            else:
                                t1, t2 = 256, 0   # im, re
                            nc.tensor.matmul(
                                py[:, :],
                                lhsT=fh[kc][:, t1 + mc * 128:t1 + (mc + 1) * 128],
                                rhs=V[kc][:, 0:129],
                                start=(kc == 0), stop=False)
                            nc.tensor.matmul(
                                py[:, :],
                                lhsT=fh[kc][:, t2 + mc * 128:t2 + (mc + 1) * 128],
                                rhs=V[kc][:, 129:258],
                                start=False, stop=(kc == 1))
                        ys = wk.tile([128, 129], F32, tag="ys")
                        nc.scalar.activation(ys[:], py[:], AF.Copy)
                        nc.sync.dma_start(
                            fri[ri * 4 + c, mc * 128:(mc + 1) * 128, :], ys[:])
    return nc


# ================================================================ kernel B2
# freq conv stack on fri slab (38 rows, ch-major) + partial inverse fft.
WF = 131  # 129 + 2 pad cols

def build_B2():
    nc = bass.Bass(trn_type="TRN2", name="kernB2")
    fri = nc.dram_tensor("fri", (8, 38 * WF), BF16, kind="ExternalInput")
    gw1 = nc.dram_tensor("gw1", (8, 9 * 64), BF16, kind="ExternalInput")
    gb1 = nc.dram_tensor("gb1", (64, 1), F32, kind="ExternalInput")
    gw2 = nc.dram_tensor("gw2", (64, 9 * 64), BF16, kind="ExternalInput")
    gb2 = nc.dram_tensor("gb2", (64, 1), F32, kind="ExternalInput")
    gw3 = nc.dram_tensor("gw3", (64, 9 * 8), BF16, kind="ExternalInput")
    gb3 = nc.dram_tensor("gb3", (8, 1), F32, kind="ExternalInput")
    mf36 = nc.dram_tensor("mf36", (64, 36), F32, kind="ExternalInput")
    mf34 = nc.dram_tensor("mf34", (64, 34), F32, kind="ExternalInput")
    cfo = nc.dram_tensor("cfo", (8, 32 * 129), F32, kind="ExternalOutput")

    N36, N34, N32 = 36 * WF, 34 * WF, 32 * WF

    with tile.TileContext(nc) as tc:
        with tc.tile_pool(name="cst", bufs=1) as cst, \
             tc.tile_pool(name="gp", bufs=1) as gp, \
             tc.tile_pool(name="ps", bufs=4, space="PSUM") as psp:
            w1t = cst.tile([8, 9 * 64], BF16)
            nc.sync.dma_start(w1t[:], gw1[:])
            w2t = cst.tile([64, 9 * 64], BF16)
            nc.sync.dma_start(w2t[:], gw2[:])
            w3t = cst.tile([64, 9 * 8], BF16)
            nc.sync.dma_start(w3t[:], gw3[:])
            b1t = cst.tile([64, 1], F32)
            nc.sync.dma_start(b1t[:], gb1[:])
            b2t = cst.tile([64, 1], F32)
            nc.sync.dma_start(b2t[:], gb2[:])
            b3t = cst.tile([8, 1], F32)
            nc.sync.dma_start(b3t[:], gb3[:])
            m36t = cst.tile([64, 36], F32)
            nc.sync.dma_start(m36t[:], mf36[:])
            m34t = cst.tile([64, 34], F32)
            nc.sync.dma_start(m34t[:], mf34[:])

            ft = gp.tile([8, 1 + 38 * WF + 4], BF16)
            nc.sync.dma_start(ft[:, 1:1 + 38 * WF], fri[:, :])
            g1 = gp.tile([64, 1 + N36 + 4], BF16)
            g2 = gp.tile([64, 1 + N34 + 4], BF16)
            g3 = gp.tile([8, N32], F32)

            for n0, nl in nchunks(N36, 512):
                pc = psp.tile([64, 512], F32, tag="pg")
                for t, (dy, dx) in enumerate(TAPS):
                    base = (1 + dy) * WF + dx
                    nc.tensor.matmul(
                        pc[:, :nl],
                        lhsT=w1t[:, t * 64:(t + 1) * 64],
                        rhs=ft[:, 1 + n0 + base:1 + n0 + base + nl],
                        start=(t == 0), stop=(t == 8))
                nc.scalar.activation(g1[:, 1 + n0:1 + n0 + nl], pc[:, :nl],
                                     AF.Relu, bias=b1t[:])
            g1v = g1[:, 1:1 + N36].rearrange("p (r q) -> p r q", r=36)
            nc.vector.memset(g1v[:, :, 0:1], 0.0)
            nc.vector.memset(g1v[:, :, 130:131], 0.0)
            for r in (0, 1, 34, 35):
                nc.vector.tensor_scalar_mul(g1v[:, r, :], g1v[:, r, :],
                                            m36t[:, r:r + 1])
            for n0, nl in nchunks(N34, 512):
                pc = psp.tile([64, 512], F32, tag="pg")
                for t, (dy, dx) in enumerate(TAPS):
                    base = (1 + dy) * WF + dx
                    nc.tensor.matmul(
                        pc[:, :nl],
                        lhsT=w2t[:, t * 64:(t + 1) * 64],
                        rhs=g1[:, 1 + n0 + base:1 + n0 + base + nl],
                        start=(t == 0), stop=(t == 8))
                nc.scalar.activation(g2[:, 1 + n0:1 + n0 + nl], pc[:, :nl],
                                     AF.Relu, bias=b2t[:])
            g2v = g2[:, 1:1 + N34].rearrange("p (r q) -> p r q", r=34)
            nc.vector.memset(g2v[:, :, 0:1], 0.0)
            nc.vector.memset(g2v[:, :, 130:131], 0.0)
            for r in (0, 33):
                nc.vector.tensor_scalar_mul(g2v[:, r, :], g2v[:, r, :],
                                            m34t[:, r:r + 1])
            for n0, nl in nchunks(N32, 512):
                pc = psp.tile([8, 512], F32, tag="pg")
                for t, (dy, dx) in enumerate(TAPS):
                    base = (1 + dy) * WF + dx
                    nc.tensor.matmul(
                        pc[:, :nl],
                        lhsT=w3t[:, t * 8:(t + 1) * 8],
                        rhs=g2[:, 1 + n0 + base:1 + n0 + base + nl],
                        start=(t == 0), stop=(t == 8))
                nc.scalar.activation(g3[:, n0:n0 + nl], pc[:, :nl],
                                     AF.Copy, bias=0.0)
            # add bias gb3 separately (Copy cannot take AP bias)
            nc.vector.tensor_scalar(out=g3[:], in0=g3[:], scalar1=b3t[:],
                                    scalar2=None, op0=ALU.add)

            # write CF slab [8 (ri,c), 32 k1-rows, 129] (strip pad cols;
            # real bins live at cols 1..129 of the WF=131 grid)
            nc.sync.dma_start(
                cfo[:, :], mkap_s(g3[:], 1, [[N32, 8], [WF, 32], [1, 129]]))
    return nc


# ================================================================ kernel B
# merged forward DFT + freq convs, one dispatch. V (row FFT) needs all
# columns of the full image (replicated); Y (col FFT) computed only for
# this core's 38-row k1 slab; freq convs 2-half row-packed (bf16).
# partition layouts: ft/g3: p = (ri*4+c)*2 + h; g1/g2: p = u*2 + h.

def build_B():
    nc = bass.Bass(trn_type="TRN2", name="kernB")
    corrT = nc.dram_tensor("corrT", (C, 256, 256), BF16, kind="ExternalInput")
    fwre = nc.dram_tensor("fwre", (256, 129), BF16, kind="ExternalInput")
    fwim = nc.dram_tensor("fwim", (256, 129), BF16, kind="ExternalInput")
    fhs = nc.dram_tensor("fhs", (128, 2 * 3 * 38), BF16, kind="ExternalInput")
    gw1 = nc.dram_tensor("gw1", (16, 9 * 128), BF16, kind="ExternalInput")
    gb1 = nc.dram_tensor("gb1", (128, 1), F32, kind="ExternalInput")
    gw2 = nc.dram_tensor("gw2", (128, 9 * 128), BF16, kind="ExternalInput")
    gb2 = nc.dram_tensor("gb2", (128, 1), F32, kind="ExternalInput")
    gw3 = nc.dram_tensor("gw3", (128, 9 * 16), BF16, kind="ExternalInput")
    gb3 = nc.dram_tensor("gb3", (16, 1), F32, kind="ExternalInput")
    mf20 = nc.dram_tensor("mf20", (128, 20), F32, kind="ExternalInput")
    mf18 = nc.dram_tensor("mf18", (128, 18), F32, kind="ExternalInput")
    cfo = nc.dram_tensor("cfo", (8, 32 * 129), F32, kind="ExternalOutput")

    EXB = 22 * WF       # ft half extent (22 rows x 131)
    EXB1 = 20 * WF
    EXB2 = 18 * WF
    EXB3 = 16 * WF

    with tile.TileContext(nc) as tc:
        with tc.tile_pool(name="cst", bufs=1) as cst, \
             tc.tile_pool(name="gp", bufs=1) as gp, \
             tc.tile_pool(name="ps", bufs=2, space="PSUM") as psp:
            fw = cst.tile([128, 2 * 2 * 129], BF16)
            for kc in range(2):
                nc.sync.dma_start(fw[:, kc * 258:kc * 258 + 129],
                                  fwre[kc * 128:(kc + 1) * 128, :])
                nc.sync.dma_start(fw[:, kc * 258 + 129:kc * 258 + 258],
                                  fwim[kc * 128:(kc + 1) * 128, :])
            fhst = cst.tile([128, 2 * 3 * 38], BF16)
            nc.sync.dma_start(fhst[:], fhs[:])
            fhsv = fhst[:].rearrange("q (k m h) -> q k m h", k=2, m=3)
            w1t = cst.tile([16, 9 * 128], BF16)
            nc.sync.dma_start(w1t[:], gw1[:])
            w2t = cst.tile([128, 9 * 128], BF16)
            nc.sync.dma_start(w2t[:], gw2[:])
            w3t = cst.tile([128, 9 * 16], BF16)
            nc.sync.dma_start(w3t[:], gw3[:])
            b1t = cst.tile([128, 1], F32)
            nc.sync.dma_start(b1t[:], gb1[:])
            b2t = cst.tile([128, 1], F32)
            nc.sync.dma_start(b2t[:], gb2[:])
            b3t = cst.tile([16, 1], F32)
            nc.sync.dma_start(b3t[:], gb3[:])
            m20t = cst.tile([128, 20], F32)
            nc.sync.dma_start(m20t[:], mf20[:])
            m18t = cst.tile([128, 18], F32)
            nc.sync.dma_start(m18t[:], mf18[:])

            ft = gp.tile([16, GD + EXB + GD], BF16)
            dum = cst.tile([128, 512], BF16)
            nc.vector.memset(dum[:], 0.0)
            for wi in range(8):
                pw = psp.tile([128, 258], F32, tag="pv", name=f"warm{wi}",
                              bufs=2)
                nc.tensor.matmul(pw[:], lhsT=dum[:, 0:128],
                                 rhs=dum[:, 0:258], start=True, stop=True)
            nc.vector.memset(ft[:], 0.0)
            with tc.tile_pool(name="vp", bufs=3) as vp:
                for c in range(C):
                    xT = vp.tile([128, 2 * 256], BF16, tag="xT",
                                 name=f"xT{c}")
                    for kc in range(2):
                        nc.sync.dma_start(
                            xT[:, kc * 256:(kc + 1) * 256],
                            corrT[c, kc * 128:(kc + 1) * 128, :])
                    V = [vp.tile([128, 2 * 129], BF16, name=f"V{c}_{m}",
                                 tag=f"V{m}") for m in range(2)]
                    for mc in range(2):
                        pv = psp.tile([128, 258], F32, tag="pv",
                                      name=f"pv{c}_{mc}", bufs=2)
                        for kc in range(2):
                            nc.tensor.matmul(
                                pv[:],
                                lhsT=xT[:, kc * 256 + mc * 128:
                                        kc * 256 + (mc + 1) * 128],
                                rhs=fw[:, kc * 258:(kc + 1) * 258],
                                start=(kc == 0), stop=(kc == 1))
                        nc.scalar.activation(V[mc][:], pv[:], AF.Copy)
                    for ri in range(2):
                        py = psp.tile([38, 129], F32, tag="py",
                                      name=f"py{c}_{ri}", bufs=1)
                        t1, t2 = (0, 2) if ri == 0 else (1, 0)
                        ti = 0
                        for hc in range(2):
                            nc.tensor.matmul(
                                py[:], lhsT=fhsv[:, hc, t1, :],
                                rhs=V[hc][:, 0:129],
                                start=(ti == 0), stop=False)
                            ti += 1
                            nc.tensor.matmul(
                                py[:], lhsT=fhsv[:, hc, t2, :],
                                rhs=V[hc][:, 129:258],
                                start=False, stop=(ti == 3))
                            ti += 1
                        ys = vp.tile([38, 129], BF16, tag="ys",
                                     name=f"ys{c}_{ri}")
                        nc.scalar.activation(ys[:], py[:], AF.Copy)
                        for h in range(2):
                            p = (ri * 4 + c) * 2 + h
                            q = nc.sync if h == 0 else nc.gpsimd
                            q.dma_start(
                                mkap_s(ft[p:p + 1, :], GD + 1,
                                       [[GD + EXB + GD, 1], [WF, 22],
                                        [1, 129]]),
                                ys[h * 16:h * 16 + 22, :])

            g1 = gp.tile([128, GD + EXB1 + GD], BF16)
            g2 = gp.tile([128, GD + EXB2 + GD], BF16)
            w1v = w1t[:].rearrange("q (t c) -> q t c", t=9)
            for n0, nl in nchunks(EXB1, 512):
                ps = psp.tile([128, 512], F32, tag="cv", name=f"e1_{n0}",
                              bufs=3)
                for t, (dy, dx) in enumerate(TAPS):
                    off = GD + n0 + (1 + dy) * WF + dx
                    nc.tensor.matmul(ps[:, :nl], lhsT=w1v[:, t, :],
                                     rhs=ft[:, off:off + nl],
                                     start=(t == 0), stop=(t == 8))
                nc.scalar.activation(g1[:, GD + n0:GD + n0 + nl], ps[:, :nl],
                                     AF.Relu, bias=b1t[:])
            g1v = g1[:, GD:GD + EXB1].rearrange("q (r c) -> q r c", r=20)
            nc.vector.memset(g1v[:, :, 0:1], 0.0)
            nc.vector.memset(g1v[:, :, 130:131], 0.0)
            for r in (0, 1, 18, 19):
                nc.vector.tensor_scalar_mul(g1v[:, r, :], g1v[:, r, :],
                                            m20t[:, r:r + 1])
            w2v = w2t[:].rearrange("q (t c) -> q t c", t=9)
            for n0, nl in nchunks(EXB2, 512):
                ps = psp.tile([128, 512], F32, tag="cv", name=f"e2_{n0}",
                              bufs=3)
                for t, (dy, dx) in enumerate(TAPS):
                    off = GD + n0 + (1 + dy) * WF + dx
                    nc.tensor.matmul(ps[:, :nl], lhsT=w2v[:, t, :],
                                     rhs=g1[:, off:off + nl],
                                     start=(t == 0), stop=(t == 8))
                nc.scalar.activation(g2[:, GD + n0:GD + n0 + nl], ps[:, :nl],
                                     AF.Relu, bias=b2t[:])
            g2v = g2[:, GD:GD + EXB2].rearrange("q (r c) -> q r c", r=18)
            nc.vector.memset(g2v[:, :, 0:1], 0.0)
            nc.vector.memset(g2v[:, :, 130:131], 0.0)
            for r in (0, 17):
                nc.vector.tensor_scalar_mul(g2v[:, r, :], g2v[:, r, :],
                                            m18t[:, r:r + 1])
            g3 = gp.tile([16, EXB3], F32)
            w3v = w3t[:].rearrange("q (t c) -> q t c", t=9)
            for n0, nl in nchunks(EXB3, 512):
                ps = psp.tile([16, 512], F32, tag="cv3", name=f"e3_{n0}",
                              bufs=2)
                for t, (dy, dx) in enumerate(TAPS):
                    off = GD + n0 + (1 + dy) * WF + dx
                    nc.tensor.matmul(ps[:, :nl], lhsT=w3v[:, t, :],
                                     rhs=g2[:, off:off + nl],
                                     start=(t == 0), stop=(t == 8))
                nc.scalar.activation(g3[:, n0:n0 + nl], ps[:, :nl], AF.Copy)
            nc.vector.tensor_scalar(out=g3[:], in0=g3[:], scalar1=b3t[:],
                                    scalar2=None, op0=ALU.add)
            nc.sync.dma_start(
                mkap(cfo[:], 0, [[2064, 16], [129, 16], [1, 129]]),
                mkap_s(g3[:], 1, [[EXB3, 16], [WF, 16], [1, 129]]))
    return nc


def run_B(corr1, fw1, fb1, fw2, fb2, fw3, fb3, trace=False):
    if "B" not in _CACHE:
        _CACHE["B"] = patch_nc(build_B())
    fwre, fwim, _, _, _, _, _, _ = _dft_mats()
    corrTh = _bf(np.ascontiguousarray(corr1.transpose(0, 2, 1)))
    # block-diag weights: in p=(j)*2+h (j=ri*4+c), hid p=u*2+h, out p=j*2+h
    w1h = np.zeros((16, 9, 128), np.float32)
    w2h = np.zeros((128, 9, 128), np.float32)
    w3h = np.zeros((128, 9, 16), np.float32)
    b1h = np.zeros((128, 1), np.float32)
    b2h = np.zeros((128, 1), np.float32)
    b3h = np.zeros((16, 1), np.float32)
    for h in range(2):
        for u in range(64):
            b1h[u * 2 + h, 0] = fb1[u]
            b2h[u * 2 + h, 0] = fb2[u]
        for j in range(8):
            b3h[j * 2 + h, 0] = fb3[j]
    for t in range(9):
        dy, dx = t // 3, t % 3
        for h in range(2):
            for u in range(64):
                for j in range(8):
                    w1h[j * 2 + h, t, u * 2 + h] = fw1[u, j, dy, dx]
                    w3h[u * 2 + h, t, j * 2 + h] = fw3[j, u, dy, dx]
                for v in range(64):
                    w2h[v * 2 + h, t, u * 2 + h] = fw2[u, v, dy, dx]
    hhs = np.arange(256)
    ins = []
    for i in range(N_CORES):
        r0 = i * ROWS
        k1s = np.arange(r0 - 3, r0 + 35)
        ok = (k1s >= 0) & (k1s < 256)
        fhsh = np.zeros((128, 2, 3, 38), np.float32)
        for hc in range(2):
            h_ = np.arange(hc * 128, hc * 128 + 128)
            th = 2 * np.pi * np.outer(h_, k1s) / 256.0
            fhsh[:, hc, 0, :] = np.cos(th) / 16.0 * ok[None, :]
            fhsh[:, hc, 1, :] = -np.sin(th) / 16.0 * ok[None, :]
            fhsh[:, hc, 2, :] = np.sin(th) / 16.0 * ok[None, :]
        m20 = np.zeros((128, 20), np.float32)
        m18 = np.zeros((128, 18), np.float32)
        for p in range(128):
            h = p % 2
            base = r0 + h * 16
            for r in range(20):
                m20[p, r] = 1.0 if 0 <= base - 2 + r < 256 else 0.0
            for r in range(18):
                m18[p, r] = 1.0 if 0 <= base - 1 + r < 256 else 0.0
        ins.append({
            "corrT": corrTh, "fwre": fwre, "fwim": fwim,
            "fhs": _bf(fhsh.reshape(128, -1)),
            "gw1": _bf(w1h.reshape(16, -1)), "gb1": b1h,
            "gw2": _bf(w2h.reshape(128, -1)), "gb2": b2h,
            "gw3": _bf(w3h.reshape(128, -1)), "gb3": b3h,
            "mf20": m20, "mf18": m18,
        })
    res = run_bass_kernel_spmd(_CACHE["B"], ins, core_ids=list(range(N_CORES)),
                               trace=trace)
    cf = np.zeros((8, 256, 129), np.float32)
    for i in range(N_CORES):
        f = res.results[i]["cfo"].reshape(16, 16, 129)
        for j in range(8):
            for h in range(2):
                cf[j, i * ROWS + h * 16:i * ROWS + h * 16 + 16, :] = \
                    f[j * 2 + h]
    return None, cf, res


# ================================================================ kernel C
# inverse DFT from full CF (host-gathered) + per-channel refinement as
# 2-half row-packed block-diagonal convs (128 partitions, bf16).
# partition layouts: u/r3: p = c*2 + h; r1/r2: p = c*32 + u*2 + h.
WPC = 258
EXTC = 22 * WPC      # u half extent (22 rows)
EXTR1 = 20 * WPC
EXTR2 = 18 * WPC
EXTR3 = 16 * WPC


def build_C():
    nc = bass.Bass(trn_type="TRN2", name="kernC")
    u0 = nc.dram_tensor("u0", (8, EXTC), BF16, kind="ExternalInput")
    raws = nc.dram_tensor("raws", (8, EXTR3), F32, kind="ExternalInput")
    cfa = nc.dram_tensor("cfa", (128, 4 * 2 * 2 * 128), BF16,
                         kind="ExternalInput")
    cfb = nc.dram_tensor("cfb", (1, 4 * 2 * 2 * 128), BF16,
                         kind="ExternalInput")
    iwm = nc.dram_tensor("iwm", (128, 2 * 512), BF16, kind="ExternalInput")
    iwbm = nc.dram_tensor("iwbm", (1, 2 * 512), BF16, kind="ExternalInput")
    ihs = nc.dram_tensor("ihs", (128, 2 * 2 * 38), BF16,
                         kind="ExternalInput")
    cw1 = nc.dram_tensor("cw1", (8, 9 * 128), BF16, kind="ExternalInput")
    cb1 = nc.dram_tensor("cb1", (128, 1), F32, kind="ExternalInput")
    cw2 = nc.dram_tensor("cw2", (128, 9 * 128), BF16, kind="ExternalInput")
    cb2 = nc.dram_tensor("cb2", (128, 1), F32, kind="ExternalInput")
    cw3 = nc.dram_tensor("cw3", (128, 9 * 8), BF16, kind="ExternalInput")
    cb3 = nc.dram_tensor("cb3", (8, 1), F32, kind="ExternalInput")
    mr20 = nc.dram_tensor("mr20", (128, 20), F32, kind="ExternalInput")
    mr18 = nc.dram_tensor("mr18", (128, 18), F32, kind="ExternalInput")
    fin = nc.dram_tensor("fin", (8, 16 * 256), F32, kind="ExternalOutput")

    with tile.TileContext(nc) as tc:
        with tc.tile_pool(name="cst", bufs=1) as cst, \
             tc.tile_pool(name="gp", bufs=1) as gp, \
             tc.tile_pool(name="ps", bufs=1, space="PSUM") as psp:
            cfat = cst.tile([128, 4 * 2 * 2 * 128], BF16)
            nc.sync.dma_start(cfat[:], cfa[:])
            cfbt = cst.tile([1, 4 * 2 * 2 * 128], BF16)
            nc.sync.dma_start(cfbt[:], cfb[:])
            iwt = cst.tile([128, 2 * 512], BF16)
            nc.sync.dma_start(iwt[:], iwm[:])
            iwbt = cst.tile([1, 2 * 512], BF16)
            nc.sync.dma_start(iwbt[:], iwbm[:])
            ihst = cst.tile([128, 2 * 2 * 38], BF16)
            nc.sync.dma_start(ihst[:], ihs[:])
            w1t = cst.tile([8, 9 * 128], BF16)
            nc.sync.dma_start(w1t[:], cw1[:])
            w2t = cst.tile([128, 9 * 128], BF16)
            nc.sync.dma_start(w2t[:], cw2[:])
            w3t = cst.tile([128, 9 * 8], BF16)
            nc.sync.dma_start(w3t[:], cw3[:])
            b1t = cst.tile([128, 1], F32)
            nc.sync.dma_start(b1t[:], cb1[:])
            b2t = cst.tile([128, 1], F32)
            nc.sync.dma_start(b2t[:], cb2[:])
            b3t = cst.tile([8, 1], F32)
            nc.sync.dma_start(b3t[:], cb3[:])
            m20t = cst.tile([128, 20], F32)
            nc.sync.dma_start(m20t[:], mr20[:])
            m18t = cst.tile([128, 18], F32)
            nc.sync.dma_start(m18t[:], mr18[:])

            dum = cst.tile([128, 512], BF16)
            nc.vector.memset(dum[:], 0.0)
            for wi in range(8):
                pw = psp.tile([128, 512], F32, tag="pb", name=f"warm{wi}",
                              bufs=2)
                nc.tensor.matmul(pw[:], lhsT=dum[:, 0:128], rhs=dum[:],
                                 start=True, stop=True)
            cfav = cfat[:].rearrange("q (c r k m) -> q c r k m", c=4, r=2, k=2)
            cfbv = cfbt[:].rearrange("q (c r k m) -> q c r k m", c=4, r=2, k=2)
            ihsv = ihst[:].rearrange("q (k t h) -> q k t h", k=2, t=2)

            # u = u0 + z (inverse DFT), packed [8, GD + EXTC + GD]
            u = gp.tile([8, GD + EXTC + GD], BF16)
            nc.vector.memset(u[:, 0:GD], 0.0)
            nc.vector.memset(u[:, GD + EXTC:], 0.0)
            nc.sync.dma_start(u[:, GD:GD + EXTC], u0[:])
            zu = gp.tile([8, EXTC], BF16)
            nc.vector.memset(zu[:], 0.0)
            with tc.tile_pool(name="ip", bufs=2) as ip:
                for c in range(C):
                    # B[kc][k1, ri, w] = sum_k cf[c,k1,k] iw[k,w] (complex)
                    Bt = [ip.tile([128, 2 * 256], BF16, tag=f"Bt{kc}",
                                  name=f"Bt_{c}_{kc}") for kc in range(2)]
                    for kc in range(2):
                        pb = psp.tile([128, 512], F32, tag="pb",
                                      name=f"pb_{c}_{kc}", bufs=2)
                        nc.tensor.matmul(
                            pb[:], lhsT=cfav[:, c, 0, kc, :],
                            rhs=iwt[:, 0:512], start=True, stop=False)
                        nc.tensor.matmul(
                            pb[:], lhsT=cfbv[:, c, 0, kc, :],
                            rhs=iwbt[:, 0:512], start=False, stop=False)
                        nc.tensor.matmul(
                            pb[:], lhsT=cfav[:, c, 1, kc, :],
                            rhs=iwt[:, 512:1024], start=False, stop=False)
                        nc.tensor.matmul(
                            pb[:], lhsT=cfbv[:, c, 1, kc, :],
                            rhs=iwbt[:, 512:1024], start=False, stop=True)
                        nc.scalar.activation(Bt[kc][:], pb[:], AF.Copy)
                    # z[hh, w] = sum_k1 ih[k1, hh] B[k1, w] (re part)
                    pz = psp.tile([38, 256], F32, tag="pz",
                                  name=f"pz_{c}", bufs=1)
                    ti = 0
                    for kc in range(2):
                        for term in range(2):
                            nc.tensor.matmul(
                                pz[:], lhsT=ihsv[:, kc, term, :],
                                rhs=Bt[kc][:, term * 256:(term + 1) * 256],
                                start=(ti == 0), stop=(ti == 3))
                            ti += 1
                    zs = ip.tile([38, 256], BF16, tag="zs", name=f"zs_{c}")
                    nc.scalar.activation(zs[:], pz[:], AF.Copy)
                    for h in range(2):
                        zq = nc.sync if h == 0 else nc.gpsimd
                        zq.dma_start(
                            mkap_s(zu[c * 2 + h:c * 2 + h + 1, :], 1,
                                   [[EXTC, 1], [WPC, 22], [1, 256]]),
                            zs[h * 16:h * 16 + 22, :])
            UH = 11 * WPC
            nc.vector.tensor_tensor(out=u[:, GD:GD + UH],
                                    in0=u[:, GD:GD + UH], in1=zu[:, :UH],
                                    op=ALU.add)
            nc.vector.tensor_tensor(out=u[:, GD + UH:GD + EXTC],
                                    in0=u[:, GD + UH:GD + EXTC],
                                    in1=zu[:, UH:], op=ALU.add)

            r1 = gp.tile([128, GD + EXTR1 + GD], BF16)
            r2 = gp.tile([128, GD + EXTR2 + GD], BF16)
            for n0, nl in nchunks(EXTR1, 512):
                ps = psp.tile([128, 512], F32, tag="cv", name=f"d1_{n0}",
                              bufs=3)
                for t, (dy, dx) in enumerate(TAPS):
                    off = GD + n0 + (1 + dy) * WPC + dx
                    nc.tensor.matmul(ps[:, :nl],
                                     lhsT=w1t[:].rearrange(
                                         "q (t c) -> q t c", t=9)[:, t, :],
                                     rhs=u[:, off:off + nl],
                                     start=(t == 0), stop=(t == 8))
                nc.scalar.activation(r1[:, GD + n0:GD + n0 + nl], ps[:, :nl],
                                     AF.Relu, bias=b1t[:])
            r1v = r1[:, GD:GD + EXTR1].rearrange("q (r c) -> q r c", r=20)
            nc.vector.memset(r1v[:, :, 0:1], 0.0)
            nc.vector.memset(r1v[:, :, 257:258], 0.0)
            for r in (0, 1, 18, 19):
                nc.vector.tensor_scalar_mul(r1v[:, r, :], r1v[:, r, :],
                                            m20t[:, r:r + 1])
            for n0, nl in nchunks(EXTR2, 512):
                ps = psp.tile([128, 512], F32, tag="cv", name=f"d2_{n0}",
                              bufs=3)
                for t, (dy, dx) in enumerate(TAPS):
                    off = GD + n0 + (1 + dy) * WPC + dx
                    nc.tensor.matmul(ps[:, :nl],
                                     lhsT=w2t[:].rearrange(
                                         "q (t c) -> q t c", t=9)[:, t, :],
                                     rhs=r1[:, off:off + nl],
                                     start=(t == 0), stop=(t == 8))
                nc.scalar.activation(r2[:, GD + n0:GD + n0 + nl], ps[:, :nl],
                                     AF.Relu, bias=b2t[:])
            r2v = r2[:, GD:GD + EXTR2].rearrange("q (r c) -> q r c", r=18)
            nc.vector.memset(r2v[:, :, 0:1], 0.0)
            nc.vector.memset(r2v[:, :, 257:258], 0.0)
            for r in (0, 17):
                nc.vector.tensor_scalar_mul(r2v[:, r, :], r2v[:, r, :],
                                            m18t[:, r:r + 1])
            r3 = gp.tile([8, EXTR3], F32)
            rawt = gp.tile([8, EXTR3], F32)
            nc.sync.dma_start(rawt[:], raws[:])
            for n0, nl in nchunks(EXTR3, 512):
                ps = psp.tile([8, 512], F32, tag="cv3", name=f"d3_{n0}",
                              bufs=2)
                for t, (dy, dx) in enumerate(TAPS):
                    off = GD + n0 + (1 + dy) * WPC + dx
                    nc.tensor.matmul(ps[:, :nl],
                                     lhsT=w3t[:].rearrange(
                                         "q (t c) -> q t c", t=9)[:, t, :],
                                     rhs=r2[:, off:off + nl],
                                     start=(t == 0), stop=(t == 8))
                nc.vector.tensor_tensor(out=r3[:, n0:n0 + nl],
                                        in0=ps[:, :nl],
                                        in1=rawt[:, n0:n0 + nl], op=ALU.add)
                nc.vector.tensor_scalar(out=r3[:, n0:n0 + nl],
                                        in0=r3[:, n0:n0 + nl], scalar1=0.0,
                                        scalar2=1.0, op0=ALU.max,
                                        op1=ALU.min)
            nc.sync.dma_start(
                fin[:, :], mkap_s(r3[:], 1, [[EXTR3, 8], [WPC, 16],
                                             [1, 256]]))
    return nc


def build_C_old():
    nc = bass.Bass(trn_type="TRN2", name="kernC")
    u = nc.dram_tensor("u", (C, 38 * WP), BF16, kind="ExternalInput")
    raw32 = nc.dram_tensor("raw32", (C, ROWS * W), F32, kind="ExternalInput")
    cw1 = nc.dram_tensor("cw1", (C, 9 * 64), BF16, kind="ExternalInput")
    cb1 = nc.dram_tensor("cb1", (64, 1), F32, kind="ExternalInput")
    cw2 = nc.dram_tensor("cw2", (64, 9 * 64), BF16, kind="ExternalInput")
    cb2 = nc.dram_tensor("cb2", (64, 1), F32, kind="ExternalInput")
    cw3 = nc.dram_tensor("cw3", (64, 9 * 4), BF16, kind="ExternalInput")
    cb3 = nc.dram_tensor("cb3", (4, 1), F32, kind="ExternalInput")
    mr36 = nc.dram_tensor("mr36", (64, 36), F32, kind="ExternalInput")
    mr34 = nc.dram_tensor("mr34", (64, 34), F32, kind="ExternalInput")
    fin = nc.dram_tensor("fin", (C, ROWS, W), F32, kind="ExternalOutput")

    N36, N34, N32 = 36 * WP, 34 * WP, 32 * WP

    def conv_taps_outer(pool_ps, lhsw, rhsrc, dstact, bias, Ntot, Kp, Mp, relu,
                        group=1):
        """taps-outer grouped conv: lhsw(t)->lhsT AP, rhsrc(t, n0, nl)->rhs AP,
        dstact(n0, nl, psum) consumes."""
        chunks = nchunks(Ntot, 512)
        for g0 in range(0, len(chunks), group):
            grp = chunks[g0:g0 + group]
            pss = [pool_ps.tile([Mp, 512], F32, tag=f"cg{j}", name=f"cg_{g0}_{j}",
                                bufs=6) for j in range(len(grp))]
            for t in range(9):
                for j, (n0, nl) in enumerate(grp):
                    nc.tensor.matmul(pss[j][:, :nl], lhsT=lhsw(t),
                                     rhs=rhsrc(t, n0, nl),
                                     start=(t == 0), stop=(t == 8))
            for j, (n0, nl) in enumerate(grp):
                dstact(n0, nl, pss[j])

    with tile.TileContext(nc) as tc:
        with tc.tile_pool(name="cst", bufs=1) as cst, \
             tc.tile_pool(name="gp", bufs=1) as gp, \
             tc.tile_pool(name="ps", bufs=1, space="PSUM") as psp:
            w1t = cst.tile([C, 9 * 64], BF16)
            nc.sync.dma_start(w1t[:], cw1[:])
            w2t = cst.tile([64, 9 * 64], BF16)
            nc.sync.dma_start(w2t[:], cw2[:])
            w3t = cst.tile([64, 9 * 4], BF16)
            nc.sync.dma_start(w3t[:], cw3[:])
            b1t = cst.tile([64, 1], F32)
            nc.sync.dma_start(b1t[:], cb1[:])
            b2t = cst.tile([64, 1], F32)
            nc.sync.dma_start(b2t[:], cb2[:])
            b3t = cst.tile([C, 1], F32)
            nc.sync.dma_start(b3t[:], cb3[:])
            m36t = cst.tile([64, 36], F32)
            nc.sync.dma_start(m36t[:], mr36[:])
            m34t = cst.tile([64, 34], F32)
            nc.sync.dma_start(m34t[:], mr34[:])

            ut = gp.tile([C, 1 + 38 * WP + 4], BF16)
            nc.sync.dma_start(ut[:, 1:1 + 38 * WP], u[:])
            r1 = gp.tile([64, 1 + N36 + 4], BF16)
            r2 = gp.tile([64, 1 + N34 + 4], BF16)

            conv_taps_outer(
                psp,
                lambda t: w1t[:, t * 64:(t + 1) * 64],
                lambda t, n0, nl: ut[:, 1 + n0 + (1 + TAPS[t][0]) * WP + TAPS[t][1]:
                                     1 + n0 + (1 + TAPS[t][0]) * WP + TAPS[t][1] + nl],
                lambda n0, nl, ps: nc.scalar.activation(
                    r1[:, 1 + n0:1 + n0 + nl], ps[:, :nl], AF.Relu, bias=b1t[:]),
                b1t, N36, 64, 64, True)
            r1v = r1[:, 1:1 + N36].rearrange("p (r q) -> p r q", r=36)
            nc.vector.memset(r1v[:, :, 0:1], 0.0)
            nc.vector.memset(r1v[:, :, 257:258], 0.0)
            for r in (0, 1, 34, 35):
                nc.vector.tensor_scalar_mul(r1v[:, r, :], r1v[:, r, :],
                                            m36t[:, r:r + 1])

            conv_taps_outer(
                psp,
                lambda t: w2t[:, t * 64:(t + 1) * 64],
                lambda t, n0, nl: r1[:, 1 + n0 + (1 + TAPS[t][0]) * WP + TAPS[t][1]:
                                     1 + n0 + (1 + TAPS[t][0]) * WP + TAPS[t][1] + nl],
                lambda n0, nl, ps: nc.scalar.activation(
                    r2[:, 1 + n0:1 + n0 + nl], ps[:, :nl], AF.Relu, bias=b2t[:]),
                b2t, N34, 64, 64, True)
            r2v = r2[:, 1:1 + N34].rearrange("p (r q) -> p r q", r=34)
            nc.vector.memset(r2v[:, :, 0:1], 0.0)
            nc.vector.memset(r2v[:, :, 257:258], 0.0)
            for r in (0, 33):
                nc.vector.tensor_scalar_mul(r2v[:, r, :], r2v[:, r, :],
                                            m34t[:, r:r + 1])

            with tc.tile_pool(name="fo", bufs=1) as fo:
                rawt = fo.tile([C, ROWS * W], F32)
                nc.sync.dma_start(rawt[:], raw32[:])
                r3 = fo.tile([C, N32], F32)
                conv_taps_outer(
                    psp,
                    lambda t: w3t[:, t * 4:(t + 1) * 4],
                    lambda t, n0, nl: r2[:, 1 + n0 + (1 + TAPS[t][0]) * WP + TAPS[t][1]:
                                         1 + n0 + (1 + TAPS[t][0]) * WP + TAPS[t][1] + nl],
                    lambda n0, nl, ps: nc.scalar.activation(
                        r3[:, n0:n0 + nl], ps[:, :nl], AF.Copy),
                    b3t, N32, 64, C, False)
                r3v = r3[:].rearrange("p (r q) -> p r q", r=32)[:, :, 1:257]
                rv = rawt[:].rearrange("p (r q) -> p r q", r=32)
                nc.vector.tensor_scalar(out=r3v, in0=r3v, scalar1=b3t[:],
                                        scalar2=None, op0=ALU.add)
                nc.vector.tensor_tensor(out=r3v, in0=r3v, in1=rv, op=ALU.add)
                nc.vector.tensor_scalar(out=r3v, in0=r3v, scalar1=0.0,
                                        scalar2=1.0, op0=ALU.max, op1=ALU.min)
                nc.sync.dma_start(fin[:, :, :], r3v)
    return nc


_CACHE = {}


def _f8(x):
    return np.asarray(x, dtype=np.float32).astype(ml_dtypes.float8_e4m3)


def _prep_A(raw, feat, pw1, pb1, pw2, pb2, pw3, pb3):
    # weights packed for DoubleRow passes (see PAIRS)
    def tap_w(pw, dydx):
        dy, dx = dydx
        return pw[:, :, dy + 1, dx + 1]  # [co, ci]

    # w1: [ci, m, p, kt, co128]
    w1h = np.zeros((128, 2, 5, 2, 128), np.float32)
    for m in range(2):
        for p in range(5):
            t0, t1 = pair_taps(p)
            w1h[:, m, p, 0, :] = tap_w(pw1, t0).T[:, m * 128:(m + 1) * 128]
            if t1 is not None:
                w1h[:, m, p, 1, :] = tap_w(pw1, t1).T[:, m * 128:(m + 1) * 128]
    # w2: [cip, t, kc, co]
    w2h = np.zeros((128, 9, 2, 128), np.float32)
    for t, (dy, dx) in enumerate(TAPS):
        wt = tap_w(pw2, (dy, dx))  # [128 co, 256 ci]
        for kc in range(2):
            w2h[:, t, kc, :] = wt[:, kc * 128:(kc + 1) * 128].T
    # w3: [ci, p, kt, 912] (col = c*228 + tpsf)
    w3h = np.zeros((128, 5, 2, 912), np.float32)
    for p in range(5):
        t0, t1 = pair_taps(p)
        for kt, tt in ((0, t0), (1, t1)):
            if tt is None:
                continue
            wt = tap_w(pw3, tt)  # [900, 128]
            for c in range(C):
                w3h[:, p, kt, c * 228:c * 228 + 225] = \
                    wt[c * 225:(c + 1) * 225].T
    b1h = np.ascontiguousarray(pb1.reshape(2, 128).T).astype(np.float32)
    b2h = pb2.reshape(128, 1).astype(np.float32)
    b3row = np.full((912,), -30.0, np.float32)
    for c in range(C):
        b3row[c * 228:c * 228 + 225] = pb3[c * 225:(c + 1) * 225]
    w3h[0, 4, 1, :] = b3row

    xpad = np.pad(raw, ((0, 0), (PAD, PAD), (PAD, PAD)), mode="reflect")
    # unfolded patches [4, 256, 256, 15, 15]
    sw = np.lib.stride_tricks.sliding_window_view(xpad, (15, 15),
                                                  axis=(1, 2))
    featp = np.pad(feat, ((0, 0), (3, 3), (0, 0)))

    ins = []
    for i in range(N_CORES):
        r0 = i * ROWS
        m36 = np.array([1.0 if 0 <= r0 - 2 + r < H else 0.0
                        for r in range(36)], np.float32)
        m34 = np.array([1.0 if 0 <= r0 - 1 + r < H else 0.0
                        for r in range(34)], np.float32)
        fbA = np.zeros((128, 38, RP), np.float32)
        fbA[:, :, 1:257] = featp[:, r0:r0 + 38, :]
        fbA = fbA.reshape(128, EXTF)
        fbh = np.zeros((128, GD + 2 * EXTF), np.float32)
        fbh[:, GD:GD + EXTF] = fbA
        fbh[:, GD + EXTF:GD + 2 * EXTF - 1] = fbA[:, 1:]
        # Xu: [8192 pix, 912] = (r, x) -> [c*228 + tpsf]; bias comes via
        # the psum ones-matmul, so patches stay unscaled
        slab = sw[:, r0:r0 + ROWS, :, :, :]  # [4, 32, 256, 15, 15]
        xuh = np.zeros((ROWS * W, 4, 228), np.float32)
        xuh[:, :, :225] = slab.reshape(4, ROWS * W, 225).transpose(1, 0, 2)
        xuh = xuh.reshape(ROWS * W, 912)
        ins.append({
            "fb": _f8(fbh),
            "w1": _f8(w1h.reshape(128, -1)), "b1": b1h,
            "w2": _f8(w2h.reshape(128, -1)), "b2": b2h,
            "w3": _f8(w3h.reshape(128, -1)),
            "xu": _bf(xuh),
            "m36": np.ascontiguousarray(np.broadcast_to(m36, (128, 36))),
            "m34": np.ascontiguousarray(np.broadcast_to(m34, (128, 34))),
        })
    return ins


def run_A(raw, feat, pw1, pb1, pw2, pb2, pw3, pb3, trace=False):
    if "A" not in _CACHE:
        _CACHE["A"] = patch_nc(build_A())
    ins = _prep_A(raw, feat, pw1, pb1, pw2, pb2, pw3, pb3)
    res = run_bass_kernel_spmd(_CACHE["A"], ins, core_ids=list(range(N_CORES)),
                               trace=trace)
    corr = np.concatenate(
        [res.results[i]["corr"].reshape(ROWS, 2, 4, 128)
         .transpose(2, 0, 1, 3).reshape(C, ROWS, W)
         for i in range(N_CORES)], axis=1)
    return corr, res


def _dft_mats():
    k = np.arange(129)
    w = np.arange(256)
    th = 2 * np.pi * np.outer(w, k) / 256.0          # [256, 129]
    fwre = _bf(np.cos(th) / 16.0)
    fwim = _bf(-np.sin(th) / 16.0)
    h = np.arange(256)
    k1 = np.arange(256)
    th2 = 2 * np.pi * np.outer(h, k1) / 256.0        # [256h, 256k1]
    fhre = _bf(np.cos(th2) / 16.0)
    fhim = _bf(-np.sin(th2) / 16.0)
    fhimn = _bf(np.sin(th2) / 16.0)
    ck = np.where((k == 0) | (k == 128), 1.0, 2.0)
    th3 = 2 * np.pi * np.outer(k, w) / 256.0         # [129k, 256w]
    iwre = _bf(ck[:, None] * np.cos(th3) / 16.0)
    iwim = _bf(ck[:, None] * np.sin(th3) / 16.0)
    iwimn = _bf(-ck[:, None] * np.sin(th3) / 16.0)
    return fwre, fwim, fhre, fhim, fhimn, iwre, iwim, iwimn


def run_B1(corr1, trace=False):
    if "B1" not in _CACHE:
        _CACHE["B1"] = patch_nc(build_B1())
    fwre, fwim, fhre, fhim, fhimn, _, _, _ = _dft_mats()
    corrT = _bf(np.ascontiguousarray(corr1.transpose(0, 2, 1)))
    inm = {"corrT": corrT, "fwre": fwre, "fwim": fwim,
           "fhre": fhre, "fhim": fhim, "fhimn": fhimn}
    res = run_bass_kernel_spmd(_CACHE["B1"], [inm] * N_CORES,
                               core_ids=list(range(N_CORES)), trace=trace)
    return res.results[0]["fri"], res


def run_B2(fri_full, fw1, fb1, fw2, fb2, fw3, fb3, trace=False):
    from einops import rearrange as rr
    if "B2" not in _CACHE:
        _CACHE["B2"] = patch_nc(build_B2())
    gw1 = _bf(rr(fw1, "co ci dy dx -> ci (dy dx co)"))
    gw2 = _bf(rr(fw2, "co ci dy dx -> ci (dy dx co)"))
    gw3 = _bf(rr(fw3, "co ci dy dx -> ci (dy dx co)"))
    gb1 = fb1.reshape(64, 1).astype(np.float32)
    gb2 = fb2.reshape(64, 1).astype(np.float32)
    gb3 = fb3.reshape(8, 1).astype(np.float32)
    ins = []
    for i in range(N_CORES):
        r0 = i * ROWS
        slab = np.zeros((8, 38, WF), np.float32)
        lo, hi = max(0, r0 - 3), min(256, r0 + 35)
        slab[:, lo - (r0 - 3):hi - (r0 - 3), 1:130] = fri_full[:, lo:hi, :]
        m36 = np.array([1.0 if 0 <= r0 - 2 + r < 256 else 0.0
                        for r in range(36)], np.float32)
        m34 = np.array([1.0 if 0 <= r0 - 1 + r < 256 else 0.0
                        for r in range(34)], np.float32)
        ins.append({
            "fri": _bf(slab.reshape(8, 38 * WF)),
            "gw1": gw1, "gb1": gb1, "gw2": gw2, "gb2": gb2,
            "gw3": gw3, "gb3": gb3,
            "mf36": np.ascontiguousarray(np.broadcast_to(m36, (64, 36))),
            "mf34": np.ascontiguousarray(np.broadcast_to(m34, (64, 34))),
        })
    res = run_bass_kernel_spmd(_CACHE["B2"], ins, core_ids=list(range(N_CORES)),
                               trace=trace)
    cf = np.concatenate([res.results[i]["cfo"].reshape(8, 32, 129)
                         for i in range(N_CORES)], axis=1)
    return cf, res


def run_C(corr1, cf, raw, cw1, cb1, cw2, cb2, cw3, cb3, trace=False):
    if "C" not in _CACHE:
        _CACHE["C"] = patch_nc(build_C())
    # block-diag weights, layouts: in p=c*2+h, hid p=c*32+u*2+h, out p=c*2+h
    w1h = np.zeros((8, 9, 128), np.float32)
    w2h = np.zeros((128, 9, 128), np.float32)
    w3h = np.zeros((128, 9, 8), np.float32)
    b1h = np.zeros((128, 1), np.float32)
    b2h = np.zeros((128, 1), np.float32)
    b3h = np.zeros((8, 1), np.float32)
    for c in range(C):
        for h in range(2):
            b3h[c * 2 + h, 0] = cb3[c, 0]
            for uu in range(16):
                b1h[c * 32 + uu * 2 + h, 0] = cb1[c, uu]
                b2h[c * 32 + uu * 2 + h, 0] = cb2[c, uu]
    for t, (dy, dx) in enumerate([(a, b) for a in range(3) for b in range(3)]):
        for c in range(C):
            for h in range(2):
                for uu in range(16):
                    w1h[c * 2 + h, t, c * 32 + uu * 2 + h] = \
                        cw1[c, uu, 0, dy, dx]
                    w3h[c * 32 + uu * 2 + h, t, c * 2 + h] = \
                        cw3[c, 0, uu, dy, dx]
                    for v in range(16):
                        w2h[c * 32 + v * 2 + h, t, c * 32 + uu * 2 + h] = \
                            cw2[c, uu, v, dy, dx]
    # inverse DFT constants (same for all cores except ihs)
    kk = np.arange(129)
    w_ = np.arange(256)
    ck = np.where((kk == 0) | (kk == 128), 1.0, 2.0)
    th3 = 2 * np.pi * np.outer(kk, w_) / 256.0
    iwre = ck[:, None] * np.cos(th3) / 16.0
    iwim = ck[:, None] * np.sin(th3) / 16.0
    iwh = np.zeros((128, 2 * 512), np.float32)
    iwbh = np.zeros((1, 2 * 512), np.float32)
    for j, m in enumerate((iwre, iwim, -iwim, iwre)):
        iwh[:, j * 256:(j + 1) * 256] = m[:128]
        iwbh[0, j * 256:(j + 1) * 256] = m[128]
    # cfa [128 k, (c, ri, kc, 128 k1)], cfb k=128 row
    cfah = np.zeros((128, 4, 2, 2, 128), np.float32)
    cfbh = np.zeros((1, 4, 2, 2, 128), np.float32)
    for c in range(C):
        for ri in range(2):
            m = cf[ri * 4 + c]  # [256 k1, 129 k]
            for kc in range(2):
                cfah[:, c, ri, kc, :] = m[kc * 128:(kc + 1) * 128, :128].T
                cfbh[0, c, ri, kc, :] = m[kc * 128:(kc + 1) * 128, 128]
    ins = []
    for i in range(N_CORES):
        r0 = i * ROWS
        u0h = np.zeros((8, 22, WPC), np.float32)
        rawh = np.zeros((8, 16, WPC), np.float32)
        ihsh = np.zeros((128, 2, 2, 38), np.float32)
        hh = np.arange(r0 - 3, r0 + 35)
        ok = (hh >= 0) & (hh < 256)
        for kc in range(2):
            k1 = np.arange(kc * 128, kc * 128 + 128)
            th = 2 * np.pi * np.outer(k1, hh) / 256.0
            ihsh[:, kc, 0, :] = np.cos(th) / 16.0 * ok[None, :]
            ihsh[:, kc, 1, :] = -np.sin(th) / 16.0 * ok[None, :]
        for c in range(C):
            for h in range(2):
                lo = r0 + h * 16 - 3
                a, b = max(0, lo), min(256, lo + 22)
                u0h[c * 2 + h, a - lo:b - lo, 1:257] = corr1[c, a:b, :]
                rawh[c * 2 + h, :, 1:257] = \
                    raw[c, r0 + h * 16:r0 + h * 16 + 16, :] + cb3[c, 0]
        m20 = np.zeros((128, 20), np.float32)
        m18 = np.zeros((128, 18), np.float32)
        for p in range(128):
            h = p % 2
            base = r0 + h * 16
            for r in range(20):
                m20[p, r] = 1.0 if 0 <= base - 2 + r < 256 else 0.0
            for r in range(18):
                m18[p, r] = 1.0 if 0 <= base - 1 + r < 256 else 0.0
        ins.append({
            "u0": _bf(u0h.reshape(8, EXTC)),
            "raws": rawh.reshape(8, EXTR3).astype(np.float32),
            "cfa": _bf(cfah.reshape(128, -1)),
            "cfb": _bf(cfbh.reshape(1, -1)),
            "iwm": _bf(iwh), "iwbm": _bf(iwbh),
            "ihs": _bf(ihsh.reshape(128, -1)),
            "cw1": _bf(w1h.reshape(8, -1)), "cb1": b1h,
            "cw2": _bf(w2h.reshape(128, -1)), "cb2": b2h,
            "cw3": _bf(w3h.reshape(128, -1)), "cb3": b3h,
            "mr20": m20, "mr18": m18,
        })
    res = run_bass_kernel_spmd(_CACHE["C"], ins, core_ids=list(range(N_CORES)),
                               trace=trace)
    fin = np.zeros((C, H, W), np.float32)
    for i in range(N_CORES):
        f = res.results[i]["fin"].reshape(8, 16, 256)
        for c in range(C):
            for h in range(2):
                fin[c, i * ROWS + h * 16:i * ROWS + h * 16 + 16, :] = \
                    f[c * 2 + h]
    return fin, res


def kernel(**inputs):
    inputs = {k: np.asarray(v, dtype=np.float32) for k, v in inputs.items()}
    raw = inputs["raw_image"][0]
    feat = inputs["aberration_features"][0]
    corr1, _ = run_A(raw, feat,
                     inputs["pw1"], inputs["pb1"], inputs["pw2"], inputs["pb2"],
                     inputs["pw3"], inputs["pb3"])
    _, cf, _ = run_B(corr1, inputs["fw1"], inputs["fb1"], inputs["fw2"],
                     inputs["fb2"], inputs["fw3"], inputs["fb3"])
    fin, _ = run_C(corr1, cf, raw, inputs["cw1"], inputs["cb1"],
                   inputs["cw2"], inputs["cb2"], inputs["cw3"],
                   inputs["cb3"])
    return fin[None].astype(np.float32)




# revision 41
# speedup vs baseline: 1.2000x; 1.0882x over previous
"""Trainium2 Bass kernel for nn_AberrationCorrectionModule.

Reference pipeline:
  1. psf_predictor: 3x conv3x3 (128->256->128->900) on aberration_features,
     softmax over 225 taps per channel -> psf
  2. deconv: 15x15 spatially-varying weighted sum over reflect-padded raw
  3. freq corrector: rfft2 -> conv3x3 stack (8->64->64->8) -> irfft2, added
  4. per-channel refinement: 4 independent 1->16->16->1 conv stacks
  5. out = clip(raw + corrected, 0, 1)

Distribution: 8 NeuronCores, H-sharded (32 rows/core), SPMD dispatches with
host gather between (FFT stage needs full-image mixing).
"""
import json
import sys

sys.path.insert(0, "/opt/trn_rl_repo")

import ml_dtypes
import numpy as np

import bass_rust
import concourse.bass as bass
import concourse.tile as tile
from concourse import mybir
from concourse.bass_utils import run_bass_kernel_spmd

F32 = mybir.dt.float32
BF16 = mybir.dt.bfloat16
AF = mybir.ActivationFunctionType
ALU = mybir.AluOpType
AX = mybir.AxisListType

N_CORES = 8
C, H, W = 4, 256, 256
ROWS = H // N_CORES  # 32
KK = 15
PAD = KK // 2  # 7
WP = W + 2  # 258
TAPS = [(dy, dx) for dy in (-1, 0, 1) for dx in (-1, 0, 1)]


def _bf(x):
    return np.asarray(x, dtype=ml_dtypes.bfloat16)


def mkap(base_ap, offset, pairs):
    a = base_ap.copy()
    a.offset = offset
    a.ap = bass_rust.VecI64Pair([list(p) for p in pairs])
    return a


def _split_multiwaits(raw: bytes) -> bytes:
    """Workaround: this walrus build rejects >1 sync wait per instruction.
    Move extra waits onto NoOp carriers inserted just before the instruction."""
    m = json.loads(raw)
    ctr = 0
    for fn in m["functions"]:
        for bb in fn.get("blocks", []):
            insts = bb.get("instructions")
            if not insts:
                continue
            out = []
            for inst in insts:
                si = inst.get("sync_info")
                ow = (si or {}).get("on_wait") or []
                if len(ow) > 1:
                    for w in ow[:-1]:
                        out.append({
                            "debug": inst.get("debug", 0),
                            "engine": inst["engine"],
                            "ins": [], "outs": [],
                            "name": f"wsplit_{ctr}",
                            "opcode": "NoOp",
                            "sync_info": {"on_update": [], "on_wait": [w]},
                        })
                        ctr += 1
                    si["on_wait"] = [ow[-1]]
                out.append(inst)
            bb["instructions"] = out
    return json.dumps(m).encode()


def patch_nc(nc):
    orig = nc.to_json_bytes
    nc.to_json_bytes = lambda: _split_multiwaits(orig())
    return nc


def nchunks(total, step):
    out, o = [], 0
    while o < total:
        out.append((o, min(step, total - o)))
        o += step
    return out


# ================================================================ kernel A
# fp8 DoubleRow rewrite.
# conv1/conv2 feature-major on a 272-pitch grid (row pitch % 16 == 0 for
# DoubleRow lhsT k-tile strides). conv3 transposed: pixels on partitions,
# psf taps on the free axis (4ch x 228, 912 cols), softmax tail on
# vector/scalar engines. Patches pre-unfolded on host to [8192, 912].

RP = 272            # row pitch
EXTF = 38 * RP      # fb copy extent
EXTH1 = 36 * RP     # h1 half extent
EXTH2 = 34 * RP     # h2 copy extent
GD = 16             # leading guard cols
F8 = mybir.dt.float8e4
DR = mybir.MatmulPerfMode.DoubleRow
# conv tap pairs: 3 horizontal A/B-copy pairs, 1 vertical, 1 zero-padded
# (dy, dx) of kt0; kind 'AB' = kt1 from shifted copy (stride EXT),
# 'V' = kt1 one row down (stride RP), 'Z' = kt1 zero weights (stride RP)
PAIRS = [((-1, -1), 'AB'), ((0, -1), 'AB'), ((1, -1), 'AB'),
         ((-1, 1), 'V'), ((1, 1), 'Z')]


def pair_taps(p):
    """taps (as (dy,dx)) covered by pair p: (kt0, kt1 or None)."""
    (dy, dx), kind = PAIRS[p]
    if kind == 'AB':
        return (dy, dx), (dy, dx + 1)
    if kind == 'V':
        return (dy, dx), (dy + 1, dx)
    return (dy, dx), None


def build_A():
    nc = bass.Bass(trn_type="TRN2", name="kernA")
    fb = nc.dram_tensor("fb", (128, GD + 2 * EXTF), F8, kind="ExternalInput")
    w1 = nc.dram_tensor("w1", (128, 2 * 5 * 2 * 128), F8, kind="ExternalInput")
    b1 = nc.dram_tensor("b1", (128, 2), F32, kind="ExternalInput")
    w2 = nc.dram_tensor("w2", (128, 9 * 2 * 128), F8, kind="ExternalInput")
    b2 = nc.dram_tensor("b2", (128, 1), F32, kind="ExternalInput")
    w3 = nc.dram_tensor("w3", (128, 5 * 2 * 912), F8, kind="ExternalInput")
    xu = nc.dram_tensor("xu", (8192, 912), BF16, kind="ExternalInput")
    m36 = nc.dram_tensor("m36", (128, 36), F32, kind="ExternalInput")
    m34 = nc.dram_tensor("m34", (128, 34), F32, kind="ExternalInput")
    corr = nc.dram_tensor("corr", (64, 512), F32, kind="ExternalOutput")

    def win(tile_ap, off, stride, nl):
        return mkap_s(tile_ap, off, [[tile_ap.ap[0][0], 128], [stride, 2],
                                     [1, nl]])

    with tile.TileContext(nc) as tc:
        with tc.tile_pool(name="cst", bufs=1) as cst, \
             tc.tile_pool(name="hp", bufs=1) as hp, \
             tc.tile_pool(name="psum", bufs=2, space="PSUM") as psp:
            w3t = cst.tile([128, 5 * 2 * 912], F8)
            nc.sync.dma_start(w3t[:], w3[:])

            w2t = cst.tile([128, 9 * 2 * 128], F8)
            nc.sync.dma_start(w2t[:], w2[:])
            b2t = cst.tile([128, 1], F32)
            nc.sync.dma_start(b2t[:], b2[:])
            m34t = cst.tile([128, 34], F32)
            nc.sync.dma_start(m34t[:], m34[:])

            h2 = hp.tile([128, GD + 2 * EXTH2 + 144], F8)
            dum = cst.tile([128, 512], BF16)
            nc.vector.memset(dum[:], 0.0)
            for wi in range(8):
                pw = psp.tile([128, 512], F32, tag="cv", name=f"warm{wi}",
                              bufs=2)
                nc.tensor.matmul(pw[:], lhsT=dum[:, 0:128], rhs=dum[:],
                                 start=True, stop=True)

            with tc.tile_pool(name="h1p", bufs=1) as h1p:
                h1 = h1p.tile([128, GD + 2 * EXTH1 + GD], F8)
                with tc.tile_pool(name="fp", bufs=1) as fp:
                    w1t = fp.tile([128, 2 * 5 * 2 * 128], F8)
                    nc.sync.dma_start(w1t[:], w1[:])
                    b1t = fp.tile([128, 2], F32)
                    nc.sync.dma_start(b1t[:], b1[:])
                    m36t = fp.tile([128, 36], F32)
                    nc.sync.dma_start(m36t[:], m36[:])
                    fbt = fp.tile([128, GD + 2 * EXTF], F8)
                    FB1 = GD + 13 * RP
                    FB2 = GD + 26 * RP
                    nc.sync.dma_start(fbt[:, :FB1], fb[:, :FB1])
                    nc.sync.dma_start(fbt[:, FB1:FB2], fb[:, FB1:FB2])
                    nc.sync.dma_start(fbt[:, FB2:], fb[:, FB2:])
                    w1v = w1t[:].rearrange("q (m p k c) -> q m p k c", m=2,
                                           p=5, k=2)

                    # conv1: 128 -> 256 (2 M halves), 5 DoubleRow passes
                    for m in range(2):
                        for n0, nl in nchunks(EXTH1, 512):
                            ps = psp.tile([128, 512], F32, tag="cv",
                                          name=f"c1_{m}_{n0}", bufs=2)
                            for p, ((dy, dx), kind) in enumerate(PAIRS):
                                off = GD + n0 + (1 + dy) * RP + dx
                                st = EXTF if kind == 'AB' else RP
                                nc.tensor.matmul(
                                    ps[:, :nl], lhsT=w1v[:, m, p, :, :],
                                    rhs=win(fbt[:], off, st, nl),
                                    start=(p == 0), stop=(p == 4),
                                    perf_mode=DR)
                            nc.scalar.activation(
                                h1[:, GD + m * EXTH1 + n0:
                                   GD + m * EXTH1 + n0 + nl],
                                ps[:, :nl], AF.Relu, bias=b1t[:, m:m + 1])
                    for m in range(2):
                        h3 = h1[:, GD + m * EXTH1:GD + (m + 1) * EXTH1] \
                            .rearrange("q (r c) -> q r c", r=36)
                        nc.vector.memset(h3[:, :, 0:1], 0.0)
                        nc.vector.memset(h3[:, :, 257:258], 0.0)
                        for r in (0, 1, 34, 35):
                            nc.vector.tensor_scalar_mul(
                                h3[:, r, :], h3[:, r, :], m36t[:, r:r + 1])

                # conv2: 256 -> 128, 9 DoubleRow passes over kc halves
                w2v = w2t[:].rearrange("q (t k c) -> q t k c", t=9, k=2)
                for n0, nl in nchunks(EXTH2, 512):
                    ps = psp.tile([128, 512], F32, tag="cv",
                                  name=f"c2_{n0}", bufs=2)
                    for t, (dy, dx) in enumerate(TAPS):
                        off = GD + n0 + (1 + dy) * RP + dx
                        nc.tensor.matmul(
                            ps[:, :nl], lhsT=w2v[:, t, :, :],
                            rhs=win(h1[:], off, EXTH1, nl),
                            start=(t == 0), stop=(t == 8), perf_mode=DR)
                    nc.scalar.activation(
                        h2[:, GD + n0:GD + n0 + nl], ps[:, :nl], AF.Relu,
                        bias=b2t[:])
                h23 = h2[:, GD:GD + EXTH2].rearrange("q (r c) -> q r c", r=34)
                nc.vector.memset(h23[:, :, 0:1], 0.0)
                nc.vector.memset(h23[:, :, 257:258], 0.0)
                for r in (0, 33):
                    nc.vector.tensor_scalar_mul(
                        h23[:, r, :], h23[:, r, :], m34t[:, r:r + 1])

            # shifted copy for conv3 lhsT k-tile pairing (copy1[x]=copy0[x+1])
            BND = 9 * RP
            for bb in range(4):
                a0 = bb * BND
                a1 = min(EXTH2 - 1, a0 + BND)
                nc.sync.dma_start(h2[:, GD + EXTH2 + a0:GD + EXTH2 + a1],
                                  h2[:, GD + 1 + a0:GD + 1 + a1])
            # ones region for the bias k-tile of conv3 pass 4
            OB = GD + 2 * EXTH2 + 2
            nc.vector.memset(h2[:, OB:OB + 128], 1.0)

            # conv3 transposed + softmax tail, per 128-pixel group.
            # bias lands in psum via a K=1 ones-matmul; exp(b3) is folded
            # into xu on host; D comes free from exp accum_out. Division
            # and output DMA are batched over 8 groups.
            w3v = w3t[:].rearrange("q (p k c) -> q p k c", p=5, k=2)
            GB = 8
            with tc.tile_pool(name="gp", bufs=4) as gp, \
                 tc.tile_pool(name="bp", bufs=2) as bp:
                for g in range(64):
                    r, cc = g // 2, g % 2
                    gi = g % GB
                    if gi == 0:
                        Ns = bp.tile([128, GB * 4], F32, tag="Ns",
                                     name=f"Ns{g}")
                        Ds = bp.tile([128, GB * 4], F32, tag="Ds",
                                     name=f"Ds{g}")
                    if g % 2 == 0:
                        Xg2 = gp.tile([128, 1824], BF16, tag="Xg",
                                      name=f"Xg{g}")
                        E2 = gp.tile([128, 1824], BF16, tag="E",
                                     name=f"E{g}")
                    xq = nc.sync if g % 2 == 0 else nc.gpsimd
                    xq.dma_start(Xg2[:, (g % 2) * 912:(g % 2) * 912 + 912],
                                 xu[g * 128:(g + 1) * 128, :])
                    pss = [psp.tile([128, 456], F32, tag=f"c3{j}",
                                    name=f"c3_{g}_{j}", bufs=3)
                           for j in range(2)]
                    for p, ((dy, dx), kind) in enumerate(PAIRS):
                        off = GD + (r + 1 + dy) * RP + cc * 128 + 1 + dx
                        if kind == 'AB':
                            st = EXTH2
                        elif kind == 'V':
                            st = RP
                        else:  # Z: kt1 = ones region (bias via w3 row 0)
                            st = OB - off
                        for j in range(2):
                            nc.tensor.matmul(
                                pss[j][:],
                                lhsT=win(h2[:], off, st, 128),
                                rhs=w3v[:, p, :, j * 456:(j + 1) * 456],
                                start=(p == 0), stop=(p == 4), perf_mode=DR)
                    eo = (g % 2) * 912
                    for c in range(4):
                        nc.scalar.activation(
                            E2[:, eo + c * 228:eo + (c + 1) * 228],
                            pss[c // 2][:, (c % 2) * 228:(c % 2) * 228 + 228],
                            AF.Exp, accum_out=Ds[:, gi * 4 + c:gi * 4 + c + 1])
                    if g % 2 == 1:
                        Pt = gp.tile([128, 1824], BF16, tag="Pt",
                                     name=f"Pt{g}")
                        nc.vector.tensor_tensor(out=Pt[:], in0=E2[:],
                                                in1=Xg2[:], op=ALU.mult)
                        nc.vector.tensor_reduce(
                            Ns[:, (gi - 1) * 4:(gi + 1) * 4],
                            Pt[:].rearrange("q (a b) -> q a b", a=8),
                            AX.X, ALU.add)
                    if gi == GB - 1:
                        nc.vector.reciprocal(Ds[:], Ds[:])
                        nc.vector.tensor_tensor(out=Ns[:], in0=Ns[:],
                                                in1=Ds[:], op=ALU.mult)
                        nc.sync.dma_start(
                            mkap(corr[:], (g - GB + 1) * 512,
                                 [[1, 128], [512, GB], [128, 4]]), Ns[:])
    return nc


def mkap_s(base_ap, off, pairs):
    a = base_ap.copy()
    a.offset = base_ap.offset + off
    a.ap = bass_rust.VecI64Pair([list(p) for p in pairs])
    return a


def build_A_old():
    nc = bass.Bass(trn_type="TRN2", name="kernA")
    feat = nc.dram_tensor("feat", (128, 38 * 256), F32, kind="ExternalInput")
    raw46 = nc.dram_tensor("raw46", (C, 46, 270), BF16, kind="ExternalInput")
    w1 = nc.dram_tensor("w1", (128, 2 * 9 * 128), BF16, kind="ExternalInput")
    b1 = nc.dram_tensor("b1", (128, 2), F32, kind="ExternalInput")
    w2 = nc.dram_tensor("w2", (128, 2 * 9 * 128), BF16, kind="ExternalInput")
    b2 = nc.dram_tensor("b2", (128, 1), F32, kind="ExternalInput")
    w3 = nc.dram_tensor("w3", (128, 9 * 1024), BF16, kind="ExternalInput")
    b3 = nc.dram_tensor("b3", (128, 8), F32, kind="ExternalInput")
    m36 = nc.dram_tensor("m36", (128, 36), F32, kind="ExternalInput")
    m34 = nc.dram_tensor("m34", (128, 34), F32, kind="ExternalInput")
    corr = nc.dram_tensor("corr", (C, ROWS, W), F32, kind="ExternalOutput")

    NF36, NF34 = 36 * WP, 34 * WP

    with tile.TileContext(nc) as tc:
        with tc.tile_pool(name="cst", bufs=1) as cst, \
             tc.tile_pool(name="hp", bufs=1) as hp, \
             tc.tile_pool(name="psum", bufs=2, space="PSUM") as psp:
            w3t = cst.tile([128, 9 * 1024], BF16)
            nc.sync.dma_start(w3t[:], w3[:])
            b3t = cst.tile([128, 8], F32)
            nc.sync.dma_start(b3t[:], b3[:])
            b2t = cst.tile([128, 1], F32)
            nc.sync.dma_start(b2t[:], b2[:])
            m34t = cst.tile([128, 34], F32)
            nc.sync.dma_start(m34t[:], m34[:])
            ones = cst.tile([128, 1], BF16)
            nc.vector.memset(ones[:], 1.0)

            h2 = hp.tile([128, NF34], BF16)

            with tc.tile_pool(name="h1p", bufs=1) as h1p:
                h1 = [h1p.tile([128, NF36 + 8], BF16, name=f"h1_{m}", tag=f"h1_{m}") for m in range(2)]
                w2t = h1p.tile([128, 2 * 9 * 128], BF16)
                nc.sync.dma_start(w2t[:], w2[:])

                with tc.tile_pool(name="fp", bufs=1) as fp:
                    w1t = fp.tile([128, 2 * 9 * 128], BF16)
                    nc.sync.dma_start(w1t[:], w1[:])
                    b1t = fp.tile([128, 2], F32)
                    nc.sync.dma_start(b1t[:], b1[:])
                    m36t = fp.tile([128, 36], F32)
                    nc.sync.dma_start(m36t[:], m36[:])
                    ff = fp.tile([128, 38 * 256], F32)
                    nc.sync.dma_start(ff[:], feat[:])
                    fb = fp.tile([128, 38 * WP + 8], BF16)
                    nc.vector.memset(fb[:], 0.0)
                    nc.vector.tensor_copy(
                        fb[:, 1:1 + 38 * WP].rearrange(
                            "p (r c) -> p r c", r=38)[:, :, 1:257],
                        ff[:].rearrange("p (r c) -> p r c", r=38))

                    # conv1: 128 -> 256 (2 M chunks), taps-outer groups of 3
                    for m in range(2):
                        ch1 = nchunks(NF36, 512)
                        for g0 in range(0, len(ch1), 3):
                            grp = ch1[g0:g0 + 3]
                            pcs = [psp.tile([128, 512], F32, tag=f"pc{j}",
                                            name=f"c1_{m}_{g0}_{j}", bufs=1)
                                   for j in range(len(grp))]
                            for t, (dy, dx) in enumerate(TAPS):
                                base = (1 + dy) * WP + dx
                                for j, (n0, nl) in enumerate(grp):
                                    nc.tensor.matmul(
                                        pcs[j][:, :nl],
                                        lhsT=w1t[:, (m * 9 + t) * 128:(m * 9 + t + 1) * 128],
                                        rhs=fb[:, 1 + n0 + base:1 + n0 + base + nl],
                                        start=(t == 0), stop=(t == 8))
                            for j, (n0, nl) in enumerate(grp):
                                nc.scalar.activation(
                                    h1[m][:, 1 + n0:1 + n0 + nl], pcs[j][:, :nl],
                                    AF.Relu, bias=b1t[:, m:m + 1])
                        h3 = h1[m][:, 1:1 + NF36].rearrange("p (r c) -> p r c", r=36)
                        nc.vector.memset(h3[:, :, 0:1], 0.0)
                        nc.vector.memset(h3[:, :, 257:258], 0.0)
                        # zero out-of-image rows (only rows 0,1,34,35 can be OOI)
                        for r in (0, 1, 34, 35):
                            nc.vector.tensor_scalar_mul(
                                h3[:, r, :], h3[:, r, :], m36t[:, r:r + 1])

                # conv2: 256 -> 128 (2 K chunks), taps-outer groups of 3
                ch2 = nchunks(NF34, 512)
                for g0 in range(0, len(ch2), 3):
                    grp = ch2[g0:g0 + 3]
                    pcs = [psp.tile([128, 512], F32, tag=f"pc{j}",
                                    name=f"c2_{g0}_{j}", bufs=1)
                           for j in range(len(grp))]
                    ti = 0
                    for kc in range(2):
                        for t, (dy, dx) in enumerate(TAPS):
                            base = (1 + dy) * WP + dx
                            for j, (n0, nl) in enumerate(grp):
                                nc.tensor.matmul(
                                    pcs[j][:, :nl],
                                    lhsT=w2t[:, (kc * 9 + t) * 128:(kc * 9 + t + 1) * 128],
                                    rhs=h1[kc][:, 1 + n0 + base:1 + n0 + base + nl],
                                    start=(ti == 0), stop=(ti == 17))
                            ti += 1
                    for j, (n0, nl) in enumerate(grp):
                        nc.scalar.activation(
                            h2[:, n0:n0 + nl], pcs[j][:, :nl], AF.Relu, bias=b2t[:])
                h23 = h2[:].rearrange("p (r c) -> p r c", r=34)
                nc.vector.memset(h23[:, :, 0:1], 0.0)
                nc.vector.memset(h23[:, :, 257:258], 0.0)
                for r in (0, 33):
                    nc.vector.tensor_scalar_mul(
                        h23[:, r, :], h23[:, r, :], m34t[:, r:r + 1])

            # conv3 + softmax + deconv per (pixchunk, channel).
            # psf channels padded 900->1024: image channel c = M-chunks
            # {2c, 2c+1}; taps 0..224 real, 225..255 padded (bias -30).
            RPC = 8
            PCN = RPC * W  # 2048
            h2v = h2[:].rearrange("p (r q) -> p r q", r=34)
            with tc.tile_pool(name="ex", bufs=2) as exp_pool, \
                 tc.tile_pool(name="xp", bufs=2) as xpool, \
                 tc.tile_pool(name="scp", bufs=2) as scp, \
                 tc.tile_pool(name="dnp", bufs=2, space="DRAM") as dnp, \
                 tc.tile_pool(name="rbp", bufs=2) as rbp:
                for pc_i in range(ROWS // RPC):
                    r0 = pc_i * RPC
                    dnd = dnp.tile([C, 2 * PCN], F32, tag="dnd")
                    for c in range(C):
                        Ea = exp_pool.tile([128, PCN], BF16, tag="Ea")
                        Eb = exp_pool.tile([128, PCN], BF16, tag="Eb")
                        Pa = exp_pool.tile([128, PCN], BF16, tag="Pa")
                        Pb = exp_pool.tile([128, PCN], BF16, tag="Pb")
                        Xa = xpool.tile([128, PCN], BF16, tag="Xa")
                        Xb = xpool.tile([128, PCN], BF16, tag="Xb")
                        # patch strips: partition t = dy*15+dx, free = pixel
                        for dy in range(KK):
                            t0 = dy * KK
                            off = c * 46 * 270 + (r0 + dy) * 270
                            if t0 + KK <= 128:
                                nc.sync.dma_start(
                                    Xa[t0:t0 + KK, :],
                                    mkap(raw46[:], off, [[1, KK], [270, RPC], [1, W]]))
                            elif t0 >= 128:
                                nc.sync.dma_start(
                                    Xb[t0 - 128:t0 - 128 + KK, :],
                                    mkap(raw46[:], off, [[1, KK], [270, RPC], [1, W]]))
                            else:
                                n1 = 128 - t0
                                nc.sync.dma_start(
                                    Xa[t0:128, :],
                                    mkap(raw46[:], off, [[1, n1], [270, RPC], [1, W]]))
                                nc.sync.dma_start(
                                    Xb[0:KK - n1, :],
                                    mkap(raw46[:], off + n1,
                                         [[1, KK - n1], [270, RPC], [1, W]]))
                        # conv3 -> exp (bias fused into exp's activation)
                        for half, E in ((0, Ea), (1, Eb)):
                            mc = c * 2 + half
                            chunks = nchunks(PCN, 512)
                            pss = [psp.tile([128, 512], F32, tag=f"pc{j}",
                                            name=f"ps_{mc}_{j}", bufs=1)
                                   for j in range(len(chunks))]
                            for t, (dy, dx) in enumerate(TAPS):
                                for j, (s0, sl) in enumerate(chunks):
                                    rr = r0 + s0 // W + 1 + dy
                                    nc.tensor.matmul(
                                        pss[j][:, :sl],
                                        lhsT=w3t[:, t * 1024 + mc * 128:
                                                 t * 1024 + (mc + 1) * 128],
                                        rhs=h2v[:, rr:rr + 2, 1 + dx:257 + dx],
                                        start=(t == 0), stop=(t == 8))
                            for j, (s0, sl) in enumerate(chunks):
                                nc.scalar.activation(
                                    E[:, s0:s0 + sl], pss[j][:, :sl], AF.Exp,
                                    bias=b3t[:, mc:mc + 1])
                        # tap sums via ones-matmuls on PE (GPSIMD C-reduce
                        # is ~40us/op; PE does it in ~0.2us/chunk)
                        nc.vector.tensor_tensor(out=Pa[:, :], in0=Ea[:, :], in1=Xa[:, :], op=ALU.mult)
                        nc.vector.tensor_tensor(out=Pb[0:97, :], in0=Eb[0:97, :], in1=Xb[0:97, :], op=ALU.mult)
                        sc = scp.tile([1, 2 * PCN], F32, tag="sc")
                        da, na = sc[:, 0:PCN], sc[:, PCN:2 * PCN]
                        for s0, sl in nchunks(PCN, 512):
                            for dst, ta, tb in ((da, Ea, Eb), (na, Pa, Pb)):
                                pr = psp.tile([1, 512], F32, tag="pr", bufs=2)
                                nc.tensor.matmul(pr[:, :sl], lhsT=ones[:, :],
                                                 rhs=ta[:, s0:s0 + sl],
                                                 start=True, stop=False)
                                nc.tensor.matmul(pr[:, :sl], lhsT=ones[0:97, :],
                                                 rhs=tb[0:97, s0:s0 + sl],
                                                 start=False, stop=True)
                                nc.vector.tensor_copy(dst[:, s0:s0 + sl], pr[:, :sl])
                        nc.sync.dma_start(dnd[c, :], sc[:, :])
                    # reshape [1,2048]x2 per ch -> [128,64] so the divide
                    # runs on all 128 lanes instead of one
                    Dt = rbp.tile([128, 64], F32, tag="Dt")
                    Nt = rbp.tile([128, 64], F32, tag="Nt")
                    for c in range(C):
                        nc.sync.dma_start(
                            Dt[32 * c:32 * c + 32, :],
                            mkap(dnd[:], c * 2 * PCN, [[64, 32], [1, 64]]))
                        nc.sync.dma_start(
                            Nt[32 * c:32 * c + 32, :],
                            mkap(dnd[:], c * 2 * PCN + PCN, [[64, 32], [1, 64]]))
                    nc.vector.reciprocal(Dt[:], Dt[:])
                    nc.vector.tensor_tensor(out=Nt[:], in0=Nt[:], in1=Dt[:], op=ALU.mult)
                    nc.sync.dma_start(corr[:, r0:r0 + RPC, :], Nt[:])
    return nc




# ================================================================ kernel B1
# Forward rfft2 via DFT matmuls, replicated on every core; writes full fri.
# V[h,k] = sum_w x[h,w] Fw[w,k];  Y[k1,k] = sum_h Fh[k1,h] V[h,k]
# fri = [Yre(4ch), Yim(4ch)] as [8, 256, 129].

def build_B1():
    nc = bass.Bass(trn_type="TRN2", name="kernB1")
    corrT = nc.dram_tensor("corrT", (C, 256, 256), BF16, kind="ExternalInput")
    fwre = nc.dram_tensor("fwre", (256, 129), BF16, kind="ExternalInput")
    fwim = nc.dram_tensor("fwim", (256, 129), BF16, kind="ExternalInput")
    fhre = nc.dram_tensor("fhre", (256, 256), BF16, kind="ExternalInput")
    fhim = nc.dram_tensor("fhim", (256, 256), BF16, kind="ExternalInput")
    fhimn = nc.dram_tensor("fhimn", (256, 256), BF16, kind="ExternalInput")
    fri = nc.dram_tensor("fri", (8, 256, 129), F32, kind="ExternalOutput")

    with tile.TileContext(nc) as tc:
        with tc.tile_pool(name="cst", bufs=1) as cst, \
             tc.tile_pool(name="wk", bufs=2) as wk, \
             tc.tile_pool(name="ps", bufs=4, space="PSUM") as psp:
            fw = [cst.tile([128, 2 * 129], BF16, name=f"fw_{i}", tag=f"fw_{i}") for i in range(2)]
            for kc in range(2):
                nc.sync.dma_start(fw[kc][:, 0:129], fwre[kc * 128:(kc + 1) * 128, :])
                nc.sync.dma_start(fw[kc][:, 129:258], fwim[kc * 128:(kc + 1) * 128, :])
            fh = [cst.tile([128, 3 * 256], BF16, name=f"fh_{i}", tag=f"fh_{i}") for i in range(2)]
            for kc in range(2):
                nc.sync.dma_start(fh[kc][:, 0:256], fhre[kc * 128:(kc + 1) * 128, :])
                nc.sync.dma_start(fh[kc][:, 256:512], fhim[kc * 128:(kc + 1) * 128, :])
                nc.sync.dma_start(fh[kc][:, 512:768], fhimn[kc * 128:(kc + 1) * 128, :])
            for c in range(C):
                xT = [wk.tile([128, 256], BF16, name=f"xT{i}", tag=f"xT{i}") for i in range(2)]
                for kc in range(2):
                    nc.sync.dma_start(xT[kc][:], corrT[c, kc * 128:(kc + 1) * 128, :])
                V = [wk.tile([128, 2 * 129], BF16, name=f"V{i}", tag=f"V{i}") for i in range(2)]
                for mc in range(2):      # output h chunk
                    for ri in range(2):  # re / im
                        pv = psp.tile([128, 129], F32, tag="pv")
                        for kc in range(2):
                            nc.tensor.matmul(
                                pv[:, :],
                                lhsT=xT[kc][:, mc * 128:(mc + 1) * 128],
                                rhs=fw[kc][:, ri * 129:(ri + 1) * 129],
                                start=(kc == 0), stop=(kc == 1))
                        nc.vector.tensor_copy(V[mc][:, ri * 129:(ri + 1) * 129], pv[:, :])
                # Y: for re out: FhRe@Vre + FhImNeg@Vim ; im out: FhIm@Vre + FhRe@Vim
                for mc in range(2):      # k1 chunk
                    for ri in range(2):  # re / im output
                        py = psp.tile([128, 129], F32, tag="pv")
                        for kc in range(2):
                            if ri == 0:
                                t1, t2 = 0, 512   # re, imneg
                            else:
                                t1, t2 = 256, 0   # im, re
                            nc.tensor.matmul(
                                py[:, :],
                                lhsT=fh[kc][:, t1 + mc * 128:t1 + (mc + 1) * 128],
                                rhs=V[kc][:, 0:129],
                                start=(kc == 0), stop=False)
                            nc.tensor.matmul(
                                py[:, :],
                                lhsT=fh[kc][:, t2 + mc * 128:t2 + (mc + 1) * 128],
                                rhs=V[kc][:, 129:258],
                                start=False, stop=(kc == 1))
                        ys = wk.tile([128, 129], F32, tag="ys")
                        nc.scalar.activation(ys[:], py[:], AF.Copy)
                        nc.sync.dma_start(
                            fri[ri * 4 + c, mc * 128:(mc + 1) * 128, :], ys[:])
    return nc


# ================================================================ kernel B2
# freq conv stack on fri slab (38 rows, ch-major) + partial inverse fft.
WF = 131  # 129 + 2 pad cols

def build_B2():
    nc = bass.Bass(trn_type="TRN2", name="kernB2")
    fri = nc.dram_tensor("fri", (8, 38 * WF), BF16, kind="ExternalInput")
    gw1 = nc.dram_tensor("gw1", (8, 9 * 64), BF16, kind="ExternalInput")
    gb1 = nc.dram_tensor("gb1", (64, 1), F32, kind="ExternalInput")
    gw2 = nc.dram_tensor("gw2", (64, 9 * 64), BF16, kind="ExternalInput")
    gb2 = nc.dram_tensor("gb2", (64, 1), F32, kind="ExternalInput")
    gw3 = nc.dram_tensor("gw3", (64, 9 * 8), BF16, kind="ExternalInput")
    gb3 = nc.dram_tensor("gb3", (8, 1), F32, kind="ExternalInput")
    mf36 = nc.dram_tensor("mf36", (64, 36), F32, kind="ExternalInput")
    mf34 = nc.dram_tensor("mf34", (64, 34), F32, kind="ExternalInput")
    cfo = nc.dram_tensor("cfo", (8, 32 * 129), F32, kind="ExternalOutput")

    N36, N34, N32 = 36 * WF, 34 * WF, 32 * WF

    with tile.TileContext(nc) as tc:
        with tc.tile_pool(name="cst", bufs=1) as cst, \
             tc.tile_pool(name="gp", bufs=1) as gp, \
             tc.tile_pool(name="ps", bufs=4, space="PSUM") as psp:
            w1t = cst.tile([8, 9 * 64], BF16)
            nc.sync.dma_start(w1t[:], gw1[:])
            w2t = cst.tile([64, 9 * 64], BF16)
            nc.sync.dma_start(w2t[:], gw2[:])
            w3t = cst.tile([64, 9 * 8], BF16)
            nc.sync.dma_start(w3t[:], gw3[:])
            b1t = cst.tile([64, 1], F32)
            nc.sync.dma_start(b1t[:], gb1[:])
            b2t = cst.tile([64, 1], F32)
            nc.sync.dma_start(b2t[:], gb2[:])
            b3t = cst.tile([8, 1], F32)
            nc.sync.dma_start(b3t[:], gb3[:])
            m36t = cst.tile([64, 36], F32)
            nc.sync.dma_start(m36t[:], mf36[:])
            m34t = cst.tile([64, 34], F32)
            nc.sync.dma_start(m34t[:], mf34[:])

            ft = gp.tile([8, 1 + 38 * WF + 4], BF16)
            nc.sync.dma_start(ft[:, 1:1 + 38 * WF], fri[:, :])
            g1 = gp.tile([64, 1 + N36 + 4], BF16)
            g2 = gp.tile([64, 1 + N34 + 4], BF16)
            g3 = gp.tile([8, N32], F32)

            for n0, nl in nchunks(N36, 512):
                pc = psp.tile([64, 512], F32, tag="pg")
                for t, (dy, dx) in enumerate(TAPS):
                    base = (1 + dy) * WF + dx
                    nc.tensor.matmul(
                        pc[:, :nl],
                        lhsT=w1t[:, t * 64:(t + 1) * 64],
                        rhs=ft[:, 1 + n0 + base:1 + n0 + base + nl],
                        start=(t == 0), stop=(t == 8))
                nc.scalar.activation(g1[:, 1 + n0:1 + n0 + nl], pc[:, :nl],
                                     AF.Relu, bias=b1t[:])
            g1v = g1[:, 1:1 + N36].rearrange("p (r q) -> p r q", r=36)
            nc.vector.memset(g1v[:, :, 0:1], 0.0)
            nc.vector.memset(g1v[:, :, 130:131], 0.0)
            for r in (0, 1, 34, 35):
                nc.vector.tensor_scalar_mul(g1v[:, r, :], g1v[:, r, :],
                                            m36t[:, r:r + 1])
            for n0, nl in nchunks(N34, 512):
                pc = psp.tile([64, 512], F32, tag="pg")
                for t, (dy, dx) in enumerate(TAPS):
                    base = (1 + dy) * WF + dx
                    nc.tensor.matmul(
                        pc[:, :nl],
                        lhsT=w2t[:, t * 64:(t + 1) * 64],
                        rhs=g1[:, 1 + n0 + base:1 + n0 + base + nl],
                        start=(t == 0), stop=(t == 8))
                nc.scalar.activation(g2[:, 1 + n0:1 + n0 + nl], pc[:, :nl],
                                     AF.Relu, bias=b2t[:])
            g2v = g2[:, 1:1 + N34].rearrange("p (r q) -> p r q", r=34)
            nc.vector.memset(g2v[:, :, 0:1], 0.0)
            nc.vector.memset(g2v[:, :, 130:131], 0.0)
            for r in (0, 33):
                nc.vector.tensor_scalar_mul(g2v[:, r, :], g2v[:, r, :],
                                            m34t[:, r:r + 1])
            for n0, nl in nchunks(N32, 512):
                pc = psp.tile([8, 512], F32, tag="pg")
                for t, (dy, dx) in enumerate(TAPS):
                    base = (1 + dy) * WF + dx
                    nc.tensor.matmul(
                        pc[:, :nl],
                        lhsT=w3t[:, t * 8:(t + 1) * 8],
                        rhs=g2[:, 1 + n0 + base:1 + n0 + base + nl],
                        start=(t == 0), stop=(t == 8))
                nc.scalar.activation(g3[:, n0:n0 + nl], pc[:, :nl],
                                     AF.Copy, bias=0.0)
            # add bias gb3 separately (Copy cannot take AP bias)
            nc.vector.tensor_scalar(out=g3[:], in0=g3[:], scalar1=b3t[:],
                                    scalar2=None, op0=ALU.add)

            # write CF slab [8 (ri,c), 32 k1-rows, 129] (strip pad cols;
            # real bins live at cols 1..129 of the WF=131 grid)
            nc.sync.dma_start(
                cfo[:, :], mkap_s(g3[:], 1, [[N32, 8], [WF, 32], [1, 129]]))
    return nc


# ================================================================ kernel B
# merged forward DFT + freq convs, one dispatch. V (row FFT) needs all
# columns of the full image (replicated); Y (col FFT) computed only for
# this core's 38-row k1 slab; freq convs 2-half row-packed (bf16).
# partition layouts: ft/g3: p = (ri*4+c)*2 + h; g1/g2: p = u*2 + h.

def build_B():
    nc = bass.Bass(trn_type="TRN2", name="kernB")
    corrT = nc.dram_tensor("corrT", (C, 256, 256), BF16, kind="ExternalInput")
    fwre = nc.dram_tensor("fwre", (256, 129), BF16, kind="ExternalInput")
    fwim = nc.dram_tensor("fwim", (256, 129), BF16, kind="ExternalInput")
    fhs = nc.dram_tensor("fhs", (128, 2 * 3 * 38), BF16, kind="ExternalInput")
    gw1 = nc.dram_tensor("gw1", (16, 9 * 128), BF16, kind="ExternalInput")
    gb1 = nc.dram_tensor("gb1", (128, 1), F32, kind="ExternalInput")
    gw2 = nc.dram_tensor("gw2", (128, 9 * 128), BF16, kind="ExternalInput")
    gb2 = nc.dram_tensor("gb2", (128, 1), F32, kind="ExternalInput")
    gw3 = nc.dram_tensor("gw3", (128, 9 * 16), BF16, kind="ExternalInput")
    gb3 = nc.dram_tensor("gb3", (16, 1), F32, kind="ExternalInput")
    mf20 = nc.dram_tensor("mf20", (128, 20), F32, kind="ExternalInput")
    mf18 = nc.dram_tensor("mf18", (128, 18), F32, kind="ExternalInput")
    cfo = nc.dram_tensor("cfo", (8, 32 * 129), F32, kind="ExternalOutput")

    EXB = 22 * WF       # ft half extent (22 rows x 131)
    EXB1 = 20 * WF
    EXB2 = 18 * WF
    EXB3 = 16 * WF

    with tile.TileContext(nc) as tc:
        with tc.tile_pool(name="cst", bufs=1) as cst, \
             tc.tile_pool(name="gp", bufs=1) as gp, \
             tc.tile_pool(name="ps", bufs=2, space="PSUM") as psp:
            fw = cst.tile([128, 2 * 2 * 129], BF16)
            for kc in range(2):
                nc.sync.dma_start(fw[:, kc * 258:kc * 258 + 129],
                                  fwre[kc * 128:(kc + 1) * 128, :])
                nc.sync.dma_start(fw[:, kc * 258 + 129:kc * 258 + 258],
                                  fwim[kc * 128:(kc + 1) * 128, :])
            fhst = cst.tile([128, 2 * 3 * 38], BF16)
            nc.sync.dma_start(fhst[:], fhs[:])
            fhsv = fhst[:].rearrange("q (k m h) -> q k m h", k=2, m=3)
            w1t = cst.tile([16, 9 * 128], BF16)
            nc.sync.dma_start(w1t[:], gw1[:])
            w2t = cst.tile([128, 9 * 128], BF16)
            nc.sync.dma_start(w2t[:], gw2[:])
            w3t = cst.tile([128, 9 * 16], BF16)
            nc.sync.dma_start(w3t[:], gw3[:])
            b1t = cst.tile([128, 1], F32)
            nc.sync.dma_start(b1t[:], gb1[:])
            b2t = cst.tile([128, 1], F32)
            nc.sync.dma_start(b2t[:], gb2[:])
            b3t = cst.tile([16, 1], F32)
            nc.sync.dma_start(b3t[:], gb3[:])
            m20t = cst.tile([128, 20], F32)
            nc.sync.dma_start(m20t[:], mf20[:])
            m18t = cst.tile([128, 18], F32)
            nc.sync.dma_start(m18t[:], mf18[:])

            ft = gp.tile([16, GD + EXB + GD], BF16)
            dum = cst.tile([128, 512], BF16)
            nc.vector.memset(dum[:], 0.0)
            for wi in range(8):
                pw = psp.tile([128, 258], F32, tag="pv", name=f"warm{wi}",
                              bufs=2)
                nc.tensor.matmul(pw[:], lhsT=dum[:, 0:128],
                                 rhs=dum[:, 0:258], start=True, stop=True)
            nc.vector.memset(ft[:], 0.0)
            with tc.tile_pool(name="vp", bufs=3) as vp:
                for c in range(C):
                    xT = vp.tile([128, 2 * 256], BF16, tag="xT",
                                 name=f"xT{c}")
                    for kc in range(2):
                        nc.sync.dma_start(
                            xT[:, kc * 256:(kc + 1) * 256],
                            corrT[c, kc * 128:(kc + 1) * 128, :])
                    V = [vp.tile([128, 2 * 129], BF16, name=f"V{c}_{m}",
                                 tag=f"V{m}") for m in range(2)]
                    for mc in range(2):
                        pv = psp.tile([128, 258], F32, tag="pv",
                                      name=f"pv{c}_{mc}", bufs=2)
                        for kc in range(2):
                            nc.tensor.matmul(
                                pv[:],
                                lhsT=xT[:, kc * 256 + mc * 128:
                                        kc * 256 + (mc + 1) * 128],
                                rhs=fw[:, kc * 258:(kc + 1) * 258],
                                start=(kc == 0), stop=(kc == 1))
                        nc.scalar.activation(V[mc][:], pv[:], AF.Copy)
                    for ri in range(2):
                        py = psp.tile([38, 129], F32, tag="py",
                                      name=f"py{c}_{ri}", bufs=1)
                        t1, t2 = (0, 2) if ri == 0 else (1, 0)
                        ti = 0
                        for hc in range(2):
                            nc.tensor.matmul(
                                py[:], lhsT=fhsv[:, hc, t1, :],
                                rhs=V[hc][:, 0:129],
                                start=(ti == 0), stop=False)
                            ti += 1
                            nc.tensor.matmul(
                                py[:], lhsT=fhsv[:, hc, t2, :],
                                rhs=V[hc][:, 129:258],
                                start=False, stop=(ti == 3))
                            ti += 1
                        ys = vp.tile([38, 129], BF16, tag="ys",
                                     name=f"ys{c}_{ri}")
                        nc.scalar.activation(ys[:], py[:], AF.Copy)
                        for h in range(2):
                            p = (ri * 4 + c) * 2 + h
                            q = nc.sync if h == 0 else nc.gpsimd
                            q.dma_start(
                                mkap_s(ft[p:p + 1, :], GD + 1,
                                       [[GD + EXB + GD, 1], [WF, 22],
                                        [1, 129]]),
                                ys[h * 16:h * 16 + 22, :])

            g1 = gp.tile([128, GD + EXB1 + GD], BF16)
            g2 = gp.tile([128, GD + EXB2 + GD], BF16)
            w1v = w1t[:].rearrange("q (t c) -> q t c", t=9)
            for n0, nl in nchunks(EXB1, 512):
                ps = psp.tile([128, 512], F32, tag="cv", name=f"e1_{n0}",
                              bufs=3)
                for t, (dy, dx) in enumerate(TAPS):
                    off = GD + n0 + (1 + dy) * WF + dx
                    nc.tensor.matmul(ps[:, :nl], lhsT=w1v[:, t, :],
                                     rhs=ft[:, off:off + nl],
                                     start=(t == 0), stop=(t == 8))
                nc.scalar.activation(g1[:, GD + n0:GD + n0 + nl], ps[:, :nl],
                                     AF.Relu, bias=b1t[:])
            g1v = g1[:, GD:GD + EXB1].rearrange("q (r c) -> q r c", r=20)
            nc.vector.memset(g1v[:, :, 0:1], 0.0)
            nc.vector.memset(g1v[:, :, 130:131], 0.0)
            for r in (0, 1, 18, 19):
                nc.vector.tensor_scalar_mul(g1v[:, r, :], g1v[:, r, :],
                                            m20t[:, r:r + 1])
            w2v = w2t[:].rearrange("q (t c) -> q t c", t=9)
            for n0, nl in nchunks(EXB2, 512):
                ps = psp.tile([128, 512], F32, tag="cv", name=f"e2_{n0}",
                              bufs=3)
                for t, (dy, dx) in enumerate(TAPS):
                    off = GD + n0 + (1 + dy) * WF + dx
                    nc.tensor.matmul(ps[:, :nl], lhsT=w2v[:, t, :],
                                     rhs=g1[:, off:off + nl],
                                     start=(t == 0), stop=(t == 8))
                nc.scalar.activation(g2[:, GD + n0:GD + n0 + nl], ps[:, :nl],
                                     AF.Relu, bias=b2t[:])
            g2v = g2[:, GD:GD + EXB2].rearrange("q (r c) -> q r c", r=18)
            nc.vector.memset(g2v[:, :, 0:1], 0.0)
            nc.vector.memset(g2v[:, :, 130:131], 0.0)
            for r in (0, 17):
                nc.vector.tensor_scalar_mul(g2v[:, r, :], g2v[:, r, :],
                                            m18t[:, r:r + 1])
            g3 = gp.tile([16, EXB3], F32)
            w3v = w3t[:].rearrange("q (t c) -> q t c", t=9)
            for n0, nl in nchunks(EXB3, 512):
                ps = psp.tile([16, 512], F32, tag="cv3", name=f"e3_{n0}",
                              bufs=2)
                for t, (dy, dx) in enumerate(TAPS):
                    off = GD + n0 + (1 + dy) * WF + dx
                    nc.tensor.matmul(ps[:, :nl], lhsT=w3v[:, t, :],
                                     rhs=g2[:, off:off + nl],
                                     start=(t == 0), stop=(t == 8))
                nc.scalar.activation(g3[:, n0:n0 + nl], ps[:, :nl], AF.Copy)
            nc.vector.tensor_scalar(out=g3[:], in0=g3[:], scalar1=b3t[:],
                                    scalar2=None, op0=ALU.add)
            nc.sync.dma_start(
                mkap(cfo[:], 0, [[2064, 16], [129, 16], [1, 129]]),
                mkap_s(g3[:], 1, [[EXB3, 16], [WF, 16], [1, 129]]))
    return nc


def run_B(corr1, fw1, fb1, fw2, fb2, fw3, fb3, trace=False):
    if "B" not in _CACHE:
        _CACHE["B"] = patch_nc(build_B())
    fwre, fwim, _, _, _, _, _, _ = _dft_mats()
    corrTh = _bf(np.ascontiguousarray(corr1.transpose(0, 2, 1)))
    # block-diag weights: in p=(j)*2+h (j=ri*4+c), hid p=u*2+h, out p=j*2+h
    w1h = np.zeros((16, 9, 128), np.float32)
    w2h = np.zeros((128, 9, 128), np.float32)
    w3h = np.zeros((128, 9, 16), np.float32)
    b1h = np.zeros((128, 1), np.float32)
    b2h = np.zeros((128, 1), np.float32)
    b3h = np.zeros((16, 1), np.float32)
    for h in range(2):
        for u in range(64):
            b1h[u * 2 + h, 0] = fb1[u]
            b2h[u * 2 + h, 0] = fb2[u]
        for j in range(8):
            b3h[j * 2 + h, 0] = fb3[j]
    for t in range(9):
        dy, dx = t // 3, t % 3
        for h in range(2):
            for u in range(64):
                for j in range(8):
                    w1h[j * 2 + h, t, u * 2 + h] = fw1[u, j, dy, dx]
                    w3h[u * 2 + h, t, j * 2 + h] = fw3[j, u, dy, dx]
                for v in range(64):
                    w2h[v * 2 + h, t, u * 2 + h] = fw2[u, v, dy, dx]
    hhs = np.arange(256)
    ins = []
    for i in range(N_CORES):
        r0 = i * ROWS
        k1s = np.arange(r0 - 3, r0 + 35)
        ok = (k1s >= 0) & (k1s < 256)
        fhsh = np.zeros((128, 2, 3, 38), np.float32)
        for hc in range(2):
            h_ = np.arange(hc * 128, hc * 128 + 128)
            th = 2 * np.pi * np.outer(h_, k1s) / 256.0
            fhsh[:, hc, 0, :] = np.cos(th) / 16.0 * ok[None, :]
            fhsh[:, hc, 1, :] = -np.sin(th) / 16.0 * ok[None, :]
            fhsh[:, hc, 2, :] = np.sin(th) / 16.0 * ok[None, :]
        m20 = np.zeros((128, 20), np.float32)
        m18 = np.zeros((128, 18), np.float32)
        for p in range(128):
            h = p % 2
            base = r0 + h * 16
            for r in range(20):
                m20[p, r] = 1.0 if 0 <= base - 2 + r < 256 else 0.0
            for r in range(18):
                m18[p, r] = 1.0 if 0 <= base - 1 + r < 256 else 0.0
        ins.append({
            "corrT": corrTh, "fwre": fwre, "fwim": fwim,
            "fhs": _bf(fhsh.reshape(128, -1)),
            "gw1": _bf(w1h.reshape(16, -1)), "gb1": b1h,
            "gw2": _bf(w2h.reshape(128, -1)), "gb2": b2h,
            "gw3": _bf(w3h.reshape(128, -1)), "gb3": b3h,
            "mf20": m20, "mf18": m18,
        })
    res = run_bass_kernel_spmd(_CACHE["B"], ins, core_ids=list(range(N_CORES)),
                               trace=trace)
    cf = np.zeros((8, 256, 129), np.float32)
    for i in range(N_CORES):
        f = res.results[i]["cfo"].reshape(16, 16, 129)
        for j in range(8):
            for h in range(2):
                cf[j, i * ROWS + h * 16:i * ROWS + h * 16 + 16, :] = \
                    f[j * 2 + h]
    return None, cf, res


# ================================================================ kernel C
# inverse DFT from full CF (host-gathered) + per-channel refinement as
# 2-half row-packed block-diagonal convs (128 partitions, bf16).
# partition layouts: u/r3: p = c*2 + h; r1/r2: p = c*32 + u*2 + h.
WPC = 258
EXTC = 22 * WPC      # u half extent (22 rows)
EXTR1 = 20 * WPC
EXTR2 = 18 * WPC
EXTR3 = 16 * WPC


def build_C():
    nc = bass.Bass(trn_type="TRN2", name="kernC")
    u0 = nc.dram_tensor("u0", (8, EXTC), BF16, kind="ExternalInput")
    raws = nc.dram_tensor("raws", (8, EXTR3), F32, kind="ExternalInput")
    cfa = nc.dram_tensor("cfa", (128, 4 * 2 * 2 * 128), BF16,
                         kind="ExternalInput")
    cfb = nc.dram_tensor("cfb", (1, 4 * 2 * 2 * 128), BF16,
                         kind="ExternalInput")
    iwm = nc.dram_tensor("iwm", (128, 2 * 512), BF16, kind="ExternalInput")
    iwbm = nc.dram_tensor("iwbm", (1, 2 * 512), BF16, kind="ExternalInput")
    ihs = nc.dram_tensor("ihs", (128, 2 * 2 * 38), BF16,
                         kind="ExternalInput")
    cw1 = nc.dram_tensor("cw1", (8, 9 * 128), BF16, kind="ExternalInput")
    cb1 = nc.dram_tensor("cb1", (128, 1), F32, kind="ExternalInput")
    cw2 = nc.dram_tensor("cw2", (128, 9 * 128), BF16, kind="ExternalInput")
    cb2 = nc.dram_tensor("cb2", (128, 1), F32, kind="ExternalInput")
    cw3 = nc.dram_tensor("cw3", (128, 9 * 8), BF16, kind="ExternalInput")
    cb3 = nc.dram_tensor("cb3", (8, 1), F32, kind="ExternalInput")
    mr20 = nc.dram_tensor("mr20", (128, 20), F32, kind="ExternalInput")
    mr18 = nc.dram_tensor("mr18", (128, 18), F32, kind="ExternalInput")
    fin = nc.dram_tensor("fin", (8, 16 * 256), F32, kind="ExternalOutput")

    with tile.TileContext(nc) as tc:
        with tc.tile_pool(name="cst", bufs=1) as cst, \
             tc.tile_pool(name="gp", bufs=1) as gp, \
             tc.tile_pool(name="ps", bufs=1, space="PSUM") as psp:
            cfat = cst.tile([128, 4 * 2 * 2 * 128], BF16)
            nc.sync.dma_start(cfat[:], cfa[:])
            cfbt = cst.tile([1, 4 * 2 * 2 * 128], BF16)
            nc.sync.dma_start(cfbt[:], cfb[:])
            iwt = cst.tile([128, 2 * 512], BF16)
            nc.sync.dma_start(iwt[:], iwm[:])
            iwbt = cst.tile([1, 2 * 512], BF16)
            nc.sync.dma_start(iwbt[:], iwbm[:])
            ihst = cst.tile([128, 2 * 2 * 38], BF16)
            nc.sync.dma_start(ihst[:], ihs[:])
            w1t = cst.tile([8, 9 * 128], BF16)
            nc.sync.dma_start(w1t[:], cw1[:])
            w2t = cst.tile([128, 9 * 128], BF16)
            nc.sync.dma_start(w2t[:], cw2[:])
            w3t = cst.tile([128, 9 * 8], BF16)
            nc.sync.dma_start(w3t[:], cw3[:])
            b1t = cst.tile([128, 1], F32)
            nc.sync.dma_start(b1t[:], cb1[:])
            b2t = cst.tile([128, 1], F32)
            nc.sync.dma_start(b2t[:], cb2[:])
            b3t = cst.tile([8, 1], F32)
            nc.sync.dma_start(b3t[:], cb3[:])
            m20t = cst.tile([128, 20], F32)
            nc.sync.dma_start(m20t[:], mr20[:])
            m18t = cst.tile([128, 18], F32)
            nc.sync.dma_start(m18t[:], mr18[:])

            dum = cst.tile([128, 512], BF16)
            nc.vector.memset(dum[:], 0.0)
            for wi in range(8):
                pw = psp.tile([128, 512], F32, tag="pb", name=f"warm{wi}",
                              bufs=2)
                nc.tensor.matmul(pw[:], lhsT=dum[:, 0:128], rhs=dum[:],
                                 start=True, stop=True)
            cfav = cfat[:].rearrange("q (c r k m) -> q c r k m", c=4, r=2, k=2)
            cfbv = cfbt[:].rearrange("q (c r k m) -> q c r k m", c=4, r=2, k=2)
            ihsv = ihst[:].rearrange("q (k t h) -> q k t h", k=2, t=2)

            # u = u0 + z (inverse DFT), packed [8, GD + EXTC + GD]
            u = gp.tile([8, GD + EXTC + GD], BF16)
            nc.vector.memset(u[:, 0:GD], 0.0)
            nc.vector.memset(u[:, GD + EXTC:], 0.0)
            nc.sync.dma_start(u[:, GD:GD + EXTC], u0[:])
            zu = gp.tile([8, EXTC], BF16)
            nc.vector.memset(zu[:], 0.0)
            with tc.tile_pool(name="ip", bufs=2) as ip:
                for c in range(C):
                    # B[kc][k1, ri, w] = sum_k cf[c,k1,k] iw[k,w] (complex)
                    Bt = [ip.tile([128, 2 * 256], BF16, tag=f"Bt{kc}",
                                  name=f"Bt_{c}_{kc}") for kc in range(2)]
                    for kc in range(2):
                        pb = psp.tile([128, 512], F32, tag="pb",
                                      name=f"pb_{c}_{kc}", bufs=2)
                        nc.tensor.matmul(
                            pb[:], lhsT=cfav[:, c, 0, kc, :],
                            rhs=iwt[:, 0:512], start=True, stop=False)
                        nc.tensor.matmul(
                            pb[:], lhsT=cfbv[:, c, 0, kc, :],
                            rhs=iwbt[:, 0:512], start=False, stop=False)
                        nc.tensor.matmul(
                            pb[:], lhsT=cfav[:, c, 1, kc, :],
                            rhs=iwt[:, 512:1024], start=False, stop=False)
                        nc.tensor.matmul(
                            pb[:], lhsT=cfbv[:, c, 1, kc, :],
                            rhs=iwbt[:, 512:1024], start=False, stop=True)
                        nc.scalar.activation(Bt[kc][:], pb[:], AF.Copy)
                    # z[hh, w] = sum_k1 ih[k1, hh] B[k1, w] (re part)
                    pz = psp.tile([38, 256], F32, tag="pz",
                                  name=f"pz_{c}", bufs=1)
                    ti = 0
                    for kc in range(2):
                        for term in range(2):
                            nc.tensor.matmul(
                                pz[:], lhsT=ihsv[:, kc, term, :],
                                rhs=Bt[kc][:, term * 256:(term + 1) * 256],
                                start=(ti == 0), stop=(ti == 3))
                            ti += 1
                    zs = ip.tile([38, 256], BF16, tag="zs", name=f"zs_{c}")
                    nc.scalar.activation(zs[:], pz[:], AF.Copy)
                    for h in range(2):
                        zq = nc.sync if h == 0 else nc.gpsimd
                        zq.dma_start(
                            mkap_s(zu[c * 2 + h:c * 2 + h + 1, :], 1,
                                   [[EXTC, 1], [WPC, 22], [1, 256]]),
                            zs[h * 16:h * 16 + 22, :])
            UH = 11 * WPC
            nc.vector.tensor_tensor(out=u[:, GD:GD + UH],
                                    in0=u[:, GD:GD + UH], in1=zu[:, :UH],
                                    op=ALU.add)
            nc.vector.tensor_tensor(out=u[:, GD + UH:GD + EXTC],
                                    in0=u[:, GD + UH:GD + EXTC],
                                    in1=zu[:, UH:], op=ALU.add)

            r1 = gp.tile([128, GD + EXTR1 + GD], BF16)
            r2 = gp.tile([128, GD + EXTR2 + GD], BF16)
            for n0, nl in nchunks(EXTR1, 512):
                ps = psp.tile([128, 512], F32, tag="cv", name=f"d1_{n0}",
                              bufs=3)
                for t, (dy, dx) in enumerate(TAPS):
                    off = GD + n0 + (1 + dy) * WPC + dx
                    nc.tensor.matmul(ps[:, :nl],
                                     lhsT=w1t[:].rearrange(
                                         "q (t c) -> q t c", t=9)[:, t, :],
                                     rhs=u[:, off:off + nl],
                                     start=(t == 0), stop=(t == 8))
                nc.scalar.activation(r1[:, GD + n0:GD + n0 + nl], ps[:, :nl],
                                     AF.Relu, bias=b1t[:])
            r1v = r1[:, GD:GD + EXTR1].rearrange("q (r c) -> q r c", r=20)
            nc.vector.memset(r1v[:, :, 0:1], 0.0)
            nc.vector.memset(r1v[:, :, 257:258], 0.0)
            for r in (0, 1, 18, 19):
                nc.vector.tensor_scalar_mul(r1v[:, r, :], r1v[:, r, :],
                                            m20t[:, r:r + 1])
            for n0, nl in nchunks(EXTR2, 512):
                ps = psp.tile([128, 512], F32, tag="cv", name=f"d2_{n0}",
                              bufs=3)
                for t, (dy, dx) in enumerate(TAPS):
                    off = GD + n0 + (1 + dy) * WPC + dx
                    nc.tensor.matmul(ps[:, :nl],
                                     lhsT=w2t[:].rearrange(
                                         "q (t c) -> q t c", t=9)[:, t, :],
                                     rhs=r1[:, off:off + nl],
                                     start=(t == 0), stop=(t == 8))
                nc.scalar.activation(r2[:, GD + n0:GD + n0 + nl], ps[:, :nl],
                                     AF.Relu, bias=b2t[:])
            r2v = r2[:, GD:GD + EXTR2].rearrange("q (r c) -> q r c", r=18)
            nc.vector.memset(r2v[:, :, 0:1], 0.0)
            nc.vector.memset(r2v[:, :, 257:258], 0.0)
            for r in (0, 17):
                nc.vector.tensor_scalar_mul(r2v[:, r, :], r2v[:, r, :],
                                            m18t[:, r:r + 1])
            r3 = gp.tile([8, EXTR3], F32)
            rawt = gp.tile([8, EXTR3], F32)
            nc.sync.dma_start(rawt[:], raws[:])
            for n0, nl in nchunks(EXTR3, 512):
                ps = psp.tile([8, 512], F32, tag="cv3", name=f"d3_{n0}",
                              bufs=2)
                for t, (dy, dx) in enumerate(TAPS):
                    off = GD + n0 + (1 + dy) * WPC + dx
                    nc.tensor.matmul(ps[:, :nl],
                                     lhsT=w3t[:].rearrange(
                                         "q (t c) -> q t c", t=9)[:, t, :],
                                     rhs=r2[:, off:off + nl],
                                     start=(t == 0), stop=(t == 8))
                nc.vector.tensor_tensor(out=r3[:, n0:n0 + nl],
                                        in0=ps[:, :nl],
                                        in1=rawt[:, n0:n0 + nl], op=ALU.add)
                nc.vector.tensor_scalar(out=r3[:, n0:n0 + nl],
                                        in0=r3[:, n0:n0 + nl], scalar1=0.0,
                                        scalar2=1.0, op0=ALU.max,
                                        op1=ALU.min)
            nc.sync.dma_start(
                fin[:, :], mkap_s(r3[:], 1, [[EXTR3, 8], [WPC, 16],
                                             [1, 256]]))
    return nc


def build_C_old():
    nc = bass.Bass(trn_type="TRN2", name="kernC")
    u = nc.dram_tensor("u", (C, 38 * WP), BF16, kind="ExternalInput")
    raw32 = nc.dram_tensor("raw32", (C, ROWS * W), F32, kind="ExternalInput")
    cw1 = nc.dram_tensor("cw1", (C, 9 * 64), BF16, kind="ExternalInput")
    cb1 = nc.dram_tensor("cb1", (64, 1), F32, kind="ExternalInput")
    cw2 = nc.dram_tensor("cw2", (64, 9 * 64), BF16, kind="ExternalInput")
    cb2 = nc.dram_tensor("cb2", (64, 1), F32, kind="ExternalInput")
    cw3 = nc.dram_tensor("cw3", (64, 9 * 4), BF16, kind="ExternalInput")
    cb3 = nc.dram_tensor("cb3", (4, 1), F32, kind="ExternalInput")
    mr36 = nc.dram_tensor("mr36", (64, 36), F32, kind="ExternalInput")
    mr34 = nc.dram_tensor("mr34", (64, 34), F32, kind="ExternalInput")
    fin = nc.dram_tensor("fin", (C, ROWS, W), F32, kind="ExternalOutput")

    N36, N34, N32 = 36 * WP, 34 * WP, 32 * WP

    def conv_taps_outer(pool_ps, lhsw, rhsrc, dstact, bias, Ntot, Kp, Mp, relu,
                        group=1):
        """taps-outer grouped conv: lhsw(t)->lhsT AP, rhsrc(t, n0, nl)->rhs AP,
        dstact(n0, nl, psum) consumes."""
        chunks = nchunks(Ntot, 512)
        for g0 in range(0, len(chunks), group):
            grp = chunks[g0:g0 + group]
            pss = [pool_ps.tile([Mp, 512], F32, tag=f"cg{j}", name=f"cg_{g0}_{j}",
                                bufs=6) for j in range(len(grp))]
            for t in range(9):
                for j, (n0, nl) in enumerate(grp):
                    nc.tensor.matmul(pss[j][:, :nl], lhsT=lhsw(t),
                                     rhs=rhsrc(t, n0, nl),
                                     start=(t == 0), stop=(t == 8))
            for j, (n0, nl) in enumerate(grp):
                dstact(n0, nl, pss[j])

    with tile.TileContext(nc) as tc:
        with tc.tile_pool(name="cst", bufs=1) as cst, \
             tc.tile_pool(name="gp", bufs=1) as gp, \
             tc.tile_pool(name="ps", bufs=1, space="PSUM") as psp:
            w1t = cst.tile([C, 9 * 64], BF16)
            nc.sync.dma_start(w1t[:], cw1[:])
            w2t = cst.tile([64, 9 * 64], BF16)
            nc.sync.dma_start(w2t[:], cw2[:])
            w3t = cst.tile([64, 9 * 4], BF16)
            nc.sync.dma_start(w3t[:], cw3[:])
            b1t = cst.tile([64, 1], F32)
            nc.sync.dma_start(b1t[:], cb1[:])
            b2t = cst.tile([64, 1], F32)
            nc.sync.dma_start(b2t[:], cb2[:])
            b3t = cst.tile([C, 1], F32)
            nc.sync.dma_start(b3t[:], cb3[:])
            m36t = cst.tile([64, 36], F32)
            nc.sync.dma_start(m36t[:], mr36[:])
            m34t = cst.tile([64, 34], F32)
            nc.sync.dma_start(m34t[:], mr34[:])

            ut = gp.tile([C, 1 + 38 * WP + 4], BF16)
            nc.sync.dma_start(ut[:, 1:1 + 38 * WP], u[:])
            r1 = gp.tile([64, 1 + N36 + 4], BF16)
            r2 = gp.tile([64, 1 + N34 + 4], BF16)

            conv_taps_outer(
                psp,
                lambda t: w1t[:, t * 64:(t + 1) * 64],
                lambda t, n0, nl: ut[:, 1 + n0 + (1 + TAPS[t][0]) * WP + TAPS[t][1]:
                                     1 + n0 + (1 + TAPS[t][0]) * WP + TAPS[t][1] + nl],
                lambda n0, nl, ps: nc.scalar.activation(
                    r1[:, 1 + n0:1 + n0 + nl], ps[:, :nl], AF.Relu, bias=b1t[:]),
                b1t, N36, 64, 64, True)
            r1v = r1[:, 1:1 + N36].rearrange("p (r q) -> p r q", r=36)
            nc.vector.memset(r1v[:, :, 0:1], 0.0)
            nc.vector.memset(r1v[:, :, 257:258], 0.0)
            for r in (0, 1, 34, 35):
                nc.vector.tensor_scalar_mul(r1v[:, r, :], r1v[:, r, :],
                                            m36t[:, r:r + 1])

            conv_taps_outer(
                psp,
                lambda t: w2t[:, t * 64:(t + 1) * 64],
                lambda t, n0, nl: r1[:, 1 + n0 + (1 + TAPS[t][0]) * WP + TAPS[t][1]:
                                     1 + n0 + (1 + TAPS[t][0]) * WP + TAPS[t][1] + nl],
                lambda n0, nl, ps: nc.scalar.activation(
                    r2[:, 1 + n0:1 + n0 + nl], ps[:, :nl], AF.Relu, bias=b2t[:]),
                b2t, N34, 64, 64, True)
            r2v = r2[:, 1:1 + N34].rearrange("p (r q) -> p r q", r=34)
            nc.vector.memset(r2v[:, :, 0:1], 0.0)
            nc.vector.memset(r2v[:, :, 257:258], 0.0)
            for r in (0, 33):
                nc.vector.tensor_scalar_mul(r2v[:, r, :], r2v[:, r, :],
                                            m34t[:, r:r + 1])

            with tc.tile_pool(name="fo", bufs=1) as fo:
                rawt = fo.tile([C, ROWS * W], F32)
                nc.sync.dma_start(rawt[:], raw32[:])
                r3 = fo.tile([C, N32], F32)
                conv_taps_outer(
                    psp,
                    lambda t: w3t[:, t * 4:(t + 1) * 4],
                    lambda t, n0, nl: r2[:, 1 + n0 + (1 + TAPS[t][0]) * WP + TAPS[t][1]:
                                         1 + n0 + (1 + TAPS[t][0]) * WP + TAPS[t][1] + nl],
                    lambda n0, nl, ps: nc.scalar.activation(
                        r3[:, n0:n0 + nl], ps[:, :nl], AF.Copy),
                    b3t, N32, 64, C, False)
                r3v = r3[:].rearrange("p (r q) -> p r q", r=32)[:, :, 1:257]
                rv = rawt[:].rearrange("p (r q) -> p r q", r=32)
                nc.vector.tensor_scalar(out=r3v, in0=r3v, scalar1=b3t[:],
                                        scalar2=None, op0=ALU.add)
                nc.vector.tensor_tensor(out=r3v, in0=r3v, in1=rv, op=ALU.add)
                nc.vector.tensor_scalar(out=r3v, in0=r3v, scalar1=0.0,
                                        scalar2=1.0, op0=ALU.max, op1=ALU.min)
                nc.sync.dma_start(fin[:, :, :], r3v)
    return nc


_CACHE = {}


def _f8(x):
    return np.asarray(x, dtype=np.float32).astype(ml_dtypes.float8_e4m3)


def _prep_A(raw, feat, pw1, pb1, pw2, pb2, pw3, pb3):
    # weights packed for DoubleRow passes (see PAIRS)
    def tap_w(pw, dydx):
        dy, dx = dydx
        return pw[:, :, dy + 1, dx + 1]  # [co, ci]

    # w1: [ci, m, p, kt, co128]
    w1h = np.zeros((128, 2, 5, 2, 128), np.float32)
    for m in range(2):
        for p in range(5):
            t0, t1 = pair_taps(p)
            w1h[:, m, p, 0, :] = tap_w(pw1, t0).T[:, m * 128:(m + 1) * 128]
            if t1 is not None:
                w1h[:, m, p, 1, :] = tap_w(pw1, t1).T[:, m * 128:(m + 1) * 128]
    # w2: [cip, t, kc, co]
    w2h = np.zeros((128, 9, 2, 128), np.float32)
    for t, (dy, dx) in enumerate(TAPS):
        wt = tap_w(pw2, (dy, dx))  # [128 co, 256 ci]
        for kc in range(2):
            w2h[:, t, kc, :] = wt[:, kc * 128:(kc + 1) * 128].T
    # w3: [ci, p, kt, 912] (col = c*228 + tpsf)
    w3h = np.zeros((128, 5, 2, 912), np.float32)
    for p in range(5):
        t0, t1 = pair_taps(p)
        for kt, tt in ((0, t0), (1, t1)):
            if tt is None:
                continue
            wt = tap_w(pw3, tt)  # [900, 128]
            for c in range(C):
                w3h[:, p, kt, c * 228:c * 228 + 225] = \
                    wt[c * 225:(c + 1) * 225].T
    b1h = np.ascontiguousarray(pb1.reshape(2, 128).T).astype(np.float32)
    b2h = pb2.reshape(128, 1).astype(np.float32)
    b3row = np.full((912,), -30.0, np.float32)
    for c in range(C):
        b3row[c * 228:c * 228 + 225] = pb3[c * 225:(c + 1) * 225]
    w3h[0, 4, 1, :] = b3row

    xpad = np.pad(raw, ((0, 0), (PAD, PAD), (PAD, PAD)), mode="reflect")
    # unfolded patches [4, 256, 256, 15, 15]
    sw = np.lib.stride_tricks.sliding_window_view(xpad, (15, 15),
                                                  axis=(1, 2))
    featp = np.pad(feat, ((0, 0), (3, 3), (0, 0)))

    ins = []
    for i in range(N_CORES):
        r0 = i * ROWS
        m36 = np.array([1.0 if 0 <= r0 - 2 + r < H else 0.0
                        for r in range(36)], np.float32)
        m34 = np.array([1.0 if 0 <= r0 - 1 + r < H else 0.0
                        for r in range(34)], np.float32)
        fbA = np.zeros((128, 38, RP), np.float32)
        fbA[:, :, 1:257] = featp[:, r0:r0 + 38, :]
        fbA = fbA.reshape(128, EXTF)
        fbh = np.zeros((128, GD + 2 * EXTF), np.float32)
        fbh[:, GD:GD + EXTF] = fbA
        fbh[:, GD + EXTF:GD + 2 * EXTF - 1] = fbA[:, 1:]
        # Xu: [8192 pix, 912] = (r, x) -> [c*228 + tpsf]; bias comes via
        # the psum ones-matmul, so patches stay unscaled
        slab = sw[:, r0:r0 + ROWS, :, :, :]  # [4, 32, 256, 15, 15]
        xuh = np.zeros((ROWS * W, 4, 228), np.float32)
        xuh[:, :, :225] = slab.reshape(4, ROWS * W, 225).transpose(1, 0, 2)
        xuh = xuh.reshape(ROWS * W, 912)
        ins.append({
            "fb": _f8(fbh),
            "w1": _f8(w1h.reshape(128, -1)), "b1": b1h,
            "w2": _f8(w2h.reshape(128, -1)), "b2": b2h,
            "w3": _f8(w3h.reshape(128, -1)),
            "xu": _bf(xuh),
            "m36": np.ascontiguousarray(np.broadcast_to(m36, (128, 36))),
            "m34": np.ascontiguousarray(np.broadcast_to(m34, (128, 34))),
        })
    return ins


def run_A(raw, feat, pw1, pb1, pw2, pb2, pw3, pb3, trace=False):
    if "A" not in _CACHE:
        _CACHE["A"] = patch_nc(build_A())
    ins = _prep_A(raw, feat, pw1, pb1, pw2, pb2, pw3, pb3)
    res = run_bass_kernel_spmd(_CACHE["A"], ins, core_ids=list(range(N_CORES)),
                               trace=trace)
    corr = np.concatenate(
        [res.results[i]["corr"].reshape(ROWS, 2, 4, 128)
         .transpose(2, 0, 1, 3).reshape(C, ROWS, W)
         for i in range(N_CORES)], axis=1)
    return corr, res


def _dft_mats():
    k = np.arange(129)
    w = np.arange(256)
    th = 2 * np.pi * np.outer(w, k) / 256.0          # [256, 129]
    fwre = _bf(np.cos(th) / 16.0)
    fwim = _bf(-np.sin(th) / 16.0)
    h = np.arange(256)
    k1 = np.arange(256)
    th2 = 2 * np.pi * np.outer(h, k1) / 256.0        # [256h, 256k1]
    fhre = _bf(np.cos(th2) / 16.0)
    fhim = _bf(-np.sin(th2) / 16.0)
    fhimn = _bf(np.sin(th2) / 16.0)
    ck = np.where((k == 0) | (k == 128), 1.0, 2.0)
    th3 = 2 * np.pi * np.outer(k, w) / 256.0         # [129k, 256w]
    iwre = _bf(ck[:, None] * np.cos(th3) / 16.0)
    iwim = _bf(ck[:, None] * np.sin(th3) / 16.0)
    iwimn = _bf(-ck[:, None] * np.sin(th3) / 16.0)
    return fwre, fwim, fhre, fhim, fhimn, iwre, iwim, iwimn


def run_B1(corr1, trace=False):
    if "B1" not in _CACHE:
        _CACHE["B1"] = patch_nc(build_B1())
    fwre, fwim, fhre, fhim, fhimn, _, _, _ = _dft_mats()
    corrT = _bf(np.ascontiguousarray(corr1.transpose(0, 2, 1)))
    inm = {"corrT": corrT, "fwre": fwre, "fwim": fwim,
           "fhre": fhre, "fhim": fhim, "fhimn": fhimn}
    res = run_bass_kernel_spmd(_CACHE["B1"], [inm] * N_CORES,
                               core_ids=list(range(N_CORES)), trace=trace)
    return res.results[0]["fri"], res


def run_B2(fri_full, fw1, fb1, fw2, fb2, fw3, fb3, trace=False):
    from einops import rearrange as rr
    if "B2" not in _CACHE:
        _CACHE["B2"] = patch_nc(build_B2())
    gw1 = _bf(rr(fw1, "co ci dy dx -> ci (dy dx co)"))
    gw2 = _bf(rr(fw2, "co ci dy dx -> ci (dy dx co)"))
    gw3 = _bf(rr(fw3, "co ci dy dx -> ci (dy dx co)"))
    gb1 = fb1.reshape(64, 1).astype(np.float32)
    gb2 = fb2.reshape(64, 1).astype(np.float32)
    gb3 = fb3.reshape(8, 1).astype(np.float32)
    ins = []
    for i in range(N_CORES):
        r0 = i * ROWS
        slab = np.zeros((8, 38, WF), np.float32)
        lo, hi = max(0, r0 - 3), min(256, r0 + 35)
        slab[:, lo - (r0 - 3):hi - (r0 - 3), 1:130] = fri_full[:, lo:hi, :]
        m36 = np.array([1.0 if 0 <= r0 - 2 + r < 256 else 0.0
                        for r in range(36)], np.float32)
        m34 = np.array([1.0 if 0 <= r0 - 1 + r < 256 else 0.0
                        for r in range(34)], np.float32)
        ins.append({
            "fri": _bf(slab.reshape(8, 38 * WF)),
            "gw1": gw1, "gb1": gb1, "gw2": gw2, "gb2": gb2,
            "gw3": gw3, "gb3": gb3,
            "mf36": np.ascontiguousarray(np.broadcast_to(m36, (64, 36))),
            "mf34": np.ascontiguousarray(np.broadcast_to(m34, (64, 34))),
        })
    res = run_bass_kernel_spmd(_CACHE["B2"], ins, core_ids=list(range(N_CORES)),
                               trace=trace)
    cf = np.concatenate([res.results[i]["cfo"].reshape(8, 32, 129)
                         for i in range(N_CORES)], axis=1)
    return cf, res


def run_C(corr1, cf, raw, cw1, cb1, cw2, cb2, cw3, cb3, trace=False):
    if "C" not in _CACHE:
        _CACHE["C"] = patch_nc(build_C())
    # block-diag weights, layouts: in p=c*2+h, hid p=c*32+u*2+h, out p=c*2+h
    w1h = np.zeros((8, 9, 128), np.float32)
    w2h = np.zeros((128, 9, 128), np.float32)
    w3h = np.zeros((128, 9, 8), np.float32)
    b1h = np.zeros((128, 1), np.float32)
    b2h = np.zeros((128, 1), np.float32)
    b3h = np.zeros((8, 1), np.float32)
    for c in range(C):
        for h in range(2):
            b3h[c * 2 + h, 0] = cb3[c, 0]
            for uu in range(16):
                b1h[c * 32 + uu * 2 + h, 0] = cb1[c, uu]
                b2h[c * 32 + uu * 2 + h, 0] = cb2[c, uu]
    for t, (dy, dx) in enumerate([(a, b) for a in range(3) for b in range(3)]):
        for c in range(C):
            for h in range(2):
                for uu in range(16):
                    w1h[c * 2 + h, t, c * 32 + uu * 2 + h] = \
                        cw1[c, uu, 0, dy, dx]
                    w3h[c * 32 + uu * 2 + h, t, c * 2 + h] = \
                        cw3[c, 0, uu, dy, dx]
                    for v in range(16):
                        w2h[c * 32 + v * 2 + h, t, c * 32 + uu * 2 + h] = \
                            cw2[c, uu, v, dy, dx]
    # inverse DFT constants (same for all cores except ihs)
    kk = np.arange(129)
    w_ = np.arange(256)
    ck = np.where((kk == 0) | (kk == 128), 1.0, 2.0)
    th3 = 2 * np.pi * np.outer(kk, w_) / 256.0
    iwre = ck[:, None] * np.cos(th3) / 16.0
    iwim = ck[:, None] * np.sin(th3) / 16.0
    iwh = np.zeros((128, 2 * 512), np.float32)
    iwbh = np.zeros((1, 2 * 512), np.float32)
    for j, m in enumerate((iwre, iwim, -iwim, iwre)):
        iwh[:, j * 256:(j + 1) * 256] = m[:128]
        iwbh[0, j * 256:(j + 1) * 256] = m[128]
    # cfa [128 k, (c, ri, kc, 128 k1)], cfb k=128 row
    cfah = np.zeros((128, 4, 2, 2, 128), np.float32)
    cfbh = np.zeros((1, 4, 2, 2, 128), np.float32)
    for c in range(C):
        for ri in range(2):
            m = cf[ri * 4 + c]  # [256 k1, 129 k]
            for kc in range(2):
                cfah[:, c, ri, kc, :] = m[kc * 128:(kc + 1) * 128, :128].T
                cfbh[0, c, ri, kc, :] = m[kc * 128:(kc + 1) * 128, 128]
    ins = []
    for i in range(N_CORES):
        r0 = i * ROWS
        u0h = np.zeros((8, 22, WPC), np.float32)
        rawh = np.zeros((8, 16, WPC), np.float32)
        ihsh = np.zeros((128, 2, 2, 38), np.float32)
        hh = np.arange(r0 - 3, r0 + 35)
        ok = (hh >= 0) & (hh < 256)
        for kc in range(2):
            k1 = np.arange(kc * 128, kc * 128 + 128)
            th = 2 * np.pi * np.outer(k1, hh) / 256.0
            ihsh[:, kc, 0, :] = np.cos(th) / 16.0 * ok[None, :]
            ihsh[:, kc, 1, :] = -np.sin(th) / 16.0 * ok[None, :]
        for c in range(C):
            for h in range(2):
                lo = r0 + h * 16 - 3
                a, b = max(0, lo), min(256, lo + 22)
                u0h[c * 2 + h, a - lo:b - lo, 1:257] = corr1[c, a:b, :]
                rawh[c * 2 + h, :, 1:257] = \
                    raw[c, r0 + h * 16:r0 + h * 16 + 16, :] + cb3[c, 0]
        m20 = np.zeros((128, 20), np.float32)
        m18 = np.zeros((128, 18), np.float32)
        for p in range(128):
            h = p % 2
            base = r0 + h * 16
            for r in range(20):
                m20[p, r] = 1.0 if 0 <= base - 2 + r < 256 else 0.0
            for r in range(18):
                m18[p, r] = 1.0 if 0 <= base - 1 + r < 256 else 0.0
        ins.append({
            "u0": _bf(u0h.reshape(8, EXTC)),
            "raws": rawh.reshape(8, EXTR3).astype(np.float32),
            "cfa": _bf(cfah.reshape(128, -1)),
            "cfb": _bf(cfbh.reshape(1, -1)),
            "iwm": _bf(iwh), "iwbm": _bf(iwbh),
            "ihs": _bf(ihsh.reshape(128, -1)),
            "cw1": _bf(w1h.reshape(8, -1)), "cb1": b1h,
            "cw2": _bf(w2h.reshape(128, -1)), "cb2": b2h,
            "cw3": _bf(w3h.reshape(128, -1)), "cb3": b3h,
            "mr20": m20, "mr18": m18,
        })
    res = run_bass_kernel_spmd(_CACHE["C"], ins, core_ids=list(range(N_CORES)),
                               trace=trace)
    fin = np.zeros((C, H, W), np.float32)
    for i in range(N_CORES):
        f = res.results[i]["fin"].reshape(8, 16, 256)
        for c in range(C):
            for h in range(2):
                fin[c, i * ROWS + h * 16:i * ROWS + h * 16 + 16, :] = \
                    f[c * 2 + h]
    return fin, res


def kernel(**inputs):
    inputs = {k: np.asarray(v, dtype=np.float32) for k, v in inputs.items()}
    raw = inputs["raw_image"][0]
    feat = inputs["aberration_features"][0]
    corr1, _ = run_A(raw, feat,
                     inputs["pw1"], inputs["pb1"], inputs["pw2"], inputs["pb2"],
                     inputs["pw3"], inputs["pb3"])
    _, cf, _ = run_B(corr1, inputs["fw1"], inputs["fb1"], inputs["fw2"],
                     inputs["fb2"], inputs["fw3"], inputs["fb3"])
    fin, _ = run_C(corr1, cf, raw, inputs["cw1"], inputs["cb1"],
                   inputs["cw2"], inputs["cb2"], inputs["cw3"],
                   inputs["cb3"])
    return fin[None].astype(np.float32)


